# revision 1
# baseline (speedup 1.0000x reference)
"""Trainium2 Bass kernel for nn_MoEMLABlock (MoE + multi-level attention block).

Strategy (8 NeuronCores, full inputs in / full output out):
  Launch A (attention, sharded over batch x level x head-half): core
    c = (b, l, hh) computes, for batch b, level l, heads hh*8..hh*8+7:
    Q/K/V projections over all 1024 tokens, softmax attention, and the
    partial O-projection [H, S] (feature-major).  No K/V recompute across
    cores.  LayerNorm 1 runs on the host (fp64) with gamma/beta folded
    into the projection weights; 1/sqrt(DH), the softmax level weights,
    and all biases are folded on the host.  Q/K biases enter the
    projection matmul as an extra ones-row contraction term; V bias and
    the O bias fold into a single per-batch constant added on the host.
    The softmax denominator is produced by the context matmul itself via
    a ones-column appended to V (psum row 64 = sumexp).  All device
    tensors arrive pre-laid-out in SBUF tile order so every DMA is one
    descriptor per partition.
  Host: sum the 4 partials per batch (+ residual + folded bias), LN2,
    router logits/softmax/top-2 (fp64), per-expert token gather.
  Launch B (expert-parallel): core e runs expert e's FFN
    gelu(x@W1+b1)@W2+b2 in bf16 (fp32 psum), gate-scaled on device, over
    its routed tokens, feature-major in and out (no device transposes).
  Host: scatter-add combine + residual.
"""

import numpy as np

H = 1024
NH = 16
DH = 64
L = 2
E = 8
FF = 4096
B = 2
S = 1024
EPS = 1e-5
P = 128
NCORES = 8
KO = H // P              # 8 contraction chunks over H
FB = 4                   # feature blocks of 128 (= head pairs) per core
QC = 2                   # query chunks of 512
KT = 8                   # key tiles of 128
MF = FF // P             # 32

_CACHE = {}
_PERF = {}


def _build_attn(wb=True):
    """Launch A program: one (batch, level, head-half) attention slice.
    wb=False elides the Q/K bias ones-row matmuls (all cores' folded
    biases are exactly zero for this input, decided by the host)."""
    import concourse.bacc as bacc
    import concourse.mybir as mybir
    import concourse.tile as tile

    F32, F32R = mybir.dt.float32, mybir.dt.float32r
    AF = mybir.ActivationFunctionType

    nc = bacc.Bacc()
    xn_h = nc.dram_tensor("xn", [P, KO, S], F32, kind="ExternalInput")   # LN1(x_b)^T tiled
    wq_h = nc.dram_tensor("wq", [FB, P, KO, P], F32, kind="ExternalInput")
    wk_h = nc.dram_tensor("wk", [FB, P, KO, P], F32, kind="ExternalInput")
    wv_h = nc.dram_tensor("wv", [P, KO, 512], F32, kind="ExternalInput")
    wo_h = nc.dram_tensor("wo", [P, FB, H], F32, kind="ExternalInput")
    bqk_h = nc.dram_tensor("bqk", [1, 1024], F32, kind="ExternalInput")  # bq | bk rows
    mb_h = nc.dram_tensor("mb", [P, KT], F32, kind="ExternalInput")      # key mask bias cols
    out_h = nc.dram_tensor("attnp", [H, S], F32, kind="ExternalOutput")

    with tile.TileContext(nc) as tc:
        with tc.tile_pool(name="consts", bufs=1) as consts, \
             tc.tile_pool(name="big", bufs=1) as big, \
             tc.tile_pool(name="wqk_s", bufs=2) as wqk_s, \
             tc.tile_pool(name="work", bufs=3) as work, \
             tc.tile_pool(name="outp", bufs=4) as outp, \
             tc.tile_pool(name="ps_mm", bufs=2, space="PSUM") as ps_mm, \
             tc.tile_pool(name="ps_sc", bufs=2, space="PSUM") as ps_sc, \
             tc.tile_pool(name="ps_cx", bufs=4, space="PSUM") as ps_cx:

            ones_f = consts.tile([1, 512], F32)
            nc.vector.memset(ones_f[:], 1.0)
            ones_row = consts.tile([1, 512], F32R)
            nc.vector.tensor_copy(ones_row[:], ones_f[:])

            bqk_sb = consts.tile([1, 1024], F32R)
            if wb:
                nc.sync.dma_start(bqk_sb[:], bqk_h[:].bitcast(F32R))
            mb_sb = consts.tile([P, KT], F32)
            nc.sync.dma_start(mb_sb[:], mb_h[:])

            # inputs, pre-tiled on the host: 1 descriptor per partition.
            # DMA issue order = first-use order (transfers share HBM bw):
            # first query-token half of xn, then wq0/wk0 so the head-pair-0
            # projections start ~10us in, with wv/xnB streaming behind.
            xn_t = big.tile([P, KO, S], F32R)
            nc.sync.dma_start(xn_t[:, :, 0:512], xn_h[:, :, 0:512].bitcast(F32R))
            wv_sb = big.tile([P, KO, 512], F32R)
            v_t = big.tile([P, KT, 8 * 65], F32R)       # per head: 64 cols V + 1 col ones

            # ---- interleaved per-head-pair: Q/K projection then attention ----
            # PE stays busy on the next pair's projections while the Act
            # engine works through this pair's exps; the normalize of block i
            # is emitted during block i+1 so its reciprocal never stalls PE.
            q_t = big.tile([P, FB, S], F32R)
            k_t = big.tile([P, FB, S], F32R)
            ctx_t = big.tile([P, FB, S], F32R)

            def proj_dma(w_h, fb, tag):
                w_fb = wqk_s.tile([P, KO, P], F32R, tag=tag, name=f"w_{tag}{fb}")
                nc.sync.dma_start(w_fb[:], w_h[fb].bitcast(F32R))
                return w_fb

            def proj_steps(dst, w_fb, bias_off, fb, qc):
                """One projection psum group as single-instruction steps, so
                it can be sprinkled into Act-bound attention sections."""
                box = {}

                def step(kc):
                    if kc == 0:
                        box["t"] = ps_mm.tile([P, 512], F32, tag="mm",
                                              name=f"qps{fb}_{qc}")
                    if kc < KO:
                        nc.tensor.matmul(
                            box["t"][:], w_fb[:, kc, :],
                            xn_t[:, kc, qc * 512:(qc + 1) * 512],
                            start=(kc == 0), stop=(kc == KO - 1 and not wb),
                        )
                    elif kc == KO and wb:
                        nc.tensor.matmul(
                            box["t"][:],
                            bqk_sb[:, bias_off + fb * P:bias_off + (fb + 1) * P],
                            ones_row[:], start=False, stop=True,
                        )
                    else:
                        nc.vector.tensor_copy(
                            dst[:, fb, qc * 512:(qc + 1) * 512], box["t"][:])

                ks = list(range(KO)) + ([KO] if wb else []) + [KO + 1]
                return [lambda k=k: step(k) for k in ks]

            def proj_fb(dst, w_h, bias_off, fb, tag):
                w_fb = proj_dma(w_h, fb, tag)
                for qc in range(QC):
                    for st in proj_steps(dst, w_fb, bias_off, fb, qc):
                        st()

            def normalize(fb, qc, cx):
                # 1/sumexp (psum row 64) broadcast to 64 partitions on the
                # otherwise-idle Pool engine, then scale ctx on DVE.
                for hh in range(2):
                    rcp = work.tile([1, 512], F32, tag="rcp")
                    nc.vector.reciprocal(rcp[:], cx[hh][64:65, :])
                    rb_sb = work.tile([64, 512], F32, tag="rb_sb")
                    nc.gpsimd.partition_broadcast(rb_sb[:], rcp[:])
                    nc.vector.tensor_mul(
                        ctx_t[hh * DH:(hh + 1) * DH, fb, qc * 512:(qc + 1) * 512],
                        cx[hh][0:64, :], rb_sb[:],
                    )

            wo_sb = big.tile([P, FB, H], F32R)

            def o_steps(ob, qc):
                # one O-projection psum group as steps (4 matmuls, copy, DMA)
                box = {}

                def step(i):
                    if i == 0:
                        box["t"] = ps_mm.tile([P, 512], F32, tag="mm",
                                              name=f"ops{ob}_{qc}")
                    if i < FB:
                        nc.tensor.matmul(
                            box["t"][:], wo_sb[:, i, ob * P:(ob + 1) * P],
                            ctx_t[:, i, qc * 512:(qc + 1) * 512],
                            start=(i == 0), stop=(i == FB - 1),
                        )
                    elif i == FB:
                        box["o"] = outp.tile([P, 512], F32, tag="o",
                                             name=f"oh{ob}_{qc}")
                        nc.vector.tensor_copy(box["o"][:], box["t"][:])
                    else:
                        nc.sync.dma_start(
                            out_h[:].rearrange("(ko p) t -> p ko t", p=P)[
                                :, ob, qc * 512:(qc + 1) * 512],
                            box["o"][:],
                        )

                return [lambda i=i: step(i) for i in range(FB + 2)]

            # Filler queue: PE work interleaved into the Act-bound attention
            # sections. Block (fb,qc) hides the next pair's Q/K projections;
            # the last pair's blocks hide the O projection of already-
            # normalized query chunks.
            def v_group(tt):
                # V projection for one key tile (token-major), ones col via memset
                vps = ps_mm.tile([P, 512], F32, tag="mm", name=f"vps{tt}")
                for kc in range(KO):
                    nc.tensor.matmul(
                        vps[:], xn_t[:, kc, tt * P:(tt + 1) * P], wv_sb[:, kc, :],
                        start=(kc == 0), stop=(kc == KO - 1),
                    )
                nc.vector.tensor_copy(
                    v4[:, tt, :, 0:64],
                    vps[:].rearrange("p (h c) -> p h c", c=64),
                )

            # Head-pair 0 queries (token half A) start as soon as xnA+wq0
            # land; wv/xnB stream behind them.  V key-tiles, the half-B
            # projections of pair 0, and everything else weave into the
            # first attention block just before each first use.
            fillers = []
            pending = None
            wq0 = proj_dma(wq_h, 0, "wq")
            wk0 = proj_dma(wk_h, 0, "wk")
            nc.sync.dma_start(wv_sb[:], wv_h[:].bitcast(F32R))
            nc.sync.dma_start(xn_t[:, :, 512:1024], xn_h[:, :, 512:1024].bitcast(F32R))
            for st in proj_steps(q_t, wq0, 0, 0, 0):
                st()
            for st in proj_steps(k_t, wk0, 512, 0, 0):
                st()
            v4 = v_t[:].rearrange("p a (h c) -> p a h c", c=65)
            nc.vector.memset(v4[:, :, :, 64:65].bitcast(F32), 1.0)

            last_w = {}
            for fb in range(FB):
                pops = 2
                if fb + 1 < FB:
                    n = fb + 1
                    wqf = proj_dma(wq_h, n, "wq")
                    wkf = proj_dma(wk_h, n, "wk")
                    if n < FB - 1:
                        fillers = [
                            st for qcx in range(QC)
                            for st in proj_steps(q_t, wqf, 0, n, qcx)
                        ] + [
                            st for qcx in range(QC)
                            for st in proj_steps(k_t, wkf, 512, n, qcx)
                        ]
                    else:
                        # only the half-A projections of the last pair here;
                        # its half-B work fills the pair's own first block
                        fillers = (
                            proj_steps(q_t, wqf, 0, n, 0)
                            + proj_steps(k_t, wkf, 512, n, 0)
                        )
                        last_w["q"], last_w["k"] = wqf, wkf
                else:
                    # scores kt>=4 of this pair need its half-B keys: pop 3
                    # per key-tile so that projection closes by kt 3
                    fillers = (
                        proj_steps(k_t, last_w["k"], 512, fb, 1)
                        + proj_steps(q_t, last_w["q"], 0, fb, 1)
                    )
                    pops = 3
                for qc in range(QC):
                    first = fb == 0 and qc == 0
                    cx0 = ps_cx.tile([65, 512], F32, tag="cx")
                    cx1 = ps_cx.tile([65, 512], F32, tag="cx")
                    cx = (cx0, cx1)
                    for kt in range(KT):
                        if first:
                            if kt == 4:
                                for st in proj_steps(k_t, wk0, 512, 0, 1):
                                    st()
                            v_group(kt)
                            if kt == 6:
                                for st in proj_steps(q_t, wq0, 0, 0, 1):
                                    st()
                        for hh in range(2):
                            sps = ps_sc.tile([P, 512], F32, tag="sc")
                            nc.tensor.matmul(
                                sps[:],
                                k_t[hh * DH:(hh + 1) * DH, fb, kt * P:(kt + 1) * P],
                                q_t[hh * DH:(hh + 1) * DH, fb, qc * 512:(qc + 1) * 512],
                                start=True, stop=True,
                            )
                            p_sb = work.tile([P, 512], F32R, tag="p")
                            nc.scalar.activation(
                                p_sb[:], sps[:], AF.Exp, bias=mb_sb[:, kt:kt + 1],
                            )
                            h = 2 * fb + hh
                            nc.tensor.matmul(
                                cx[hh][:],
                                v_t[:, kt, h * 65:(h + 1) * 65],
                                p_sb[:],
                                start=(kt == 0), stop=(kt == KT - 1),
                            )
                        if not first:
                            for _ in range(pops):
                                if fillers:
                                    fillers.pop(0)()
                    if pending is not None:
                        normalize(*pending)
                    pending = (fb, qc, cx)
                    if fb == FB - 1 and qc == 0:
                        # last pair: qc0 normalizes now so its O groups can
                        # fill qc1's attention section
                        normalize(*pending)
                        pending = None
                        fillers = [
                            st for ob in range(KO) for st in o_steps(ob, 0)
                        ]
                while fillers:
                    fillers.pop(0)()
                if fb == 0:
                    nc.sync.dma_start(wo_sb[:], wo_h[:].bitcast(F32R))
            normalize(*pending)

            # ---- remaining O projection (all of qc1) ----
            for ob in range(KO):
                for st in o_steps(ob, 1):
                    st()

    nc.finalize()
    return nc


def _build_expert_fp8(C, CN):
    """Launch B program, fp8 e4m3 DoubleRow variant: one expert FFN over C
    routed tokens, feature-major in/out.  Weights arrive pre-scaled by 64;
    the activation's scale=1/64 undoes it exactly.  Contraction runs 256
    deep per matmul (2 rows per partition, MatmulPerfMode.DoubleRow)."""
    import concourse.bacc as bacc
    import concourse.mybir as mybir
    import concourse.tile as tile

    F32, F32R, FP8 = mybir.dt.float32, mybir.dt.float32r, mybir.dt.float8e4
    AF = mybir.ActivationFunctionType
    DR = mybir.MatmulPerfMode.DoubleRow
    NCH = C // CN
    INV = 1.0 / 64.0

    nc = bacc.Bacc()
    xt_h = nc.dram_tensor("xt", [P, KO, C], FP8, kind="ExternalInput")   # LN2(x)^T tiled
    w1_h = nc.dram_tensor("w1", [MF, P, KO, P], FP8, kind="ExternalInput")
    w2_h = nc.dram_tensor("w2", [KO, P, MF, P], FP8, kind="ExternalInput")
    b1_h = nc.dram_tensor("b1c", [P, MF], F32, kind="ExternalInput")
    b2_h = nc.dram_tensor("b2c", [P, KO], F32, kind="ExternalInput")
    g_h = nc.dram_tensor("gates", [1, C], F32, kind="ExternalInput")
    y_h = nc.dram_tensor("y", [H, C], F32, kind="ExternalOutput")        # gated expert out^T

    with tile.TileContext(nc) as tc:
        with tc.tile_pool(name="consts", bufs=1) as consts, \
             tc.tile_pool(name="big", bufs=1) as big, \
             tc.tile_pool(name="w1s", bufs=4) as w1s, \
             tc.tile_pool(name="w2s", bufs=2) as w2s, \
             tc.tile_pool(name="work", bufs=2) as work, \
             tc.tile_pool(name="ps_mm", bufs=3, space="PSUM") as ps_mm, \
             tc.tile_pool(name="ps_gb", bufs=1, space="PSUM") as ps_gb:

            ones_f = consts.tile([1, P], F32)
            nc.vector.memset(ones_f[:], 1.0)
            ones_row = consts.tile([1, P], F32R)
            nc.vector.tensor_copy(ones_row[:], ones_f[:])

            x_t = big.tile([P, KO, C], FP8)
            nc.sync.dma_start(x_t[:], xt_h[:])
            xv = x_t[:].rearrange("p (dc i) t -> p dc i t", i=2)
            b1t = consts.tile([P, MF], F32)
            nc.sync.dma_start(b1t[:], b1_h[:])
            b2t = consts.tile([P, KO], F32)
            nc.sync.dma_start(b2t[:], b2_h[:])
            g_sb = consts.tile([1, C], F32R)
            nc.sync.dma_start(g_sb[:], g_h[:].bitcast(F32R))

            # ---- W1 pass + gelu (scale undoes the x64 weight prescale) ----
            h_t = big.tile([P, MF, C], FP8)
            for mf in range(MF):
                w1_mf = w1s.tile([P, KO, P], FP8, tag="w1")
                nc.sync.dma_start(w1_mf[:], w1_h[mf])
                wv1 = w1_mf[:].rearrange("p (dc i) m -> p dc i m", i=2)
                for nch in range(NCH):
                    hps = ps_mm.tile([P, CN], F32, tag="mm")
                    for dc in range(4):
                        nc.tensor.matmul(
                            hps[:], wv1[:, dc], xv[:, dc, :, nch * CN:(nch + 1) * CN],
                            start=(dc == 0), stop=(dc == 3), perf_mode=DR,
                        )
                    nc.scalar.activation(
                        h_t[:, mf, nch * CN:(nch + 1) * CN], hps[:],
                        AF.Gelu_apprx_tanh, bias=b1t[:, mf:mf + 1], scale=INV,
                    )

            # gate row broadcast to all partitions (needed from W2 phase on)
            gb_sb = big.tile([P, C], F32)
            for nch in range(NCH):
                gps = ps_gb.tile([P, CN], F32, tag="gb")
                nc.tensor.matmul(gps[:], ones_row[:], g_sb[:, nch * CN:(nch + 1) * CN],
                                 start=True, stop=True)
                nc.vector.tensor_copy(gb_sb[:, nch * CN:(nch + 1) * CN], gps[:])

            # ---- W2 pass + bias + gate ----
            hv = h_t[:].rearrange("p (dc i) t -> p dc i t", i=2)
            for oh in range(KO):
                w2_oh = w2s.tile([P, MF, P], FP8, tag="w2")
                nc.sync.dma_start(w2_oh[:], w2_h[oh])
                wv2 = w2_oh[:].rearrange("p (dc i) m -> p dc i m", i=2)
                y_sb = work.tile([P, C], F32, tag="y")
                for nch in range(NCH):
                    yps = ps_mm.tile([P, CN], F32, tag="mm")
                    for dc in range(MF // 2):
                        nc.tensor.matmul(
                            yps[:], wv2[:, dc], hv[:, dc, :, nch * CN:(nch + 1) * CN],
                            start=(dc == 0), stop=(dc == MF // 2 - 1), perf_mode=DR,
                        )
                    ytmp = work.tile([P, CN], F32, tag="ytmp")
                    nc.scalar.activation(ytmp[:], yps[:], AF.Identity,
                                         bias=b2t[:, oh:oh + 1], scale=INV)
                    nc.vector.tensor_mul(
                        y_sb[:, nch * CN:(nch + 1) * CN], ytmp[:],
                        gb_sb[:, nch * CN:(nch + 1) * CN],
                    )
                nc.sync.dma_start(
                    y_h[:].rearrange("(ko p) t -> p ko t", p=P)[:, oh, :], y_sb[:],
                )

    nc.finalize()
    return nc


def _build_expert(C, CN):
    """Launch B program: one expert FFN over C routed tokens, feature-major
    in/out.  The W1 pass runs as 3 fp8-e4m3 DoubleRow passes over host-split
    hi/lo components of x and W1 (x: x16 / x256, W1: x1024 / x16384; the two
    cross products share psum scale 2^18, hi*hi is 2^14) — more accurate than
    bf16 and 25% fewer PE cycles.  h and the W2 pass stay bf16."""
    import concourse.bacc as bacc
    import concourse.mybir as mybir
    import concourse.tile as tile

    F32, F32R, BF16 = mybir.dt.float32, mybir.dt.float32r, mybir.dt.bfloat16
    FP8 = mybir.dt.float8e4
    AF = mybir.ActivationFunctionType
    DR = mybir.MatmulPerfMode.DoubleRow
    NCH = C // CN

    nc = bacc.Bacc()
    xh_h = nc.dram_tensor("xh", [P, KO, C], FP8, kind="ExternalInput")   # LN2(x)^T hi
    xl_h = nc.dram_tensor("xl", [P, KO, C], FP8, kind="ExternalInput")   # LN2(x)^T lo
    w1_h = nc.dram_tensor("w1", [MF, P, 2, KO, P], FP8, kind="ExternalInput")
    w2_h = nc.dram_tensor("w2", [KO, P, MF, P], BF16, kind="ExternalInput")
    b1_h = nc.dram_tensor("b1c", [P, MF], F32, kind="ExternalInput")
    b2_h = nc.dram_tensor("b2c", [P, KO], F32, kind="ExternalInput")
    g_h = nc.dram_tensor("gates", [1, C], F32, kind="ExternalInput")
    y_h = nc.dram_tensor("y", [H, C], F32, kind="ExternalOutput")        # gated expert out^T

    with tile.TileContext(nc) as tc:
        with tc.tile_pool(name="consts", bufs=1) as consts, \
             tc.tile_pool(name="big", bufs=1) as big, \
             tc.tile_pool(name="w1s", bufs=4) as w1s, \
             tc.tile_pool(name="w2s", bufs=2) as w2s, \
             tc.tile_pool(name="work", bufs=2) as work, \
             tc.tile_pool(name="ps_mm", bufs=3, space="PSUM") as ps_mm, \
             tc.tile_pool(name="ps_gb", bufs=1, space="PSUM") as ps_gb:

            ones_f = consts.tile([1, P], F32)
            nc.vector.memset(ones_f[:], 1.0)
            ones_row = consts.tile([1, P], F32R)
            nc.vector.tensor_copy(ones_row[:], ones_f[:])

            # x first (hi then the first weight chunk then lo), then the
            # tiny bias/gate tensors (needed only once compute is rolling)
            x_hi = big.tile([P, KO, C], FP8)
            nc.sync.dma_start(x_hi[:], xh_h[:])
            w1_first = w1s.tile([P, 2, KO, P], FP8, tag="w1")
            nc.sync.dma_start(w1_first[:], w1_h[0])
            x_lo = big.tile([P, KO, C], FP8)
            nc.sync.dma_start(x_lo[:], xl_h[:])
            xhv = x_hi[:].rearrange("p (dc i) t -> p dc i t", i=2)
            xlv = x_lo[:].rearrange("p (dc i) t -> p dc i t", i=2)
            b1t = consts.tile([P, MF], F32)
            nc.sync.dma_start(b1t[:], b1_h[:])
            b2t = consts.tile([P, KO], F32)
            nc.sync.dma_start(b2t[:], b2_h[:])
            g_sb = consts.tile([1, C], F32R)
            nc.sync.dma_start(g_sb[:], g_h[:].bitcast(F32R))

            # ---- W1 pass (fp8 hi/lo, 3 DoubleRow passes) + gelu ----
            h_t = big.tile([P, MF, C], BF16)
            for mf in range(MF):
                if mf == 0:
                    w1_mf = w1_first
                else:
                    w1_mf = w1s.tile([P, 2, KO, P], FP8, tag="w1")
                    nc.sync.dma_start(w1_mf[:], w1_h[mf])
                w1v = w1_mf[:].rearrange("p a (dc i) m -> p a dc i m", i=2)
                for nch in range(NCH):
                    sl = slice(nch * CN, (nch + 1) * CN)
                    psa = ps_mm.tile([P, CN], F32, tag="mmA", bufs=2)
                    for dc in range(4):
                        nc.tensor.matmul(
                            psa[:], w1v[:, 0, dc], xhv[:, dc, :, sl],
                            start=(dc == 0), stop=(dc == 3), perf_mode=DR,
                        )
                    psb = ps_mm.tile([P, CN], F32, tag="mmB", bufs=2)
                    for dc in range(4):
                        nc.tensor.matmul(
                            psb[:], w1v[:, 1, dc], xhv[:, dc, :, sl],
                            start=(dc == 0), stop=False, perf_mode=DR,
                        )
                    for dc in range(4):
                        nc.tensor.matmul(
                            psb[:], w1v[:, 0, dc], xlv[:, dc, :, sl],
                            start=False, stop=(dc == 3), perf_mode=DR,
                        )
                    psa_sb = work.tile([P, CN], F32, tag="psa_sb")
                    nc.vector.tensor_copy(psa_sb[:], psa[:])
                    cmb = work.tile([P, CN], F32, tag="cmb")
                    nc.vector.scalar_tensor_tensor(
                        cmb[:], psb[:], 1.0 / 16.0, psa_sb[:],
                        mybir.AluOpType.mult, mybir.AluOpType.add,
                    )
                    nc.scalar.activation(
                        h_t[:, mf, sl], cmb[:],
                        AF.Gelu_apprx_tanh, bias=b1t[:, mf:mf + 1],
                        scale=1.0 / 16384.0,
                    )

            # gate row broadcast to all partitions (needed from W2 phase on)
            gb_sb = big.tile([P, C], F32)
            for nch in range(NCH):
                gps = ps_gb.tile([P, CN], F32, tag="gb")
                nc.tensor.matmul(gps[:], ones_row[:], g_sb[:, nch * CN:(nch + 1) * CN],
                                 start=True, stop=True)
                nc.vector.tensor_copy(gb_sb[:, nch * CN:(nch + 1) * CN], gps[:])

            # ---- W2 pass + bias + gate ----
            for oh in range(KO):
                w2_oh = w2s.tile([P, MF, P], BF16, tag="w2")
                nc.sync.dma_start(w2_oh[:], w2_h[oh])
                y_sb = work.tile([P, C], F32, tag="y")
                for nch in range(NCH):
                    yps = ps_mm.tile([P, CN], F32, tag="mm")
                    for kc2 in range(MF):
                        nc.tensor.matmul(
                            yps[:], w2_oh[:, kc2, :], h_t[:, kc2, nch * CN:(nch + 1) * CN],
                            start=(kc2 == 0), stop=(kc2 == MF - 1),
                        )
                    ytmp = work.tile([P, CN], F32, tag="ytmp")
                    nc.scalar.activation(ytmp[:], yps[:], AF.Identity, bias=b2t[:, oh:oh + 1])
                    nc.vector.tensor_mul(
                        y_sb[:, nch * CN:(nch + 1) * CN], ytmp[:],
                        gb_sb[:, nch * CN:(nch + 1) * CN],
                    )
                    nc.sync.dma_start(
                        y_h[:].rearrange("(ko p) t -> p ko t", p=P)[
                            :, oh, nch * CN:(nch + 1) * CN],
                        y_sb[:, nch * CN:(nch + 1) * CN],
                    )

    nc.finalize()
    return nc


def _get_attn(wb=True):
    key = ("attn", wb)
    if key not in _CACHE:
        _CACHE[key] = _build_attn(wb)
    return _CACHE[key]


def _get_expert(C, CN, fp8):
    key = ("exp", C, CN, fp8)
    if key not in _CACHE:
        _CACHE[key] = _build_expert_fp8(C, CN) if fp8 else _build_expert(C, CN)
    return _CACHE[key]


def _ln(x64):
    m = x64.mean(-1, keepdims=True)
    v = x64.var(-1, keepdims=True)
    return (x64 - m) / np.sqrt(v + EPS)


def _bf16(a):
    import ml_dtypes
    return np.ascontiguousarray(np.asarray(a).astype(ml_dtypes.bfloat16))


def _fp8(a):
    import ml_dtypes
    return np.ascontiguousarray(np.asarray(a).astype(ml_dtypes.float8_e4m3))


def _pko(a2d, x):
    """[H-like, X] row-major -> [P, n, X] SBUF tile layout (casts to f32)."""
    n = a2d.shape[0] // P
    return np.ascontiguousarray(
        np.asarray(a2d, dtype=np.float32).reshape(n, P, x).transpose(1, 0, 2))


def _pkod(a2d, x):
    """Same as _pko but dtype-preserving."""
    a = np.asarray(a2d)
    n = a.shape[0] // P
    return np.ascontiguousarray(a.reshape(n, P, x).transpose(1, 0, 2))


def kernel(**inputs):
    import os as _os
    import time as _time
    from concourse.bass_utils import run_bass_kernel_spmd

    f = lambda k: np.asarray(inputs[k], dtype=np.float32)
    x = f("hidden_states")                       # [B, S, H]
    mask = np.asarray(inputs["attention_mask"])  # [B, S] int32
    ln1_g, ln1_b = f("ln1_g").astype(np.float64), f("ln1_b").astype(np.float64)
    ln2_g, ln2_b = f("ln2_g").astype(np.float64), f("ln2_b").astype(np.float64)
    Wq, Wk, Wv, Wo = (f(k).astype(np.float64) for k in ("Wq", "Wk", "Wv", "Wo"))
    bq, bk, bv, bo = (f(k).astype(np.float64) for k in ("bq", "bk", "bv", "bo"))
    level_logits = f("level_logits").astype(np.float64)
    Wr, br = f("Wr").astype(np.float64), f("br").astype(np.float64)
    W1, b1 = f("W1").astype(np.float64), f("b1").astype(np.float64)
    W2, b2 = f("W2").astype(np.float64), f("b2").astype(np.float64)

    # ---- host folding ----
    scale = 1.0 / np.sqrt(DH)
    wq_eff = (ln1_g[None, :, None] * Wq) * scale              # [L,H,H]
    bq_eff = (bq + ln1_b @ Wq) * scale                        # [L,H]
    wk_eff = ln1_g[None, :, None] * Wk
    bk_eff = bk + ln1_b @ Wk
    wv_eff = ln1_g[None, :, None] * Wv
    bv_eff = bv + ln1_b @ Wv                                  # folded into boc below
    lw = np.exp(level_logits - level_logits.max())
    lw = lw / lw.sum()                                        # softmax(level_logits)
    wo_eff = lw[:, None, None] * Wo
    boc_eff = np.einsum("l,lh->h", lw, bo) + np.einsum("lf,lfh->h", bv_eff, wo_eff)

    xn1 = _ln(x.astype(np.float64)).astype(np.float32)        # LN1 (gamma/beta folded)

    def colt(vec):  # [H or F] -> [P, n] per-partition column layout
        v32 = np.ascontiguousarray(np.asarray(vec, dtype=np.float32))
        return np.ascontiguousarray(v32.reshape(-1, P).T)

    mbias = ((1.0 - mask.astype(np.float32)) * np.float32(-1e9))  # [B,S]
    xn1_T = np.swapaxes(xn1, 1, 2)                            # [B,H,S]

    in_maps = []
    for c in range(NCORES):
        b, l, hh = c >> 2, (c >> 1) & 1, c & 1
        sl = slice(hh * 512, (hh + 1) * 512)
        wq32 = wq_eff[l][:, sl].astype(np.float32)            # [H,512]
        wk32 = wk_eff[l][:, sl].astype(np.float32)
        in_maps.append({
            "xn": _pko(xn1_T[b], S),
            "wq": np.ascontiguousarray(
                _pko(wq32, 512).reshape(P, KO, FB, P).transpose(2, 0, 1, 3)),
            "wk": np.ascontiguousarray(
                _pko(wk32, 512).reshape(P, KO, FB, P).transpose(2, 0, 1, 3)),
            "wv": _pko(wv_eff[l][:, sl].astype(np.float32), 512),
            "wo": _pko(wo_eff[l][sl, :].astype(np.float32), H),
            "bqk": np.concatenate([bq_eff[l][sl], bk_eff[l][sl]]).astype(np.float32)[None, :],
            "mb": colt(mbias[b]),
        })

    wb = any(float(np.abs(m["bqk"]).max()) > 0.0 for m in in_maps)
    nc_a = _get_attn(wb)
    t0 = _time.time()
    res_a = run_bass_kernel_spmd(nc_a, in_maps, core_ids=list(range(NCORES)))
    _PERF["a_wall_s"] = _time.time() - t0
    _PERF["attn_wb"] = wb
    _PERF["a_exec_ns"] = res_a.exec_time_ns

    # ---- host: combine partials, residual, LN2, router, top-2 routing ----
    xres = x.astype(np.float64)                                # [B,S,H]
    for c in range(NCORES):
        b = c >> 2
        xres[b] += res_a.results[c]["attnp"].astype(np.float64).T
    xres += boc_eff[None, None, :]
    xres = xres.reshape(B * S, H)

    xn2 = _ln(xres)                                           # [B*S, H] (gamma/beta folded)
    logits = xn2 @ (ln2_g[:, None] * Wr) + (br + ln2_b @ Wr)  # [B*S, E]
    pm = logits.max(-1, keepdims=True)
    probs = np.exp(logits - pm)
    probs /= probs.sum(-1, keepdims=True)
    order = np.argsort(-probs, axis=-1, kind="stable")
    topi = order[:, :2]                                       # [T,2]
    topv = np.take_along_axis(probs, topi, axis=-1)
    gates = topv / topv.sum(-1, keepdims=True)                # [T,2]

    tok_idx, gate_val = [], []
    for e in range(E):
        sel = np.nonzero(topi == e)
        tok_idx.append(sel[0])
        gate_val.append(gates[sel[0], sel[1]])
    counts = [len(t) for t in tok_idx]
    C = max(512, ((max(counts) + 3) // 4) * 4)
    while True:  # need NCH with C % NCH == 0 and 256 <= C/NCH <= 512
        nch = (C + 511) // 512
        if C % nch == 0 and C // nch >= 256:
            break
        C += 4
    CN = C // ((C + 511) // 512)

    w1f = ln2_g[None, :, None] * W1                           # [E,H,F]
    b1f = b1 + ln2_b @ W1                                     # [E,F]
    xn2_T32 = np.ascontiguousarray(xn2.T.astype(np.float32))  # [H, B*S]

    fp8 = bool(_os.environ.get("KERNEL_MOE_FP8"))  # ~2e-2 rel err: off by default
    if not fp8:
        # hi/lo fp8 split of the LN2 output for the W1 pass (done once)
        xh_full = _fp8(xn2_T32 * np.float32(16.0))
        xl_full = _fp8(
            (xn2_T32 - xh_full.astype(np.float32) / np.float32(16.0)) * np.float32(256.0))
    in_maps_b = []
    for e in range(E):
        g = np.zeros((1, C), np.float32)
        g[0, :counts[e]] = gate_val[e].astype(np.float32)
        if fp8:
            xt = np.zeros((H, C), np.float32)
            xt[:, :counts[e]] = xn2_T32[:, tok_idx[e]]
            w1_32 = (w1f[e] * 64.0).astype(np.float32)        # [H,FF]
            w2_32 = (W2[e] * 64.0).astype(np.float32)         # [FF,H]
            in_maps_b.append({
                "xt": _fp8(_pko(xt, C)),
                "w1": _fp8(_pko(w1_32, FF).reshape(P, KO, MF, P).transpose(2, 0, 1, 3)),
                "w2": _fp8(_pko(w2_32, H).reshape(P, MF, KO, P).transpose(2, 0, 1, 3)),
                "b1c": colt(b1f[e]), "b2c": colt(b2[e]), "gates": g,
            })
            continue
        xh = np.zeros((H, C), xh_full.dtype)
        xh[:, :counts[e]] = xh_full[:, tok_idx[e]]
        xl = np.zeros((H, C), xl_full.dtype)
        xl[:, :counts[e]] = xl_full[:, tok_idx[e]]
        w1_32 = w1f[e].astype(np.float32)                     # [H,FF]
        w1h = _fp8(w1_32 * np.float32(1024.0))
        w1l = _fp8((w1_32 - w1h.astype(np.float32) / np.float32(1024.0))
                   * np.float32(16384.0))
        w1h_t = _pkod(w1h, FF).reshape(P, KO, MF, P).transpose(2, 0, 1, 3)
        w1l_t = _pkod(w1l, FF).reshape(P, KO, MF, P).transpose(2, 0, 1, 3)
        in_maps_b.append({
            "xh": _pkod(xh, C),
            "xl": _pkod(xl, C),
            "w1": np.ascontiguousarray(np.stack([w1h_t, w1l_t], axis=2)),
            "w2": _bf16(_pko(W2[e].astype(np.float32), H)
                        .reshape(P, MF, KO, P).transpose(2, 0, 1, 3)),
            "b1c": colt(b1f[e]),
            "b2c": colt(b2[e]),
            "gates": g,
        })

    nc_b = _get_expert(C, CN, fp8)
    t0 = _time.time()
    res_b = run_bass_kernel_spmd(nc_b, in_maps_b, core_ids=list(range(NCORES)))
    _PERF["b_wall_s"] = _time.time() - t0
    _PERF["b_exec_ns"] = res_b.exec_time_ns
    _PERF["capacity"] = C
    _PERF["counts"] = counts
    _PERF["moe_fp8"] = fp8

    if _os.environ.get("KERNEL_STASH"):
        _PERF["a_prog"] = (nc_a, in_maps)
        _PERF["b_prog"] = (nc_b, in_maps_b)

    out = xres.copy()
    for e in range(E):
        if counts[e]:
            out[tok_idx[e]] += res_b.results[e]["y"][:, :counts[e]].astype(np.float64).T
    return out.reshape(B, S, H).astype(np.float32)



# revision 11
# speedup vs baseline: 1.1803x; 1.1803x over previous
"""Trainium2 Bass kernel for nn_MoEMLABlock (MoE + multi-level attention block).

Strategy (8 NeuronCores, full inputs in / full output out):
  Launch A (attention, sharded over batch x level x head-half): core
    c = (b, l, hh) computes, for batch b, level l, heads hh*8..hh*8+7:
    Q/K/V projections over all 1024 tokens, softmax attention, and the
    partial O-projection [H, S] (feature-major).  No K/V recompute across
    cores.  LayerNorm 1 runs on the host (fp64) with gamma/beta folded
    into the projection weights; 1/sqrt(DH), the softmax level weights,
    and all biases are folded on the host.  Q/K biases enter the
    projection matmul as an extra ones-row contraction term; V bias and
    the O bias fold into a single per-batch constant added on the host.
    The softmax denominator is produced by the context matmul itself via
    a ones-column appended to V (psum row 64 = sumexp).  All device
    tensors arrive pre-laid-out in SBUF tile order so every DMA is one
    descriptor per partition.
  Host: sum the 4 partials per batch (+ residual + folded bias), LN2,
    router logits/softmax/top-2 (fp64), per-expert token gather.
  Launch B (expert-parallel): core e runs expert e's FFN
    gelu(x@W1+b1)@W2+b2 in bf16 (fp32 psum), gate-scaled on device, over
    its routed tokens, feature-major in and out (no device transposes).
  Host: scatter-add combine + residual.
"""

import numpy as np

H = 1024
NH = 16
DH = 64
L = 2
E = 8
FF = 4096
B = 2
S = 1024
EPS = 1e-5
P = 128
NCORES = 8
KO = H // P              # 8 contraction chunks over H
FB = 4                   # feature blocks of 128 (= head pairs) per core
QC = 2                   # query chunks of 512
KT = 8                   # key tiles of 128
MF = FF // P             # 32

_CACHE = {}
_PERF = {}


def _build_attn(wb=True):
    """Launch A program: one (batch, level, head-half) attention slice.
    wb=False elides the Q/K bias ones-row matmuls (all cores' folded
    biases are exactly zero for this input, decided by the host)."""
    import concourse.bacc as bacc
    import concourse.mybir as mybir
    import concourse.tile as tile

    F32, F32R = mybir.dt.float32, mybir.dt.float32r
    AF = mybir.ActivationFunctionType

    nc = bacc.Bacc()
    xn_h = nc.dram_tensor("xn", [P, KO, S], F32, kind="ExternalInput")   # LN1(x_b)^T tiled
    wq_h = nc.dram_tensor("wq", [FB, P, KO, P], F32, kind="ExternalInput")
    wk_h = nc.dram_tensor("wk", [FB, P, KO, P], F32, kind="ExternalInput")
    wv_h = nc.dram_tensor("wv", [P, KO, 512], F32, kind="ExternalInput")
    wo_h = nc.dram_tensor("wo", [P, FB, H], F32, kind="ExternalInput")
    bqk_h = nc.dram_tensor("bqk", [1, 1024], F32, kind="ExternalInput")  # bq | bk rows
    mb_h = nc.dram_tensor("mb", [P, KT], F32, kind="ExternalInput")      # key mask bias cols
    out_h = nc.dram_tensor("attnp", [H, S], F32, kind="ExternalOutput")

    with tile.TileContext(nc) as tc:
        with tc.tile_pool(name="consts", bufs=1) as consts, \
             tc.tile_pool(name="big", bufs=1) as big, \
             tc.tile_pool(name="wqk_s", bufs=2) as wqk_s, \
             tc.tile_pool(name="work", bufs=3) as work, \
             tc.tile_pool(name="outp", bufs=4) as outp, \
             tc.tile_pool(name="ps_mm", bufs=2, space="PSUM") as ps_mm, \
             tc.tile_pool(name="ps_sc", bufs=2, space="PSUM") as ps_sc, \
             tc.tile_pool(name="ps_cx", bufs=4, space="PSUM") as ps_cx:

            ones_f = consts.tile([1, 512], F32)
            nc.vector.memset(ones_f[:], 1.0)
            ones_row = consts.tile([1, 512], F32R)
            nc.vector.tensor_copy(ones_row[:], ones_f[:])

            bqk_sb = consts.tile([1, 1024], F32R)
            if wb:
                nc.sync.dma_start(bqk_sb[:], bqk_h[:].bitcast(F32R))
            mb_sb = consts.tile([P, KT], F32)
            nc.sync.dma_start(mb_sb[:], mb_h[:])

            # inputs, pre-tiled on the host: 1 descriptor per partition.
            # DMA issue order = first-use order (transfers share HBM bw):
            # first query-token half of xn, then wq0/wk0 so the head-pair-0
            # projections start ~10us in, with wv/xnB streaming behind.
            xn_t = big.tile([P, KO, S], F32R)
            nc.sync.dma_start(xn_t[:, :, 0:512], xn_h[:, :, 0:512].bitcast(F32R))
            wv_sb = big.tile([P, KO, 512], F32R)
            v_t = big.tile([P, KT, 8 * 65], F32R)       # per head: 64 cols V + 1 col ones

            # ---- interleaved per-head-pair: Q/K projection then attention ----
            # PE stays busy on the next pair's projections while the Act
            # engine works through this pair's exps; the normalize of block i
            # is emitted during block i+1 so its reciprocal never stalls PE.
            q_t = big.tile([P, FB, S], F32R)
            k_t = big.tile([P, FB, S], F32R)
            ctx_t = big.tile([P, FB, S], F32R)

            def proj_dma(w_h, fb, tag):
                w_fb = wqk_s.tile([P, KO, P], F32R, tag=tag, name=f"w_{tag}{fb}")
                nc.sync.dma_start(w_fb[:], w_h[fb].bitcast(F32R))
                return w_fb

            def proj_steps(dst, w_fb, bias_off, fb, qc):
                """One projection psum group as single-instruction steps, so
                it can be sprinkled into Act-bound attention sections."""
                box = {}

                def step(kc):
                    if kc == 0:
                        box["t"] = ps_mm.tile([P, 512], F32, tag="mm",
                                              name=f"qps{fb}_{qc}")
                    if kc < KO:
                        nc.tensor.matmul(
                            box["t"][:], w_fb[:, kc, :],
                            xn_t[:, kc, qc * 512:(qc + 1) * 512],
                            start=(kc == 0), stop=(kc == KO - 1 and not wb),
                        )
                    elif kc == KO and wb:
                        nc.tensor.matmul(
                            box["t"][:],
                            bqk_sb[:, bias_off + fb * P:bias_off + (fb + 1) * P],
                            ones_row[:], start=False, stop=True,
                        )
                    else:
                        nc.vector.tensor_copy(
                            dst[:, fb, qc * 512:(qc + 1) * 512], box["t"][:])

                ks = list(range(KO)) + ([KO] if wb else []) + [KO + 1]
                return [lambda k=k: step(k) for k in ks]

            def proj_fb(dst, w_h, bias_off, fb, tag):
                w_fb = proj_dma(w_h, fb, tag)
                for qc in range(QC):
                    for st in proj_steps(dst, w_fb, bias_off, fb, qc):
                        st()

            def normalize(fb, qc, cx):
                # 1/sumexp (psum row 64) broadcast to 64 partitions on the
                # otherwise-idle Pool engine, then scale ctx on DVE.
                for hh in range(2):
                    rcp = work.tile([1, 512], F32, tag="rcp")
                    nc.vector.reciprocal(rcp[:], cx[hh][64:65, :])
                    rb_sb = work.tile([64, 512], F32, tag="rb_sb")
                    nc.gpsimd.partition_broadcast(rb_sb[:], rcp[:])
                    nc.vector.tensor_mul(
                        ctx_t[hh * DH:(hh + 1) * DH, fb, qc * 512:(qc + 1) * 512],
                        cx[hh][0:64, :], rb_sb[:],
                    )

            wo_sb = big.tile([P, FB, H], F32R)

            def o_steps(ob, qc):
                # one O-projection psum group as steps (4 matmuls, copy, DMA)
                box = {}

                def step(i):
                    if i == 0:
                        box["t"] = ps_mm.tile([P, 512], F32, tag="mm",
                                              name=f"ops{ob}_{qc}")
                    if i < FB:
                        nc.tensor.matmul(
                            box["t"][:], wo_sb[:, i, ob * P:(ob + 1) * P],
                            ctx_t[:, i, qc * 512:(qc + 1) * 512],
                            start=(i == 0), stop=(i == FB - 1),
                        )
                    elif i == FB:
                        box["o"] = outp.tile([P, 512], F32, tag="o",
                                             name=f"oh{ob}_{qc}")
                        nc.vector.tensor_copy(box["o"][:], box["t"][:])
                    else:
                        nc.sync.dma_start(
                            out_h[:].rearrange("(ko p) t -> p ko t", p=P)[
                                :, ob, qc * 512:(qc + 1) * 512],
                            box["o"][:],
                        )

                return [lambda i=i: step(i) for i in range(FB + 2)]

            # Filler queue: PE work interleaved into the Act-bound attention
            # sections. Block (fb,qc) hides the next pair's Q/K projections;
            # the last pair's blocks hide the O projection of already-
            # normalized query chunks.
            def v_group(tt):
                # V projection for one key tile (token-major), ones col via memset
                vps = ps_mm.tile([P, 512], F32, tag="mm", name=f"vps{tt}")
                for kc in range(KO):
                    nc.tensor.matmul(
                        vps[:], xn_t[:, kc, tt * P:(tt + 1) * P], wv_sb[:, kc, :],
                        start=(kc == 0), stop=(kc == KO - 1),
                    )
                nc.vector.tensor_copy(
                    v4[:, tt, :, 0:64],
                    vps[:].rearrange("p (h c) -> p h c", c=64),
                )

            # Head-pair 0 queries (token half A) start as soon as xnA+wq0
            # land; wv/xnB stream behind them.  V key-tiles, the half-B
            # projections of pair 0, and everything else weave into the
            # first attention block just before each first use.
            fillers = []
            pending = None
            wq0 = proj_dma(wq_h, 0, "wq")
            wk0 = proj_dma(wk_h, 0, "wk")
            nc.sync.dma_start(wv_sb[:], wv_h[:].bitcast(F32R))
            nc.sync.dma_start(xn_t[:, :, 512:1024], xn_h[:, :, 512:1024].bitcast(F32R))
            for st in proj_steps(q_t, wq0, 0, 0, 0):
                st()
            for st in proj_steps(k_t, wk0, 512, 0, 0):
                st()
            v4 = v_t[:].rearrange("p a (h c) -> p a h c", c=65)
            nc.vector.memset(v4[:, :, :, 64:65].bitcast(F32), 1.0)

            last_w = {}
            for fb in range(FB):
                pops = 2
                if fb + 1 < FB:
                    n = fb + 1
                    wqf = proj_dma(wq_h, n, "wq")
                    wkf = proj_dma(wk_h, n, "wk")
                    if n < FB - 1:
                        fillers = [
                            st for qcx in range(QC)
                            for st in proj_steps(q_t, wqf, 0, n, qcx)
                        ] + [
                            st for qcx in range(QC)
                            for st in proj_steps(k_t, wkf, 512, n, qcx)
                        ]
                    else:
                        # only the half-A projections of the last pair here;
                        # its half-B work fills the pair's own first block
                        fillers = (
                            proj_steps(q_t, wqf, 0, n, 0)
                            + proj_steps(k_t, wkf, 512, n, 0)
                        )
                        last_w["q"], last_w["k"] = wqf, wkf
                else:
                    # scores kt>=4 of this pair need its half-B keys: pop 3
                    # per key-tile so that projection closes by kt 3
                    fillers = (
                        proj_steps(k_t, last_w["k"], 512, fb, 1)
                        + proj_steps(q_t, last_w["q"], 0, fb, 1)
                    )
                    pops = 3
                for qc in range(QC):
                    first = fb == 0 and qc == 0
                    cx0 = ps_cx.tile([65, 512], F32, tag="cx")
                    cx1 = ps_cx.tile([65, 512], F32, tag="cx")
                    cx = (cx0, cx1)
                    for kt in range(KT):
                        if first:
                            if kt == 4:
                                for st in proj_steps(k_t, wk0, 512, 0, 1):
                                    st()
                            v_group(kt)
                            if kt == 6:
                                for st in proj_steps(q_t, wq0, 0, 0, 1):
                                    st()
                        for hh in range(2):
                            sps = ps_sc.tile([P, 512], F32, tag="sc")
                            nc.tensor.matmul(
                                sps[:],
                                k_t[hh * DH:(hh + 1) * DH, fb, kt * P:(kt + 1) * P],
                                q_t[hh * DH:(hh + 1) * DH, fb, qc * 512:(qc + 1) * 512],
                                start=True, stop=True,
                            )
                            p_sb = work.tile([P, 512], F32R, tag="p")
                            nc.scalar.activation(
                                p_sb[:], sps[:], AF.Exp, bias=mb_sb[:, kt:kt + 1],
                            )
                            h = 2 * fb + hh
                            nc.tensor.matmul(
                                cx[hh][:],
                                v_t[:, kt, h * 65:(h + 1) * 65],
                                p_sb[:],
                                start=(kt == 0), stop=(kt == KT - 1),
                            )
                        if not first:
                            for _ in range(pops):
                                if fillers:
                                    fillers.pop(0)()
                    if pending is not None:
                        normalize(*pending)
                    pending = (fb, qc, cx)
                    if fb == FB - 1 and qc == 0:
                        # last pair: qc0 normalizes now so its O groups can
                        # fill qc1's attention section
                        normalize(*pending)
                        pending = None
                        fillers = [
                            st for ob in range(KO) for st in o_steps(ob, 0)
                        ]
                while fillers:
                    fillers.pop(0)()
                if fb == 0:
                    nc.sync.dma_start(wo_sb[:], wo_h[:].bitcast(F32R))
            normalize(*pending)

            # ---- remaining O projection (all of qc1) ----
            for ob in range(KO):
                for st in o_steps(ob, 1):
                    st()

    nc.finalize()
    return nc


def _build_expert_fp8(C, CN):
    """Launch B program, fp8 e4m3 DoubleRow variant: one expert FFN over C
    routed tokens, feature-major in/out.  Weights arrive pre-scaled by 64;
    the activation's scale=1/64 undoes it exactly.  Contraction runs 256
    deep per matmul (2 rows per partition, MatmulPerfMode.DoubleRow)."""
    import concourse.bacc as bacc
    import concourse.mybir as mybir
    import concourse.tile as tile

    F32, F32R, FP8 = mybir.dt.float32, mybir.dt.float32r, mybir.dt.float8e4
    AF = mybir.ActivationFunctionType
    DR = mybir.MatmulPerfMode.DoubleRow
    NCH = C // CN
    INV = 1.0 / 64.0

    nc = bacc.Bacc()
    xt_h = nc.dram_tensor("xt", [P, KO, C], FP8, kind="ExternalInput")   # LN2(x)^T tiled
    w1_h = nc.dram_tensor("w1", [MF, P, KO, P], FP8, kind="ExternalInput")
    w2_h = nc.dram_tensor("w2", [KO, P, MF, P], FP8, kind="ExternalInput")
    b1_h = nc.dram_tensor("b1c", [P, MF], F32, kind="ExternalInput")
    b2_h = nc.dram_tensor("b2c", [P, KO], F32, kind="ExternalInput")
    g_h = nc.dram_tensor("gates", [1, C], F32, kind="ExternalInput")
    y_h = nc.dram_tensor("y", [H, C], F32, kind="ExternalOutput")        # gated expert out^T

    with tile.TileContext(nc) as tc:
        with tc.tile_pool(name="consts", bufs=1) as consts, \
             tc.tile_pool(name="big", bufs=1) as big, \
             tc.tile_pool(name="w1s", bufs=4) as w1s, \
             tc.tile_pool(name="w2s", bufs=2) as w2s, \
             tc.tile_pool(name="work", bufs=2) as work, \
             tc.tile_pool(name="ps_mm", bufs=3, space="PSUM") as ps_mm, \
             tc.tile_pool(name="ps_gb", bufs=1, space="PSUM") as ps_gb:

            ones_f = consts.tile([1, P], F32)
            nc.vector.memset(ones_f[:], 1.0)
            ones_row = consts.tile([1, P], F32R)
            nc.vector.tensor_copy(ones_row[:], ones_f[:])

            x_t = big.tile([P, KO, C], FP8)
            nc.sync.dma_start(x_t[:], xt_h[:])
            xv = x_t[:].rearrange("p (dc i) t -> p dc i t", i=2)
            b1t = consts.tile([P, MF], F32)
            nc.sync.dma_start(b1t[:], b1_h[:])
            b2t = consts.tile([P, KO], F32)
            nc.sync.dma_start(b2t[:], b2_h[:])
            g_sb = consts.tile([1, C], F32R)
            nc.sync.dma_start(g_sb[:], g_h[:].bitcast(F32R))

            # ---- W1 pass + gelu (scale undoes the x64 weight prescale) ----
            h_t = big.tile([P, MF, C], FP8)
            for mf in range(MF):
                w1_mf = w1s.tile([P, KO, P], FP8, tag="w1")
                nc.sync.dma_start(w1_mf[:], w1_h[mf])
                wv1 = w1_mf[:].rearrange("p (dc i) m -> p dc i m", i=2)
                for nch in range(NCH):
                    hps = ps_mm.tile([P, CN], F32, tag="mm")
                    for dc in range(4):
                        nc.tensor.matmul(
                            hps[:], wv1[:, dc], xv[:, dc, :, nch * CN:(nch + 1) * CN],
                            start=(dc == 0), stop=(dc == 3), perf_mode=DR,
                        )
                    nc.scalar.activation(
                        h_t[:, mf, nch * CN:(nch + 1) * CN], hps[:],
                        AF.Gelu_apprx_tanh, bias=b1t[:, mf:mf + 1], scale=INV,
                    )

            # gate row broadcast to all partitions (needed from W2 phase on)
            gb_sb = big.tile([P, C], F32)
            for nch in range(NCH):
                gps = ps_gb.tile([P, CN], F32, tag="gb")
                nc.tensor.matmul(gps[:], ones_row[:], g_sb[:, nch * CN:(nch + 1) * CN],
                                 start=True, stop=True)
                nc.vector.tensor_copy(gb_sb[:, nch * CN:(nch + 1) * CN], gps[:])

            # ---- W2 pass + bias + gate ----
            hv = h_t[:].rearrange("p (dc i) t -> p dc i t", i=2)
            for oh in range(KO):
                w2_oh = w2s.tile([P, MF, P], FP8, tag="w2")
                nc.sync.dma_start(w2_oh[:], w2_h[oh])
                wv2 = w2_oh[:].rearrange("p (dc i) m -> p dc i m", i=2)
                y_sb = work.tile([P, C], F32, tag="y")
                for nch in range(NCH):
                    yps = ps_mm.tile([P, CN], F32, tag="mm")
                    for dc in range(MF // 2):
                        nc.tensor.matmul(
                            yps[:], wv2[:, dc], hv[:, dc, :, nch * CN:(nch + 1) * CN],
                            start=(dc == 0), stop=(dc == MF // 2 - 1), perf_mode=DR,
                        )
                    ytmp = work.tile([P, CN], F32, tag="ytmp")
                    nc.scalar.activation(ytmp[:], yps[:], AF.Identity,
                                         bias=b2t[:, oh:oh + 1], scale=INV)
                    nc.vector.tensor_mul(
                        y_sb[:, nch * CN:(nch + 1) * CN], ytmp[:],
                        gb_sb[:, nch * CN:(nch + 1) * CN],
                    )
                nc.sync.dma_start(
                    y_h[:].rearrange("(ko p) t -> p ko t", p=P)[:, oh, :], y_sb[:],
                )

    nc.finalize()
    return nc


def _build_expert_v2(C, CN, w2_hilo=False):
    """Launch B v2: one expert FFN over C routed tokens, feature-major in/out.
    W1 pass: 3 fp8-e4m3 DoubleRow passes over host-split hi/lo of x and W1
    (the combine reads both psums directly).  h is stored as single fp8
    (scale 1: gelu output magnitudes sit in e4m3's normal range).  W2 pass:
    fp8 DoubleRow — single pass over W2*64 (w2_hilo=False) or two passes
    over hi/lo-split W2 for ~2x tighter output error."""
    import concourse.bacc as bacc
    import concourse.mybir as mybir
    import concourse.tile as tile

    F32, F32R = mybir.dt.float32, mybir.dt.float32r
    FP8 = mybir.dt.float8e4
    AF = mybir.ActivationFunctionType
    DR = mybir.MatmulPerfMode.DoubleRow
    NCH = C // CN

    nc = bacc.Bacc()
    xh_h = nc.dram_tensor("xh", [P, KO, C], FP8, kind="ExternalInput")   # LN2(x)^T hi
    xl_h = nc.dram_tensor("xl", [P, KO, C], FP8, kind="ExternalInput")   # LN2(x)^T lo
    w1_h = nc.dram_tensor("w1", [MF, P, 2, KO, P], FP8, kind="ExternalInput")
    NW2 = 2 if w2_hilo else 1
    w2_h = nc.dram_tensor("w2", [KO, P, NW2, MF, P], FP8, kind="ExternalInput")
    b1_h = nc.dram_tensor("b1c", [P, MF], F32, kind="ExternalInput")
    b2_h = nc.dram_tensor("b2c", [P, KO], F32, kind="ExternalInput")
    g_h = nc.dram_tensor("gates", [1, C], F32, kind="ExternalInput")
    y_h = nc.dram_tensor("y", [H, C], F32, kind="ExternalOutput")        # gated expert out^T

    with tile.TileContext(nc) as tc:
        with tc.tile_pool(name="consts", bufs=1) as consts, \
             tc.tile_pool(name="big", bufs=1) as big, \
             tc.tile_pool(name="w1s", bufs=4) as w1s, \
             tc.tile_pool(name="w2s", bufs=2) as w2s, \
             tc.tile_pool(name="work", bufs=2) as work, \
             tc.tile_pool(name="ps_mm", bufs=3, space="PSUM") as ps_mm, \
             tc.tile_pool(name="ps_gb", bufs=1, space="PSUM") as ps_gb:

            ones_f = consts.tile([1, P], F32)
            nc.vector.memset(ones_f[:], 1.0)
            ones_row = consts.tile([1, P], F32R)
            nc.vector.tensor_copy(ones_row[:], ones_f[:])

            # x first (hi then the first weight chunk then lo), then the
            # tiny bias/gate tensors (needed only once compute is rolling)
            x_hi = big.tile([P, KO, C], FP8)
            nc.sync.dma_start(x_hi[:], xh_h[:])
            w1_first = w1s.tile([P, 2, KO, P], FP8, tag="w1")
            nc.sync.dma_start(w1_first[:], w1_h[0])
            x_lo = big.tile([P, KO, C], FP8)
            nc.sync.dma_start(x_lo[:], xl_h[:])
            xhv = x_hi[:].rearrange("p (dc i) t -> p dc i t", i=2)
            xlv = x_lo[:].rearrange("p (dc i) t -> p dc i t", i=2)
            b1t = consts.tile([P, MF], F32)
            nc.sync.dma_start(b1t[:], b1_h[:])
            b2t = consts.tile([P, KO], F32)
            nc.sync.dma_start(b2t[:], b2_h[:])
            g_sb = consts.tile([1, C], F32R)
            nc.sync.dma_start(g_sb[:], g_h[:].bitcast(F32R))

            # ---- W1 pass (fp8 hi/lo, 3 DoubleRow passes) + gelu -> fp8 h ----
            # All hi/lo components share one psum scale (x: x16, w1: x1024 —
            # the lo parts are encoded at the same scale as the hi parts, which
            # fp8's floating format permits), so the 12 matmuls accumulate in a
            # single psum and gelu reads it directly: no DVE combine.
            h_t = big.tile([P, MF, C], FP8)
            for mf in range(MF):
                if mf == 0:
                    w1_mf = w1_first
                else:
                    w1_mf = w1s.tile([P, 2, KO, P], FP8, tag="w1")
                    nc.sync.dma_start(w1_mf[:], w1_h[mf])
                w1v = w1_mf[:].rearrange("p a (dc i) m -> p a dc i m", i=2)
                for nch in range(NCH):
                    sl = slice(nch * CN, (nch + 1) * CN)
                    psa = ps_mm.tile([P, CN], F32, tag="mm")
                    for dc in range(4):
                        nc.tensor.matmul(
                            psa[:], w1v[:, 0, dc], xhv[:, dc, :, sl],
                            start=(dc == 0), stop=False, perf_mode=DR,
                        )
                    for dc in range(4):
                        nc.tensor.matmul(
                            psa[:], w1v[:, 1, dc], xhv[:, dc, :, sl],
                            start=False, stop=False, perf_mode=DR,
                        )
                    for dc in range(4):
                        nc.tensor.matmul(
                            psa[:], w1v[:, 0, dc], xlv[:, dc, :, sl],
                            start=False, stop=(dc == 3), perf_mode=DR,
                        )
                    nc.scalar.activation(
                        h_t[:, mf, sl], psa[:],
                        AF.Gelu_apprx_tanh, bias=b1t[:, mf:mf + 1],
                        scale=1.0 / 16384.0,
                    )

            # gate row broadcast to all partitions (needed from W2 phase on)
            gb_sb = big.tile([P, C], F32)
            for nch in range(NCH):
                gps = ps_gb.tile([P, CN], F32, tag="gb")
                nc.tensor.matmul(gps[:], ones_row[:], g_sb[:, nch * CN:(nch + 1) * CN],
                                 start=True, stop=True)
                nc.vector.tensor_copy(gb_sb[:, nch * CN:(nch + 1) * CN], gps[:])

            # ---- W2 pass (fp8 DoubleRow) + bias + gate ----
            hv = h_t[:].rearrange("p (dc i) t -> p dc i t", i=2)
            for oh in range(KO):
                w2_oh = w2s.tile([P, NW2, MF, P], FP8, tag="w2")
                nc.sync.dma_start(w2_oh[:], w2_h[oh])
                wv2 = w2_oh[:].rearrange("p a (dc i) m -> p a dc i m", i=2)
                y_sb = work.tile([P, C], F32, tag="y")
                for nch in range(NCH):
                    sl = slice(nch * CN, (nch + 1) * CN)
                    yps = ps_mm.tile([P, CN], F32, tag="mm")
                    for a in range(NW2):
                        for dc in range(MF // 2):
                            nc.tensor.matmul(
                                yps[:], wv2[:, a, dc], hv[:, dc, :, sl],
                                start=(a == 0 and dc == 0),
                                stop=(a == NW2 - 1 and dc == MF // 2 - 1),
                                perf_mode=DR,
                            )
                    ytmp = work.tile([P, CN], F32, tag="ytmp")
                    nc.scalar.activation(ytmp[:], yps[:], AF.Identity,
                                         bias=b2t[:, oh:oh + 1], scale=1.0 / 64.0)
                    nc.vector.tensor_mul(
                        y_sb[:, sl], ytmp[:], gb_sb[:, sl],
                    )
                    nc.sync.dma_start(
                        y_h[:].rearrange("(ko p) t -> p ko t", p=P)[:, oh, sl],
                        y_sb[:, sl],
                    )

    nc.finalize()
    return nc


def _build_expert(C, CN):
    """Launch B program: one expert FFN over C routed tokens, feature-major
    in/out.  The W1 pass runs as 3 fp8-e4m3 DoubleRow passes over host-split
    hi/lo components of x and W1 (x: x16 / x256, W1: x1024 / x16384; the two
    cross products share psum scale 2^18, hi*hi is 2^14) — more accurate than
    bf16 and 25% fewer PE cycles.  h and the W2 pass stay bf16."""
    import concourse.bacc as bacc
    import concourse.mybir as mybir
    import concourse.tile as tile

    F32, F32R, BF16 = mybir.dt.float32, mybir.dt.float32r, mybir.dt.bfloat16
    FP8 = mybir.dt.float8e4
    AF = mybir.ActivationFunctionType
    DR = mybir.MatmulPerfMode.DoubleRow
    NCH = C // CN

    nc = bacc.Bacc()
    xh_h = nc.dram_tensor("xh", [P, KO, C], FP8, kind="ExternalInput")   # LN2(x)^T hi
    xl_h = nc.dram_tensor("xl", [P, KO, C], FP8, kind="ExternalInput")   # LN2(x)^T lo
    w1_h = nc.dram_tensor("w1", [MF, P, 2, KO, P], FP8, kind="ExternalInput")
    w2_h = nc.dram_tensor("w2", [KO, P, MF, P], BF16, kind="ExternalInput")
    b1_h = nc.dram_tensor("b1c", [P, MF], F32, kind="ExternalInput")
    b2_h = nc.dram_tensor("b2c", [P, KO], F32, kind="ExternalInput")
    g_h = nc.dram_tensor("gates", [1, C], F32, kind="ExternalInput")
    y_h = nc.dram_tensor("y", [H, C], F32, kind="ExternalOutput")        # gated expert out^T

    with tile.TileContext(nc) as tc:
        with tc.tile_pool(name="consts", bufs=1) as consts, \
             tc.tile_pool(name="big", bufs=1) as big, \
             tc.tile_pool(name="w1s", bufs=4) as w1s, \
             tc.tile_pool(name="w2s", bufs=2) as w2s, \
             tc.tile_pool(name="work", bufs=2) as work, \
             tc.tile_pool(name="ps_mm", bufs=3, space="PSUM") as ps_mm, \
             tc.tile_pool(name="ps_gb", bufs=1, space="PSUM") as ps_gb:

            ones_f = consts.tile([1, P], F32)
            nc.vector.memset(ones_f[:], 1.0)
            ones_row = consts.tile([1, P], F32R)
            nc.vector.tensor_copy(ones_row[:], ones_f[:])

            # x first (hi then the first weight chunk then lo), then the
            # tiny bias/gate tensors (needed only once compute is rolling)
            x_hi = big.tile([P, KO, C], FP8)
            nc.sync.dma_start(x_hi[:], xh_h[:])
            w1_first = w1s.tile([P, 2, KO, P], FP8, tag="w1")
            nc.sync.dma_start(w1_first[:], w1_h[0])
            x_lo = big.tile([P, KO, C], FP8)
            nc.sync.dma_start(x_lo[:], xl_h[:])
            xhv = x_hi[:].rearrange("p (dc i) t -> p dc i t", i=2)
            xlv = x_lo[:].rearrange("p (dc i) t -> p dc i t", i=2)
            b1t = consts.tile([P, MF], F32)
            nc.sync.dma_start(b1t[:], b1_h[:])
            b2t = consts.tile([P, KO], F32)
            nc.sync.dma_start(b2t[:], b2_h[:])
            g_sb = consts.tile([1, C], F32R)
            nc.sync.dma_start(g_sb[:], g_h[:].bitcast(F32R))

            # ---- W1 pass (fp8 hi/lo, 3 DoubleRow passes) + gelu ----
            h_t = big.tile([P, MF, C], BF16)
            for mf in range(MF):
                if mf == 0:
                    w1_mf = w1_first
                else:
                    w1_mf = w1s.tile([P, 2, KO, P], FP8, tag="w1")
                    nc.sync.dma_start(w1_mf[:], w1_h[mf])
                w1v = w1_mf[:].rearrange("p a (dc i) m -> p a dc i m", i=2)
                for nch in range(NCH):
                    sl = slice(nch * CN, (nch + 1) * CN)
                    psa = ps_mm.tile([P, CN], F32, tag="mmA", bufs=2)
                    for dc in range(4):
                        nc.tensor.matmul(
                            psa[:], w1v[:, 0, dc], xhv[:, dc, :, sl],
                            start=(dc == 0), stop=(dc == 3), perf_mode=DR,
                        )
                    psb = ps_mm.tile([P, CN], F32, tag="mmB", bufs=2)
                    for dc in range(4):
                        nc.tensor.matmul(
                            psb[:], w1v[:, 1, dc], xhv[:, dc, :, sl],
                            start=(dc == 0), stop=False, perf_mode=DR,
                        )
                    for dc in range(4):
                        nc.tensor.matmul(
                            psb[:], w1v[:, 0, dc], xlv[:, dc, :, sl],
                            start=False, stop=(dc == 3), perf_mode=DR,
                        )
                    psa_sb = work.tile([P, CN], F32, tag="psa_sb")
                    nc.vector.tensor_copy(psa_sb[:], psa[:])
                    cmb = work.tile([P, CN], F32, tag="cmb")
                    nc.vector.scalar_tensor_tensor(
                        cmb[:], psb[:], 1.0 / 16.0, psa_sb[:],
                        mybir.AluOpType.mult, mybir.AluOpType.add,
                    )
                    nc.scalar.activation(
                        h_t[:, mf, sl], cmb[:],
                        AF.Gelu_apprx_tanh, bias=b1t[:, mf:mf + 1],
                        scale=1.0 / 16384.0,
                    )

            # gate row broadcast to all partitions (needed from W2 phase on)
            gb_sb = big.tile([P, C], F32)
            for nch in range(NCH):
                gps = ps_gb.tile([P, CN], F32, tag="gb")
                nc.tensor.matmul(gps[:], ones_row[:], g_sb[:, nch * CN:(nch + 1) * CN],
                                 start=True, stop=True)
                nc.vector.tensor_copy(gb_sb[:, nch * CN:(nch + 1) * CN], gps[:])

            # ---- W2 pass + bias + gate ----
            for oh in range(KO):
                w2_oh = w2s.tile([P, MF, P], BF16, tag="w2")
                nc.sync.dma_start(w2_oh[:], w2_h[oh])
                y_sb = work.tile([P, C], F32, tag="y")
                for nch in range(NCH):
                    yps = ps_mm.tile([P, CN], F32, tag="mm")
                    for kc2 in range(MF):
                        nc.tensor.matmul(
                            yps[:], w2_oh[:, kc2, :], h_t[:, kc2, nch * CN:(nch + 1) * CN],
                            start=(kc2 == 0), stop=(kc2 == MF - 1),
                        )
                    ytmp = work.tile([P, CN], F32, tag="ytmp")
                    nc.scalar.activation(ytmp[:], yps[:], AF.Identity, bias=b2t[:, oh:oh + 1])
                    nc.vector.tensor_mul(
                        y_sb[:, nch * CN:(nch + 1) * CN], ytmp[:],
                        gb_sb[:, nch * CN:(nch + 1) * CN],
                    )
                    nc.sync.dma_start(
                        y_h[:].rearrange("(ko p) t -> p ko t", p=P)[
                            :, oh, nch * CN:(nch + 1) * CN],
                        y_sb[:, nch * CN:(nch + 1) * CN],
                    )

    nc.finalize()
    return nc


def _get_attn(wb=True):
    key = ("attn", wb)
    if key not in _CACHE:
        _CACHE[key] = _build_attn(wb)
    return _CACHE[key]


def _get_expert(C, CN, mode):
    key = ("exp", C, CN, mode)
    if key not in _CACHE:
        if mode == "fp8":
            _CACHE[key] = _build_expert_fp8(C, CN)
        elif mode == "hilo":
            _CACHE[key] = _build_expert(C, CN)
        else:
            _CACHE[key] = _build_expert_v2(C, CN, w2_hilo=(mode == "v2hilo"))
    return _CACHE[key]


def _ln(x64):
    m = x64.mean(-1, keepdims=True)
    v = x64.var(-1, keepdims=True)
    return (x64 - m) / np.sqrt(v + EPS)


def _bf16(a):
    import ml_dtypes
    return np.ascontiguousarray(np.asarray(a).astype(ml_dtypes.bfloat16))


def _fp8(a):
    import ml_dtypes
    return np.ascontiguousarray(np.asarray(a).astype(ml_dtypes.float8_e4m3))


def _pko(a2d, x):
    """[H-like, X] row-major -> [P, n, X] SBUF tile layout (casts to f32)."""
    n = a2d.shape[0] // P
    return np.ascontiguousarray(
        np.asarray(a2d, dtype=np.float32).reshape(n, P, x).transpose(1, 0, 2))


def _pkod(a2d, x):
    """Same as _pko but dtype-preserving."""
    a = np.asarray(a2d)
    n = a.shape[0] // P
    return np.ascontiguousarray(a.reshape(n, P, x).transpose(1, 0, 2))


def kernel(**inputs):
    import os as _os
    import time as _time
    from concourse.bass_utils import run_bass_kernel_spmd

    f = lambda k: np.asarray(inputs[k], dtype=np.float32)
    x = f("hidden_states")                       # [B, S, H]
    mask = np.asarray(inputs["attention_mask"])  # [B, S] int32
    ln1_g, ln1_b = f("ln1_g").astype(np.float64), f("ln1_b").astype(np.float64)
    ln2_g, ln2_b = f("ln2_g").astype(np.float64), f("ln2_b").astype(np.float64)
    Wq, Wk, Wv, Wo = (f(k).astype(np.float64) for k in ("Wq", "Wk", "Wv", "Wo"))
    bq, bk, bv, bo = (f(k).astype(np.float64) for k in ("bq", "bk", "bv", "bo"))
    level_logits = f("level_logits").astype(np.float64)
    Wr, br = f("Wr").astype(np.float64), f("br").astype(np.float64)
    W1, b1 = f("W1").astype(np.float64), f("b1").astype(np.float64)
    W2, b2 = f("W2").astype(np.float64), f("b2").astype(np.float64)

    # ---- host folding ----
    scale = 1.0 / np.sqrt(DH)
    wq_eff = (ln1_g[None, :, None] * Wq) * scale              # [L,H,H]
    bq_eff = (bq + ln1_b @ Wq) * scale                        # [L,H]
    wk_eff = ln1_g[None, :, None] * Wk
    bk_eff = bk + ln1_b @ Wk
    wv_eff = ln1_g[None, :, None] * Wv
    bv_eff = bv + ln1_b @ Wv                                  # folded into boc below
    lw = np.exp(level_logits - level_logits.max())
    lw = lw / lw.sum()                                        # softmax(level_logits)
    wo_eff = lw[:, None, None] * Wo
    boc_eff = np.einsum("l,lh->h", lw, bo) + np.einsum("lf,lfh->h", bv_eff, wo_eff)

    xn1 = _ln(x.astype(np.float64)).astype(np.float32)        # LN1 (gamma/beta folded)

    def colt(vec):  # [H or F] -> [P, n] per-partition column layout
        v32 = np.ascontiguousarray(np.asarray(vec, dtype=np.float32))
        return np.ascontiguousarray(v32.reshape(-1, P).T)

    mbias = ((1.0 - mask.astype(np.float32)) * np.float32(-1e9))  # [B,S]
    xn1_T = np.swapaxes(xn1, 1, 2)                            # [B,H,S]

    in_maps = []
    for c in range(NCORES):
        b, l, hh = c >> 2, (c >> 1) & 1, c & 1
        sl = slice(hh * 512, (hh + 1) * 512)
        wq32 = wq_eff[l][:, sl].astype(np.float32)            # [H,512]
        wk32 = wk_eff[l][:, sl].astype(np.float32)
        in_maps.append({
            "xn": _pko(xn1_T[b], S),
            "wq": np.ascontiguousarray(
                _pko(wq32, 512).reshape(P, KO, FB, P).transpose(2, 0, 1, 3)),
            "wk": np.ascontiguousarray(
                _pko(wk32, 512).reshape(P, KO, FB, P).transpose(2, 0, 1, 3)),
            "wv": _pko(wv_eff[l][:, sl].astype(np.float32), 512),
            "wo": _pko(wo_eff[l][sl, :].astype(np.float32), H),
            "bqk": np.concatenate([bq_eff[l][sl], bk_eff[l][sl]]).astype(np.float32)[None, :],
            "mb": colt(mbias[b]),
        })

    wb = any(float(np.abs(m["bqk"]).max()) > 0.0 for m in in_maps)
    nc_a = _get_attn(wb)
    t0 = _time.time()
    res_a = run_bass_kernel_spmd(nc_a, in_maps, core_ids=list(range(NCORES)))
    _PERF["a_wall_s"] = _time.time() - t0
    _PERF["attn_wb"] = wb
    _PERF["a_exec_ns"] = res_a.exec_time_ns

    # ---- host: combine partials, residual, LN2, router, top-2 routing ----
    xres = x.astype(np.float64)                                # [B,S,H]
    for c in range(NCORES):
        b = c >> 2
        xres[b] += res_a.results[c]["attnp"].astype(np.float64).T
    xres += boc_eff[None, None, :]
    xres = xres.reshape(B * S, H)

    xn2 = _ln(xres)                                           # [B*S, H] (gamma/beta folded)
    logits = xn2 @ (ln2_g[:, None] * Wr) + (br + ln2_b @ Wr)  # [B*S, E]
    pm = logits.max(-1, keepdims=True)
    probs = np.exp(logits - pm)
    probs /= probs.sum(-1, keepdims=True)
    order = np.argsort(-probs, axis=-1, kind="stable")
    topi = order[:, :2]                                       # [T,2]
    topv = np.take_along_axis(probs, topi, axis=-1)
    gates = topv / topv.sum(-1, keepdims=True)                # [T,2]

    ps = np.sort(probs, axis=-1)
    _PERF["router_gap23"] = float((ps[:, -2] - ps[:, -3]).min())
    _PERF["topi"] = topi.copy()

    tok_idx, gate_val = [], []
    for e in range(E):
        sel = np.nonzero(topi == e)
        tok_idx.append(sel[0])
        gate_val.append(gates[sel[0], sel[1]])
    counts = [len(t) for t in tok_idx]
    C = max(512, ((max(counts) + 3) // 4) * 4)
    while True:  # need NCH with C % NCH == 0 and 256 <= C/NCH <= 512
        nch = (C + 511) // 512
        if C % nch == 0 and C // nch >= 256:
            break
        C += 4
    CN = C // ((C + 511) // 512)

    w1f = ln2_g[None, :, None] * W1                           # [E,H,F]
    b1f = b1 + ln2_b @ W1                                     # [E,F]
    xn2_T32 = np.ascontiguousarray(xn2.T.astype(np.float32))  # [H, B*S]

    mode = _os.environ.get("KERNEL_MOE_MODE", "v2")
    if mode != "fp8":
        # hi/lo fp8 split of the LN2 output for the W1 pass (done once).
        # v2 modes encode lo at the SAME scale as hi (x16) so the hi and lo
        # passes accumulate in one psum; the old "hilo" kernel wants x256.
        xh_full = _fp8(xn2_T32 * np.float32(16.0))
        xlo_scale = np.float32(256.0) if mode == "hilo" else np.float32(16.0)
        xl_full = _fp8(
            (xn2_T32 - xh_full.astype(np.float32) / np.float32(16.0)) * xlo_scale)
    in_maps_b = []
    for e in range(E):
        g = np.zeros((1, C), np.float32)
        g[0, :counts[e]] = gate_val[e].astype(np.float32)
        if mode == "fp8":
            xt = np.zeros((H, C), np.float32)
            xt[:, :counts[e]] = xn2_T32[:, tok_idx[e]]
            w1_32 = (w1f[e] * 64.0).astype(np.float32)        # [H,FF]
            w2_32 = (W2[e] * 64.0).astype(np.float32)         # [FF,H]
            in_maps_b.append({
                "xt": _fp8(_pko(xt, C)),
                "w1": _fp8(_pko(w1_32, FF).reshape(P, KO, MF, P).transpose(2, 0, 1, 3)),
                "w2": _fp8(_pko(w2_32, H).reshape(P, MF, KO, P).transpose(2, 0, 1, 3)),
                "b1c": colt(b1f[e]), "b2c": colt(b2[e]), "gates": g,
            })
            continue
        xh = np.zeros((H, C), xh_full.dtype)
        xh[:, :counts[e]] = xh_full[:, tok_idx[e]]
        xl = np.zeros((H, C), xl_full.dtype)
        xl[:, :counts[e]] = xl_full[:, tok_idx[e]]
        w1_32 = w1f[e].astype(np.float32)                     # [H,FF]
        w1h = _fp8(w1_32 * np.float32(1024.0))
        w1lo_scale = np.float32(16384.0) if mode == "hilo" else np.float32(1024.0)
        w1l = _fp8((w1_32 - w1h.astype(np.float32) / np.float32(1024.0))
                   * w1lo_scale)
        w1h_t = _pkod(w1h, FF).reshape(P, KO, MF, P).transpose(2, 0, 1, 3)
        w1l_t = _pkod(w1l, FF).reshape(P, KO, MF, P).transpose(2, 0, 1, 3)
        imap = {
            "xh": _pkod(xh, C),
            "xl": _pkod(xl, C),
            "w1": np.ascontiguousarray(np.stack([w1h_t, w1l_t], axis=2)),
            "b1c": colt(b1f[e]),
            "b2c": colt(b2[e]),
            "gates": g,
        }
        if mode == "hilo":
            imap["w2"] = _bf16(_pko(W2[e].astype(np.float32), H)
                               .reshape(P, MF, KO, P).transpose(2, 0, 1, 3))
        else:
            # fp8 W2, prescaled x64 (undone by the output activation scale)
            w2_32 = W2[e].astype(np.float32)                  # [FF,H]
            w2h = _fp8(w2_32 * np.float32(64.0))
            parts = [_pkod(w2h, H).reshape(P, MF, KO, P).transpose(2, 0, 1, 3)]
            if mode == "v2hilo":
                # lo at the same x64 scale as hi: single-psum accumulation
                w2l = _fp8((w2_32 - w2h.astype(np.float32) / np.float32(64.0))
                           * np.float32(64.0))
                parts.append(_pkod(w2l, H).reshape(P, MF, KO, P).transpose(2, 0, 1, 3))
            imap["w2"] = np.ascontiguousarray(np.stack(parts, axis=2))
        in_maps_b.append(imap)

    nc_b = _get_expert(C, CN, mode)
    t0 = _time.time()
    res_b = run_bass_kernel_spmd(nc_b, in_maps_b, core_ids=list(range(NCORES)))
    _PERF["b_wall_s"] = _time.time() - t0
    _PERF["b_exec_ns"] = res_b.exec_time_ns
    _PERF["capacity"] = C
    _PERF["counts"] = counts
    _PERF["moe_mode"] = mode

    if _os.environ.get("KERNEL_STASH"):
        _PERF["a_prog"] = (nc_a, in_maps)
        _PERF["b_prog"] = (nc_b, in_maps_b)

    out = xres.copy()
    for e in range(E):
        if counts[e]:
            out[tok_idx[e]] += res_b.results[e]["y"][:, :counts[e]].astype(np.float64).T
    return out.reshape(B, S, H).astype(np.float32)



# revision 32
# speedup vs baseline: 1.2404x; 1.0509x over previous
"""Trainium2 Bass kernel for nn_MoEMLABlock (MoE + multi-level attention block).

Strategy (8 NeuronCores, full inputs in / full output out):
  Launch A (attention, sharded over batch x level x head-half): core
    c = (b, l, hh) computes, for batch b, level l, heads hh*8..hh*8+7:
    Q/K/V projections over all 1024 tokens, softmax attention, and the
    partial O-projection [H, S] (feature-major).  No K/V recompute across
    cores.  LayerNorm 1 runs on the host (fp64) with gamma/beta folded
    into the projection weights; 1/sqrt(DH), the softmax level weights,
    and all biases are folded on the host.  Q/K biases enter the
    projection matmul as an extra ones-row contraction term; V bias and
    the O bias fold into a single per-batch constant added on the host.
    The softmax denominator is produced by the context matmul itself via
    a ones-column appended to V (psum row 64 = sumexp).  All device
    tensors arrive pre-laid-out in SBUF tile order so every DMA is one
    descriptor per partition.
  Host: sum the 4 partials per batch (+ residual + folded bias), LN2,
    router logits/softmax/top-2 (fp64), per-expert token gather.
  Launch B (expert-parallel): core e runs expert e's FFN
    gelu(x@W1+b1)@W2+b2 in bf16 (fp32 psum), gate-scaled on device, over
    its routed tokens, feature-major in and out (no device transposes).
  Host: scatter-add combine + residual.
"""

import numpy as np

H = 1024
NH = 16
DH = 64
L = 2
E = 8
FF = 4096
B = 2
S = 1024
EPS = 1e-5
P = 128
NCORES = 8
KO = H // P              # 8 contraction chunks over H
FB = 4                   # feature blocks of 128 (= head pairs) per core
QC = 2                   # query chunks of 512
KT = 8                   # key tiles of 128
MF = FF // P             # 32

_CACHE = {}
_PERF = {}


def _build_attn(wb=True):
    """Launch A program: one (batch, level, head-half) attention slice.
    wb=False elides the Q/K bias ones-row matmuls (all cores' folded
    biases are exactly zero for this input, decided by the host)."""
    import concourse.bacc as bacc
    import concourse.mybir as mybir
    import concourse.tile as tile

    F32, F32R = mybir.dt.float32, mybir.dt.float32r
    AF = mybir.ActivationFunctionType

    nc = bacc.Bacc()
    xn_h = nc.dram_tensor("xn", [P, KO, S], F32, kind="ExternalInput")   # LN1(x_b)^T tiled
    wq_h = nc.dram_tensor("wq", [FB, P, KO, P], F32, kind="ExternalInput")
    wk_h = nc.dram_tensor("wk", [FB, P, KO, P], F32, kind="ExternalInput")
    wv_h = nc.dram_tensor("wv", [P, KO, 512], F32, kind="ExternalInput")
    wo_h = nc.dram_tensor("wo", [P, FB, H], F32, kind="ExternalInput")
    bqk_h = nc.dram_tensor("bqk", [1, 1024], F32, kind="ExternalInput")  # bq | bk rows
    mb_h = nc.dram_tensor("mb", [P, KT], F32, kind="ExternalInput")      # key mask bias cols
    out_h = nc.dram_tensor("attnp", [H, S], F32, kind="ExternalOutput")

    with tile.TileContext(nc) as tc:
        with tc.tile_pool(name="consts", bufs=1) as consts, \
             tc.tile_pool(name="big", bufs=1) as big, \
             tc.tile_pool(name="wqk_s", bufs=2) as wqk_s, \
             tc.tile_pool(name="work", bufs=3) as work, \
             tc.tile_pool(name="outp", bufs=4) as outp, \
             tc.tile_pool(name="ps_mm", bufs=2, space="PSUM") as ps_mm, \
             tc.tile_pool(name="ps_sc", bufs=2, space="PSUM") as ps_sc, \
             tc.tile_pool(name="ps_cx", bufs=4, space="PSUM") as ps_cx:

            ones_f = consts.tile([1, 512], F32)
            nc.vector.memset(ones_f[:], 1.0)
            ones_row = consts.tile([1, 512], F32R)
            nc.vector.tensor_copy(ones_row[:], ones_f[:])

            bqk_sb = consts.tile([1, 1024], F32R)
            if wb:
                nc.sync.dma_start(bqk_sb[:], bqk_h[:].bitcast(F32R))
            mb_sb = consts.tile([P, KT], F32)
            nc.sync.dma_start(mb_sb[:], mb_h[:])

            # inputs, pre-tiled on the host: 1 descriptor per partition.
            # DMA issue order = first-use order (transfers share HBM bw):
            # first query-token half of xn, then wq0/wk0 so the head-pair-0
            # projections start ~10us in, with wv/xnB streaming behind.
            xn_t = big.tile([P, KO, S], F32R)
            nc.sync.dma_start(xn_t[:, :, 0:512], xn_h[:, :, 0:512].bitcast(F32R))
            wv_sb = big.tile([P, KO, 512], F32R)
            v_t = big.tile([P, KT, 8 * 65], F32R)       # per head: 64 cols V + 1 col ones

            # ---- interleaved per-head-pair: Q/K projection then attention ----
            # PE stays busy on the next pair's projections while the Act
            # engine works through this pair's exps; the normalize of block i
            # is emitted during block i+1 so its reciprocal never stalls PE.
            q_t = big.tile([P, FB, S], F32R)
            k_t = big.tile([P, FB, S], F32R)
            ctx_t = big.tile([P, FB, S], F32R)

            def proj_dma(w_h, fb, tag):
                w_fb = wqk_s.tile([P, KO, P], F32R, tag=tag, name=f"w_{tag}{fb}")
                nc.sync.dma_start(w_fb[:], w_h[fb].bitcast(F32R))
                return w_fb

            def proj_steps(dst, w_fb, bias_off, fb, qc):
                """One projection psum group as single-instruction steps, so
                it can be sprinkled into Act-bound attention sections."""
                box = {}

                def step(kc):
                    if kc == 0:
                        box["t"] = ps_mm.tile([P, 512], F32, tag="mm",
                                              name=f"qps{fb}_{qc}")
                    if kc < KO:
                        nc.tensor.matmul(
                            box["t"][:], w_fb[:, kc, :],
                            xn_t[:, kc, qc * 512:(qc + 1) * 512],
                            start=(kc == 0), stop=(kc == KO - 1 and not wb),
                        )
                    elif kc == KO and wb:
                        nc.tensor.matmul(
                            box["t"][:],
                            bqk_sb[:, bias_off + fb * P:bias_off + (fb + 1) * P],
                            ones_row[:], start=False, stop=True,
                        )
                    else:
                        nc.vector.tensor_copy(
                            dst[:, fb, qc * 512:(qc + 1) * 512], box["t"][:])

                ks = list(range(KO)) + ([KO] if wb else []) + [KO + 1]
                return [lambda k=k: step(k) for k in ks]

            def proj_fb(dst, w_h, bias_off, fb, tag):
                w_fb = proj_dma(w_h, fb, tag)
                for qc in range(QC):
                    for st in proj_steps(dst, w_fb, bias_off, fb, qc):
                        st()

            def normalize(fb, qc, cx):
                # 1/sumexp (psum row 64) broadcast to 64 partitions on the
                # otherwise-idle Pool engine, then scale ctx on DVE.
                for hh in range(2):
                    rcp = work.tile([1, 512], F32, tag="rcp")
                    nc.vector.reciprocal(rcp[:], cx[hh][64:65, :])
                    rb_sb = work.tile([64, 512], F32, tag="rb_sb")
                    nc.gpsimd.partition_broadcast(rb_sb[:], rcp[:])
                    nc.vector.tensor_mul(
                        ctx_t[hh * DH:(hh + 1) * DH, fb, qc * 512:(qc + 1) * 512],
                        cx[hh][0:64, :], rb_sb[:],
                    )

            wo_sb = big.tile([P, FB, H], F32R)

            def o_steps(ob, qc):
                # one O-projection psum group as steps (4 matmuls, copy, DMA)
                box = {}

                def step(i):
                    if i == 0:
                        box["t"] = ps_mm.tile([P, 512], F32, tag="mm",
                                              name=f"ops{ob}_{qc}")
                    if i < FB:
                        nc.tensor.matmul(
                            box["t"][:], wo_sb[:, i, ob * P:(ob + 1) * P],
                            ctx_t[:, i, qc * 512:(qc + 1) * 512],
                            start=(i == 0), stop=(i == FB - 1),
                        )
                    elif i == FB:
                        box["o"] = outp.tile([P, 512], F32, tag="o",
                                             name=f"oh{ob}_{qc}")
                        nc.vector.tensor_copy(box["o"][:], box["t"][:])
                    else:
                        nc.sync.dma_start(
                            out_h[:].rearrange("(ko p) t -> p ko t", p=P)[
                                :, ob, qc * 512:(qc + 1) * 512],
                            box["o"][:],
                        )

                return [lambda i=i: step(i) for i in range(FB + 2)]

            # Filler queue: PE work interleaved into the Act-bound attention
            # sections. Block (fb,qc) hides the next pair's Q/K projections;
            # the last pair's blocks hide the O projection of already-
            # normalized query chunks.
            def v_group(tt):
                # V projection for one key tile (token-major), ones col via memset
                vps = ps_mm.tile([P, 512], F32, tag="mm", name=f"vps{tt}")
                for kc in range(KO):
                    nc.tensor.matmul(
                        vps[:], xn_t[:, kc, tt * P:(tt + 1) * P], wv_sb[:, kc, :],
                        start=(kc == 0), stop=(kc == KO - 1),
                    )
                nc.vector.tensor_copy(
                    v4[:, tt, :, 0:64],
                    vps[:].rearrange("p (h c) -> p h c", c=64),
                )

            # Head-pair 0 queries (token half A) start as soon as xnA+wq0
            # land; wv/xnB stream behind them.  V key-tiles, the half-B
            # projections of pair 0, and everything else weave into the
            # first attention block just before each first use.
            fillers = []
            pending = None
            wq0 = proj_dma(wq_h, 0, "wq")
            wk0 = proj_dma(wk_h, 0, "wk")
            nc.sync.dma_start(wv_sb[:], wv_h[:].bitcast(F32R))
            nc.sync.dma_start(xn_t[:, :, 512:1024], xn_h[:, :, 512:1024].bitcast(F32R))
            for st in proj_steps(q_t, wq0, 0, 0, 0):
                st()
            for st in proj_steps(k_t, wk0, 512, 0, 0):
                st()
            v4 = v_t[:].rearrange("p a (h c) -> p a h c", c=65)
            nc.vector.memset(v4[:, :, :, 64:65].bitcast(F32), 1.0)

            last_w = {}
            for fb in range(FB):
                pops = 2
                if fb + 1 < FB:
                    n = fb + 1
                    wqf = proj_dma(wq_h, n, "wq")
                    wkf = proj_dma(wk_h, n, "wk")
                    if n < FB - 1:
                        fillers = [
                            st for qcx in range(QC)
                            for st in proj_steps(q_t, wqf, 0, n, qcx)
                        ] + [
                            st for qcx in range(QC)
                            for st in proj_steps(k_t, wkf, 512, n, qcx)
                        ]
                    else:
                        # only the half-A projections of the last pair here;
                        # its half-B work fills the pair's own first block
                        fillers = (
                            proj_steps(q_t, wqf, 0, n, 0)
                            + proj_steps(k_t, wkf, 512, n, 0)
                        )
                        last_w["q"], last_w["k"] = wqf, wkf
                else:
                    # scores kt>=4 of this pair need its half-B keys: pop 3
                    # per key-tile so that projection closes by kt 3
                    fillers = (
                        proj_steps(k_t, last_w["k"], 512, fb, 1)
                        + proj_steps(q_t, last_w["q"], 0, fb, 1)
                    )
                    pops = 3
                for qc in range(QC):
                    first = fb == 0 and qc == 0
                    cx0 = ps_cx.tile([65, 512], F32, tag="cx")
                    cx1 = ps_cx.tile([65, 512], F32, tag="cx")
                    cx = (cx0, cx1)
                    for kt in range(KT):
                        if first:
                            if kt == 4:
                                for st in proj_steps(k_t, wk0, 512, 0, 1):
                                    st()
                            v_group(kt)
                            if kt == 6:
                                for st in proj_steps(q_t, wq0, 0, 0, 1):
                                    st()
                        for hh in range(2):
                            sps = ps_sc.tile([P, 512], F32, tag="sc")
                            nc.tensor.matmul(
                                sps[:],
                                k_t[hh * DH:(hh + 1) * DH, fb, kt * P:(kt + 1) * P],
                                q_t[hh * DH:(hh + 1) * DH, fb, qc * 512:(qc + 1) * 512],
                                start=True, stop=True,
                            )
                            p_sb = work.tile([P, 512], F32R, tag="p")
                            nc.scalar.activation(
                                p_sb[:], sps[:], AF.Exp, bias=mb_sb[:, kt:kt + 1],
                            )
                            h = 2 * fb + hh
                            nc.tensor.matmul(
                                cx[hh][:],
                                v_t[:, kt, h * 65:(h + 1) * 65],
                                p_sb[:],
                                start=(kt == 0), stop=(kt == KT - 1),
                            )
                        if not first:
                            for _ in range(pops):
                                if fillers:
                                    fillers.pop(0)()
                    if pending is not None:
                        normalize(*pending)
                    pending = (fb, qc, cx)
                    if fb == FB - 1 and qc == 0:
                        # last pair: qc0 normalizes now so its O groups can
                        # fill qc1's attention section
                        normalize(*pending)
                        pending = None
                        fillers = [
                            st for ob in range(KO) for st in o_steps(ob, 0)
                        ]
                while fillers:
                    fillers.pop(0)()
                if fb == 0:
                    nc.sync.dma_start(wo_sb[:], wo_h[:].bitcast(F32R))
            normalize(*pending)

            # ---- remaining O projection (all of qc1) ----
            for ob in range(KO):
                for st in o_steps(ob, 1):
                    st()

    nc.finalize()
    return nc


def _build_expert_fp8(C, CN):
    """Launch B program, fp8 e4m3 DoubleRow variant: one expert FFN over C
    routed tokens, feature-major in/out.  Weights arrive pre-scaled by 64;
    the activation's scale=1/64 undoes it exactly.  Contraction runs 256
    deep per matmul (2 rows per partition, MatmulPerfMode.DoubleRow)."""
    import concourse.bacc as bacc
    import concourse.mybir as mybir
    import concourse.tile as tile

    F32, F32R, FP8 = mybir.dt.float32, mybir.dt.float32r, mybir.dt.float8e4
    AF = mybir.ActivationFunctionType
    DR = mybir.MatmulPerfMode.DoubleRow
    NCH = C // CN
    INV = 1.0 / 64.0

    nc = bacc.Bacc()
    xt_h = nc.dram_tensor("xt", [P, KO, C], FP8, kind="ExternalInput")   # LN2(x)^T tiled
    w1_h = nc.dram_tensor("w1", [MF, P, KO, P], FP8, kind="ExternalInput")
    w2_h = nc.dram_tensor("w2", [KO, P, MF, P], FP8, kind="ExternalInput")
    b1_h = nc.dram_tensor("b1c", [P, MF], F32, kind="ExternalInput")
    b2_h = nc.dram_tensor("b2c", [P, KO], F32, kind="ExternalInput")
    g_h = nc.dram_tensor("gates", [1, C], F32, kind="ExternalInput")
    y_h = nc.dram_tensor("y", [H, C], F32, kind="ExternalOutput")        # gated expert out^T

    with tile.TileContext(nc) as tc:
        with tc.tile_pool(name="consts", bufs=1) as consts, \
             tc.tile_pool(name="big", bufs=1) as big, \
             tc.tile_pool(name="w1s", bufs=4) as w1s, \
             tc.tile_pool(name="w2s", bufs=8) as w2s, \
             tc.tile_pool(name="work", bufs=2) as work, \
             tc.tile_pool(name="ps_mm", bufs=3, space="PSUM") as ps_mm, \
             tc.tile_pool(name="ps_gb", bufs=1, space="PSUM") as ps_gb:

            ones_f = consts.tile([1, P], F32)
            nc.vector.memset(ones_f[:], 1.0)
            ones_row = consts.tile([1, P], F32R)
            nc.vector.tensor_copy(ones_row[:], ones_f[:])

            x_t = big.tile([P, KO, C], FP8)
            nc.sync.dma_start(x_t[:], xt_h[:])
            xv = x_t[:].rearrange("p (dc i) t -> p dc i t", i=2)
            b1t = consts.tile([P, MF], F32)
            nc.sync.dma_start(b1t[:], b1_h[:])
            b2t = consts.tile([P, KO], F32)
            nc.sync.dma_start(b2t[:], b2_h[:])
            g_sb = consts.tile([1, C], F32R)
            nc.sync.dma_start(g_sb[:], g_h[:].bitcast(F32R))

            # ---- W1 pass + gelu (scale undoes the x64 weight prescale) ----
            h_t = big.tile([P, MF, C], FP8)
            for mf in range(MF):
                w1_mf = w1s.tile([P, KO, P], FP8, tag="w1")
                nc.sync.dma_start(w1_mf[:], w1_h[mf])
                wv1 = w1_mf[:].rearrange("p (dc i) m -> p dc i m", i=2)
                for nch in range(NCH):
                    hps = ps_mm.tile([P, CN], F32, tag="mm")
                    for dc in range(4):
                        nc.tensor.matmul(
                            hps[:], wv1[:, dc], xv[:, dc, :, nch * CN:(nch + 1) * CN],
                            start=(dc == 0), stop=(dc == 3), perf_mode=DR,
                        )
                    nc.scalar.activation(
                        h_t[:, mf, nch * CN:(nch + 1) * CN], hps[:],
                        AF.Gelu_apprx_tanh, bias=b1t[:, mf:mf + 1], scale=INV,
                    )

            # gate row broadcast to all partitions (needed from W2 phase on)
            gb_sb = big.tile([P, C], F32)
            for nch in range(NCH):
                gps = ps_gb.tile([P, CN], F32, tag="gb")
                nc.tensor.matmul(gps[:], ones_row[:], g_sb[:, nch * CN:(nch + 1) * CN],
                                 start=True, stop=True)
                nc.vector.tensor_copy(gb_sb[:, nch * CN:(nch + 1) * CN], gps[:])

            # ---- W2 pass + bias + gate ----
            hv = h_t[:].rearrange("p (dc i) t -> p dc i t", i=2)
            for oh in range(KO):
                w2_oh = w2s.tile([P, MF, P], FP8, tag="w2")
                nc.sync.dma_start(w2_oh[:], w2_h[oh])
                wv2 = w2_oh[:].rearrange("p (dc i) m -> p dc i m", i=2)
                y_sb = work.tile([P, C], F32, tag="y")
                for nch in range(NCH):
                    yps = ps_mm.tile([P, CN], F32, tag="mm")
                    for dc in range(MF // 2):
                        nc.tensor.matmul(
                            yps[:], wv2[:, dc], hv[:, dc, :, nch * CN:(nch + 1) * CN],
                            start=(dc == 0), stop=(dc == MF // 2 - 1), perf_mode=DR,
                        )
                    ytmp = work.tile([P, CN], F32, tag="ytmp")
                    nc.scalar.activation(ytmp[:], yps[:], AF.Identity,
                                         bias=b2t[:, oh:oh + 1], scale=INV)
                    nc.vector.tensor_mul(
                        y_sb[:, nch * CN:(nch + 1) * CN], ytmp[:],
                        gb_sb[:, nch * CN:(nch + 1) * CN],
                    )
                nc.sync.dma_start(
                    y_h[:].rearrange("(ko p) t -> p ko t", p=P)[:, oh, :], y_sb[:],
                )

    nc.finalize()
    return nc


def _build_attn_v2(qs=8.0):
    """Launch A v2: fp8-DoubleRow attention for the all-ones-mask / zero-bias
    fast path.  One (batch, level, head-half) slice = 8 heads per core.

    Layouts (dual-fp8 Ldweights needs its two DR rows exactly 128 cols apart):
      xn8v [P, 4dc, 8tt, 2i, 128]  fp8(16*LN1x)^T: V stationary + Q/K moving
      wq8/wk8 [4pb, P, KO, P]      pb=(hg,j): cols = head 4hg+c/32, feat 32j+c%32
      q8 [P, 2hg, 2j, S]           = qs*q   (DVE scalar-mul of proj psum)
      k8 [P, 2hg, 8kt, 2j, 128]    = k
      v8 [P, 8h, 4tp, 2j, 128]     = v cols 0:64, col 64 = 1.0 (sumexp), rest pad
      p8 [P, 2j, 512]              = exp(scores) per (h, qc, ktpair)
    Scores psum pairs (kt even/odd) land in one 2-bank [128, 1024] psum tile so
    a single Act instruction computes both exps (scale 1/qs folds the q scale).
    ctx DR([128,2j,65] x [128,2j,512]) accumulates [65,512]; row 64 = sumexp.
    Normalize (recip + Pool broadcast + DVE mul) -> ctx_t f32r; O-proj f32r."""
    import concourse.bacc as bacc
    import concourse.mybir as mybir
    import concourse.tile as tile

    F32, F32R, FP8 = mybir.dt.float32, mybir.dt.float32r, mybir.dt.float8e4
    AF = mybir.ActivationFunctionType
    DR = mybir.MatmulPerfMode.DoubleRow
    TP = KT // 2

    nc = bacc.Bacc()
    xn8_h = nc.dram_tensor("xn8v", [P, 4, KT, 2, P], FP8, kind="ExternalInput")
    wq8_h = nc.dram_tensor("wq8", [4, P, KO, P], FP8, kind="ExternalInput")
    wk8_h = nc.dram_tensor("wk8", [4, P, KO, P], FP8, kind="ExternalInput")
    wv8_h = nc.dram_tensor("wv8", [P, KO, 512], FP8, kind="ExternalInput")
    wo_h = nc.dram_tensor("wo", [P, FB, H], F32, kind="ExternalInput")
    out_h = nc.dram_tensor("attnp", [H, S], F32, kind="ExternalOutput")

    with tile.TileContext(nc) as tc:
        with tc.tile_pool(name="big", bufs=1) as big, \
             tc.tile_pool(name="p8p", bufs=4) as p8p, \
             tc.tile_pool(name="work", bufs=3) as work, \
             tc.tile_pool(name="outp", bufs=4) as outp, \
             tc.tile_pool(name="ps_mm", bufs=2, space="PSUM") as ps_mm, \
             tc.tile_pool(name="ps_sc", bufs=2, space="PSUM") as ps_sc, \
             tc.tile_pool(name="ps_cx", bufs=2, space="PSUM") as ps_cx:

            xn8 = big.tile([P, 4, KT, 2, P], FP8)
            q8 = big.tile([P, 2, 2, S], FP8)
            k8 = big.tile([P, 2, KT, 2, P], FP8)
            v8 = big.tile([P, 8, TP, 2, P], FP8)
            ctx_t = big.tile([P, FB, S], F32R)
            wo_sb = big.tile([P, FB, H], F32R)
            wv8_sb = big.tile([P, KO, 512], FP8)
            wq8_sb = big.tile([P, 4, KO, P], FP8)
            wk8_sb = big.tile([P, 4, KO, P], FP8)

            # DMA order = first-use order (transfers serialize on the bus)
            nc.sync.dma_start(xn8[:, :, 0:4], xn8_h[:, :, 0:4])
            nc.sync.dma_start(wq8_sb[:], wq8_h[:].rearrange("b p ko c -> p b ko c"))
            nc.sync.dma_start(wk8_sb[:], wk8_h[:].rearrange("b p ko c -> p b ko c"))
            nc.sync.dma_start(wv8_sb[:], wv8_h[:])
            nc.sync.dma_start(xn8[:, :, 4:8], xn8_h[:, :, 4:8])

            nc.vector.memset(v8[:, :, :, :, 64:65], 1.0)

            def qk_group(dst8, w_sb, hg, j, qc, scale):
                """One Q/K projection psum group -> fp8 dst, as steps.
                Each step covers one 128-token tile (4 DR matmuls over dc)."""
                pb = 2 * hg + j
                box = {}

                def step(st):
                    if st == 0:
                        box["t"] = ps_mm.tile([P, 512], F32, tag="mm",
                                              name=f"qkps{pb}_{qc}_{id(dst8) % 97}")
                    if st < 4:
                        tt = 4 * qc + st
                        wv = w_sb[:, pb].rearrange("p (dc i) m -> p dc i m", i=2)
                        for dc in range(4):
                            nc.tensor.matmul(
                                box["t"][:, 128 * st:128 * st + 128],
                                wv[:, dc], xn8[:, dc, tt],
                                start=(dc == 0), stop=(dc == 3), perf_mode=DR,
                            )
                    else:
                        if dst8 is q8:
                            dst = q8[:, hg, j, 512 * qc:512 * qc + 512]
                        else:
                            dst = k8[:, hg, 4 * qc:4 * qc + 4, j, :]
                        nc.vector.tensor_scalar_mul(dst, box["t"][:], scale)

                return [lambda d=d: step(d) for d in range(5)]

            def v_group(tt):
                """V projection for one key tile -> fp8 v8, as steps."""
                box = {}

                def step(dc):
                    if dc == 0:
                        box["t"] = ps_mm.tile([P, 512], F32, tag="mm",
                                              name=f"vps{tt}")
                    if dc < 4:
                        wvv = wv8_sb[:].rearrange("p (dc i) m -> p dc i m", i=2)
                        nc.tensor.matmul(
                            box["t"][:], xn8[:, dc, tt], wvv[:, dc],
                            start=(dc == 0), stop=(dc == 3), perf_mode=DR,
                        )
                    else:
                        nc.vector.tensor_scalar_mul(
                            v8[:, :, tt // 2, tt % 2, 0:64],
                            box["t"][:].rearrange("p (h c) -> p h c", c=64),
                            1.0 / 4096.0,
                        )

                return [lambda d=d: step(d) for d in range(5)]

            def o_steps(ob, qc):
                box = {}

                def step(i):
                    if i == 0:
                        box["t"] = ps_mm.tile([P, 512], F32, tag="mm",
                                              name=f"ops{ob}_{qc}")
                    if i < FB:
                        nc.tensor.matmul(
                            box["t"][:], wo_sb[:, i, ob * P:(ob + 1) * P],
                            ctx_t[:, i, qc * 512:(qc + 1) * 512],
                            start=(i == 0), stop=(i == FB - 1),
                        )
                    elif i == FB:
                        box["o"] = outp.tile([P, 512], F32, tag="o",
                                             name=f"oh{ob}_{qc}")
                        nc.vector.tensor_copy(box["o"][:], box["t"][:])
                    else:
                        nc.sync.dma_start(
                            out_h[:].rearrange("(ko p) t -> p ko t", p=P)[
                                :, ob, qc * 512:(qc + 1) * 512],
                            box["o"][:],
                        )

                return [lambda i=i: step(i) for i in range(FB + 2)]

            def normalize(h, qc, cx):
                rcp = work.tile([1, 512], F32, tag="rcp")
                nc.vector.reciprocal(rcp[:], cx[64:65, :])
                rb_sb = work.tile([64, 512], F32, tag="rb_sb")
                nc.gpsimd.partition_broadcast(rb_sb[:], rcp[:])
                nc.vector.tensor_mul(
                    ctx_t[(h % 2) * DH:(h % 2 + 1) * DH, h // 2,
                          qc * 512:(qc + 1) * 512],
                    cx[0:64, :], rb_sb[:],
                )

            # Bootstrap: q/k for heads 0-3 over the first token/key halves
            for st in qk_group(q8, wq8_sb, 0, 0, 0, 1.0 / 512.0):
                st()
            for st in qk_group(q8, wq8_sb, 0, 1, 0, 1.0 / 512.0):
                st()
            for st in qk_group(k8, wk8_sb, 0, 0, 0, 1.0 / 4096.0):
                st()
            for st in qk_group(k8, wk8_sb, 0, 1, 0, 1.0 / 4096.0):
                st()
            nc.sync.dma_start(wo_sb[:], wo_h[:].bitcast(F32R))

            # Named filler groups: popped for PE pacing during the Act-bound
            # attention stream, but force-drained via need() before any
            # consumer is emitted (emission order defines the dataflow).
            fillers = []                             # [name, step, step, ...]
            done = set()

            def add_group(name, steps):
                fillers.extend(steps)
                fillers.append(name)     # marker AFTER steps: done == emitted

            def pop_one():
                while fillers and isinstance(fillers[0], str):
                    done.add(fillers.pop(0))
                if fillers:
                    fillers.pop(0)()
                while fillers and isinstance(fillers[0], str):
                    done.add(fillers.pop(0))

            def need(*names):
                while any(n not in done for n in names):
                    assert fillers, f"missing groups: {names}"
                    pop_one()

            for j in range(2):                       # V first key half
                add_group(f"v{2*j}", v_group(2 * j))
                add_group(f"v{2*j+1}", v_group(2 * j + 1))
            for j in range(2):                       # keys half 2, heads 0-3
                add_group(f"k0{j}1", qk_group(k8, wk8_sb, 0, j, 1, 1.0 / 4096.0))
            for tt in range(4, 8):                   # V second key half
                add_group(f"v{tt}", v_group(tt))
            for j in range(2):                       # k heads 4-7
                for kc in range(2):
                    add_group(f"k1{j}{kc}",
                              qk_group(k8, wk8_sb, 1, j, kc, 1.0 / 4096.0))
            for j in range(2):                       # q heads 4-7 qc0
                add_group(f"q1{j}0", qk_group(q8, wq8_sb, 1, j, 0, 1.0 / 512.0))
            for hg in range(2):                      # q qc1 (all heads)
                for j in range(2):
                    add_group(f"q{hg}{j}1",
                              qk_group(q8, wq8_sb, hg, j, 1, 1.0 / 512.0))
            done.update(["q000", "q010", "k000", "k010"])   # bootstrap groups

            pending = None
            for qc in range(2):
                for h in range(8):
                    hg, hl = h // 4, h % 4
                    psl = slice(32 * hl, 32 * hl + 32)
                    need(f"q{hg}0{qc}", f"q{hg}1{qc}")
                    cx = ps_cx.tile([65, 512], F32, tag="cx")
                    for tp in range(TP):
                        need(f"k{hg}0{tp // 2}", f"k{hg}1{tp // 2}")
                        sps = ps_sc.tile([P, 1024], F32, tag="sc")
                        for j2 in range(2):
                            nc.tensor.matmul(
                                sps[:, 512 * j2:512 * j2 + 512],
                                k8[psl, hg, 2 * tp + j2, :, :],
                                q8[psl, hg, :, 512 * qc:512 * qc + 512],
                                start=True, stop=True, perf_mode=DR,
                                tile_position=(32 * hl, 0),
                            )
                        p8t = p8p.tile([P, 2, 512], FP8, tag="p8")
                        nc.scalar.activation(p8t[:], sps[:], AF.Exp,
                                             scale=1.0 / qs)
                        need(f"v{2*tp}", f"v{2*tp+1}")
                        nc.tensor.matmul(
                            cx[:], v8[:, h, tp, :, 0:65], p8t[:],
                            start=(tp == 0), stop=(tp == TP - 1), perf_mode=DR,
                        )
                        for _ in range(5):
                            pop_one()
                    if pending is not None:
                        normalize(*pending)
                    pending = (h, qc, cx)
                    if qc == 0 and h == 7:
                        # O-projection of qc0 fills qc1's attention stream
                        normalize(*pending)
                        pending = None
                        for ob in range(KO):
                            add_group(f"o{ob}0", o_steps(ob, 0))
            normalize(*pending)
            while fillers:
                pop_one()
            for ob in range(KO):                     # O-projection tail (qc1)
                for st in o_steps(ob, 1):
                    st()

    nc.finalize()
    return nc


def _build_attn_v3(inv_scale=1.0 / 4096.0):
    """Launch A v3: baseline f32r attention core + hi/lo fp8 DoubleRow
    projections (all-ones-mask / zero-bias fast path).

    Q/K/V projections run as 3 scale-matched fp8 passes (wh*xh, wl*xh,
    wh*xl; x encoded x16 hi and lo, w x256 hi and lo) accumulating in one
    psum: 25% fewer PE cycles than f32r and ~0.13% component error --
    small enough that router top-2 selections stay glued to the reference
    (logit noise ~1e-5 vs min top-2/3 gap ~1.5e-5 after the softmax
    contraction).  Scores, exp, ctx, normalize, and the O-projection are
    bit-identical to the f32r baseline.  x ships only in the DR-stationary
    layout [P, 4dc, 8tt, 2i, 128] (1MB/component vs 4MB f32): Q/K consume
    it as 128-token moving slices, V as stride-128 stationary."""
    import concourse.bacc as bacc
    import concourse.mybir as mybir
    import concourse.tile as tile

    F32, F32R, FP8 = mybir.dt.float32, mybir.dt.float32r, mybir.dt.float8e4
    AF = mybir.ActivationFunctionType
    DR = mybir.MatmulPerfMode.DoubleRow

    nc = bacc.Bacc()
    xh_h = nc.dram_tensor("xh8v", [P, 4, KT, 2, P], FP8, kind="ExternalInput")
    xl_h = nc.dram_tensor("xl8v", [P, 4, KT, 2, P], FP8, kind="ExternalInput")
    wq_h = nc.dram_tensor("wq8", [FB, 2, P, KO, P], FP8, kind="ExternalInput")
    wk_h = nc.dram_tensor("wk8", [FB, 2, P, KO, P], FP8, kind="ExternalInput")
    wv_h = nc.dram_tensor("wv8", [2, P, KO, 512], FP8, kind="ExternalInput")
    wo_h = nc.dram_tensor("wo", [P, FB, H], F32, kind="ExternalInput")
    out_h = nc.dram_tensor("attnp", [H, S], F32, kind="ExternalOutput")

    with tile.TileContext(nc) as tc:
        with tc.tile_pool(name="big", bufs=1) as big, \
             tc.tile_pool(name="wqk_s", bufs=2) as wqk_s, \
             tc.tile_pool(name="work", bufs=3) as work, \
             tc.tile_pool(name="outp", bufs=4) as outp, \
             tc.tile_pool(name="ps_mm", bufs=2, space="PSUM") as ps_mm, \
             tc.tile_pool(name="ps_sc", bufs=3, space="PSUM") as ps_sc, \
             tc.tile_pool(name="ps_cx", bufs=3, space="PSUM") as ps_cx:

            xh8 = big.tile([P, 4, KT, 2, P], FP8)
            xl8 = big.tile([P, 4, KT, 2, P], FP8)
            wv8_sb = big.tile([P, 2, KO, 512], FP8)
            q_t = big.tile([P, FB, S], F32R)
            k_t = big.tile([P, FB, S], F32R)
            ctx_t = big.tile([P, FB, S], F32R)
            v_t = big.tile([P, KT, 8 * 65], F32R)    # 64 cols V + 1 col ones
            wo_sb = big.tile([P, FB, H], F32R)

            # DMA order = first-use order
            def wqk_dma(w_h, fb, tag):
                w_fb = wqk_s.tile([P, 2, KO, P], FP8, tag=tag, name=f"w_{tag}{fb}")
                nc.sync.dma_start(w_fb[:], w_h[fb].rearrange("a p ko c -> p a ko c"))
                return w_fb

            wq0 = wqk_dma(wq_h, 0, "wq")
            wk0 = wqk_dma(wk_h, 0, "wk")
            nc.sync.dma_start(xh8[:, :, 0:4], xh_h[:, :, 0:4])
            nc.sync.dma_start(xl8[:, :, 0:4], xl_h[:, :, 0:4])
            nc.sync.dma_start(wv8_sb[:, 0], wv_h[0])
            nc.sync.dma_start(wv8_sb[:, 1], wv_h[1])
            nc.sync.dma_start(xh8[:, :, 4:8], xh_h[:, :, 4:8])
            nc.sync.dma_start(xl8[:, :, 4:8], xl_h[:, :, 4:8])

            v4 = v_t[:].rearrange("p a (h c) -> p a h c", c=65)
            nc.vector.memset(v4[:, :, :, 64:65].bitcast(F32), 1.0)

            def qk_steps(dst, w_fb, fb, qc):
                """One Q/K hi/lo projection psum group as steps: per token
                tile, 12 DR matmuls (3 passes x 4 dc) share one psum."""
                box = {}
                wv_ = w_fb[:].rearrange("p a (dc i) m -> p a dc i m", i=2)

                def step(st):
                    if st == 0:
                        box["t"] = ps_mm.tile([P, 512], F32, tag="mm",
                                              name=f"qkps{fb}_{qc}")
                    if st < 4:
                        tt = 4 * qc + st
                        for pa, (wa, xa) in enumerate(
                                ((0, xh8), (1, xh8), (0, xl8))):
                            for dc in range(4):
                                nc.tensor.matmul(
                                    box["t"][:, 128 * st:128 * st + 128],
                                    wv_[:, wa, dc], xa[:, dc, tt],
                                    start=(pa == 0 and dc == 0),
                                    stop=(pa == 2 and dc == 3), perf_mode=DR,
                                )
                    else:
                        nc.vector.tensor_scalar_mul(
                            dst[:, fb, qc * 512:(qc + 1) * 512], box["t"][:],
                            inv_scale)

                return [lambda d=d: step(d) for d in range(5)]

            def v_steps(tt):
                """V hi/lo projection for one key tile (token-major psum)."""
                box = {}
                wvv = wv8_sb[:].rearrange("p a (dc i) m -> p a dc i m", i=2)

                def step(st):
                    if st == 0:
                        box["t"] = ps_mm.tile([P, 512], F32, tag="mm",
                                              name=f"vps{tt}")
                    if st < 3:
                        xa = (xh8, xl8, xh8)[st]
                        wa = (0, 0, 1)[st]
                        for dc in range(4):
                            nc.tensor.matmul(
                                box["t"][:], xa[:, dc, tt], wvv[:, wa, dc],
                                start=(st == 0 and dc == 0),
                                stop=(st == 2 and dc == 3), perf_mode=DR,
                            )
                    else:
                        nc.vector.tensor_scalar_mul(
                            v4[:, tt, :, 0:64],
                            box["t"][:].rearrange("p (h c) -> p h c", c=64),
                            inv_scale)

                return [lambda d=d: step(d) for d in range(4)]

            def o_steps(ob, qc):
                box = {}

                def step(i):
                    if i == 0:
                        box["t"] = ps_mm.tile([P, 512], F32, tag="mm",
                                              name=f"ops{ob}_{qc}")
                    if i < FB:
                        nc.tensor.matmul(
                            box["t"][:], wo_sb[:, i, ob * P:(ob + 1) * P],
                            ctx_t[:, i, qc * 512:(qc + 1) * 512],
                            start=(i == 0), stop=(i == FB - 1),
                        )
                    elif i == FB:
                        box["o"] = outp.tile([P, 512], F32, tag="o",
                                             name=f"oh{ob}_{qc}")
                        if qc == 1 and ob % 2 == 0:
                            # tail: Act is idle and can read psum
                            nc.scalar.copy(box["o"][:], box["t"][:])
                        else:
                            nc.vector.tensor_copy(box["o"][:], box["t"][:])
                    else:
                        nc.sync.dma_start(
                            out_h[:].rearrange("(ko p) t -> p ko t", p=P)[
                                :, ob, qc * 512:(qc + 1) * 512],
                            box["o"][:],
                        )

                return [lambda i=i: step(i) for i in range(FB + 2)]

            def normalize(fb, qc, cx):
                for hh in range(2):
                    rcp = work.tile([1, 512], F32, tag="rcp")
                    nc.vector.reciprocal(rcp[:], cx[hh][64:65, :])
                    rb_sb = work.tile([64, 512], F32, tag="rb_sb")
                    nc.gpsimd.partition_broadcast(rb_sb[:], rcp[:])
                    nc.vector.tensor_mul(
                        ctx_t[hh * DH:(hh + 1) * DH, fb, qc * 512:(qc + 1) * 512],
                        cx[hh][0:64, :], rb_sb[:],
                    )

            # Named filler groups with forced prerequisite draining
            fillers = []
            done = set()

            def add_group(name, steps):
                fillers.extend(steps)
                fillers.append(name)

            def pop_one():
                while fillers and isinstance(fillers[0], str):
                    done.add(fillers.pop(0))
                if fillers:
                    fillers.pop(0)()
                while fillers and isinstance(fillers[0], str):
                    done.add(fillers.pop(0))

            def need(*names):
                while any(n not in done for n in names):
                    assert fillers, f"missing groups: {names}"
                    pop_one()

            # Bootstrap: pair-0 queries (first half) + keys (first half)
            for st in qk_steps(q_t, wq0, 0, 0):
                st()
            for st in qk_steps(k_t, wk0, 0, 0):
                st()
            done.update(["q00", "k00"])
            nc.sync.dma_start(wo_sb[:], wo_h[:].bitcast(F32R))

            add_group("k01", qk_steps(k_t, wk0, 0, 1))
            for tt in range(4):
                add_group(f"v{tt}", v_steps(tt))
            add_group("q01", qk_steps(q_t, wq0, 0, 1))
            for tt in range(4, 8):
                add_group(f"v{tt}", v_steps(tt))
            for fb in range(1, FB):
                wqf = wqk_dma(wq_h, fb, "wq")
                wkf = wqk_dma(wk_h, fb, "wk")
                add_group(f"k{fb}0", qk_steps(k_t, wkf, fb, 0))
                add_group(f"k{fb}1", qk_steps(k_t, wkf, fb, 1))
                add_group(f"q{fb}0", qk_steps(q_t, wqf, fb, 0))
                add_group(f"q{fb}1", qk_steps(q_t, wqf, fb, 1))

            pending = None
            for qc in range(QC):
                for fb in range(FB):
                    need(f"q{fb}{qc}", f"k{fb}0", f"k{fb}1")
                    cx0 = ps_cx.tile([65, 512], F32, tag="cx")
                    cx1 = ps_cx.tile([65, 512], F32, tag="cx")
                    cx = (cx0, cx1)
                    # ctx lags its scores/exp by one (kt,hh) unit so the PE
                    # queue never head-blocks waiting for the Act exp
                    ctx_q = []
                    for kt in range(KT):
                        need(f"v{kt}")
                        for hh in range(2):
                            sps = ps_sc.tile([P, 512], F32, tag="sc")
                            nc.tensor.matmul(
                                sps[:],
                                k_t[hh * DH:(hh + 1) * DH, fb, kt * P:(kt + 1) * P],
                                q_t[hh * DH:(hh + 1) * DH, fb, qc * 512:(qc + 1) * 512],
                                start=True, stop=True,
                            )
                            p_sb = work.tile([P, 512], F32R, tag="p")
                            nc.scalar.activation(p_sb[:], sps[:], AF.Exp)

                            def ctx_mm(kt=kt, hh=hh, p_sb=p_sb):
                                nc.tensor.matmul(
                                    cx[hh][:],
                                    v_t[:, kt, (2 * fb + hh) * 65:(2 * fb + hh + 1) * 65],
                                    p_sb[:],
                                    start=(kt == 0), stop=(kt == KT - 1),
                                )

                            ctx_q.append(ctx_mm)
                            if len(ctx_q) > 1:
                                ctx_q.pop(0)()
                            if (2 * kt + hh) % 4 != 3:
                                pop_one()
                    ctx_q.pop(0)()
                    if pending is not None:
                        normalize(*pending)
                    pending = (fb, qc, cx)
                    if qc == 0 and fb == FB - 1:
                        # O-projection of qc0 fills qc1's attention stream
                        normalize(*pending)
                        pending = None
                        for ob in range(KO):
                            add_group(f"o{ob}0", o_steps(ob, 0))
            normalize(*pending)
            while fillers:
                pop_one()
            for ob in range(KO):                     # O-projection tail (qc1)
                for st in o_steps(ob, 1):
                    st()

    nc.finalize()
    return nc


def _build_expert_v2(C, CN, w2_hilo=False):
    """Launch B v2: one expert FFN over C routed tokens, feature-major in/out.
    W1 pass: 3 fp8-e4m3 DoubleRow passes over host-split hi/lo of x and W1
    (the combine reads both psums directly).  h is stored as single fp8
    (scale 1: gelu output magnitudes sit in e4m3's normal range).  W2 pass:
    fp8 DoubleRow — single pass over W2*64 (w2_hilo=False) or two passes
    over hi/lo-split W2 for ~2x tighter output error."""
    import concourse.bacc as bacc
    import concourse.mybir as mybir
    import concourse.tile as tile

    F32, F32R = mybir.dt.float32, mybir.dt.float32r
    FP8 = mybir.dt.float8e4
    AF = mybir.ActivationFunctionType
    DR = mybir.MatmulPerfMode.DoubleRow
    NCH = C // CN

    nc = bacc.Bacc()
    xh_h = nc.dram_tensor("xh", [P, KO, C], FP8, kind="ExternalInput")   # LN2(x)^T hi
    xl_h = nc.dram_tensor("xl", [P, KO, C], FP8, kind="ExternalInput")   # LN2(x)^T lo
    w1_h = nc.dram_tensor("w1", [MF, P, 2, KO, P], FP8, kind="ExternalInput")
    NW2 = 2 if w2_hilo else 1
    w2_h = nc.dram_tensor("w2", [KO, P, NW2, MF, P], FP8, kind="ExternalInput")
    b1_h = nc.dram_tensor("b1c", [P, MF], F32, kind="ExternalInput")
    b2_h = nc.dram_tensor("b2c", [P, KO], F32, kind="ExternalInput")
    g_h = nc.dram_tensor("gates", [1, C], F32, kind="ExternalInput")
    BF16 = mybir.dt.bfloat16
    y_h = nc.dram_tensor("y", [H, C], BF16, kind="ExternalOutput")       # gated expert out^T

    with tile.TileContext(nc) as tc:
        with tc.tile_pool(name="consts", bufs=1) as consts, \
             tc.tile_pool(name="big", bufs=1) as big, \
             tc.tile_pool(name="w1s", bufs=4) as w1s, \
             tc.tile_pool(name="w2s", bufs=8) as w2s, \
             tc.tile_pool(name="work", bufs=2) as work, \
             tc.tile_pool(name="ps_mm", bufs=3, space="PSUM") as ps_mm, \
             tc.tile_pool(name="ps_gb", bufs=1, space="PSUM") as ps_gb:

            ones_f = consts.tile([1, P], F32)
            nc.vector.memset(ones_f[:], 1.0)
            ones_row = consts.tile([1, P], F32R)
            nc.vector.tensor_copy(ones_row[:], ones_f[:])

            # x first (hi then the first weight chunk then lo), then the
            # tiny bias/gate tensors (needed only once compute is rolling)
            x_hi = big.tile([P, KO, C], FP8)
            nc.sync.dma_start(x_hi[:], xh_h[:])
            w1_first = w1s.tile([P, 2, KO, P], FP8, tag="w1")
            nc.sync.dma_start(w1_first[:], w1_h[0])
            x_lo = big.tile([P, KO, C], FP8)
            nc.sync.dma_start(x_lo[:], xl_h[:])
            xhv = x_hi[:].rearrange("p (dc i) t -> p dc i t", i=2)
            xlv = x_lo[:].rearrange("p (dc i) t -> p dc i t", i=2)
            b1t = consts.tile([P, MF], F32)
            nc.sync.dma_start(b1t[:], b1_h[:])
            b2t = consts.tile([P, KO], F32)
            nc.sync.dma_start(b2t[:], b2_h[:])
            g_sb = consts.tile([1, C], F32R)
            nc.sync.dma_start(g_sb[:], g_h[:].bitcast(F32R))

            # ---- W1 pass (fp8 hi/lo, 3 DoubleRow passes) + gelu -> fp8 h ----
            # All hi/lo components share one psum scale (x: x16, w1: x1024 —
            # the lo parts are encoded at the same scale as the hi parts, which
            # fp8's floating format permits), so the 12 matmuls accumulate in a
            # single psum and gelu reads it directly: no DVE combine.
            # W2 chunks prefetch through the (compute-bound) W1 phase so the
            # W2 phase never waits on the DMA bus.
            h_t = big.tile([P, MF, C], FP8)
            w2_tiles = []
            for mf in range(MF):
                if mf == 0:
                    w1_mf = w1_first
                else:
                    w1_mf = w1s.tile([P, 2, KO, P], FP8, tag="w1")
                    nc.sync.dma_start(w1_mf[:], w1_h[mf])
                if mf % 4 == 3:
                    oh = mf // 4
                    w2t = w2s.tile([P, NW2, MF, P], FP8, tag="w2",
                                   name=f"w2c{oh}")
                    nc.sync.dma_start(w2t[:], w2_h[oh])
                    w2_tiles.append(w2t)
                w1v = w1_mf[:].rearrange("p a (dc i) m -> p a dc i m", i=2)
                for nch in range(NCH):
                    sl = slice(nch * CN, (nch + 1) * CN)
                    psa = ps_mm.tile([P, CN], F32, tag="mm")
                    for dc in range(4):
                        nc.tensor.matmul(
                            psa[:], w1v[:, 0, dc], xhv[:, dc, :, sl],
                            start=(dc == 0), stop=False, perf_mode=DR,
                        )
                    for dc in range(4):
                        nc.tensor.matmul(
                            psa[:], w1v[:, 1, dc], xhv[:, dc, :, sl],
                            start=False, stop=False, perf_mode=DR,
                        )
                    for dc in range(4):
                        nc.tensor.matmul(
                            psa[:], w1v[:, 0, dc], xlv[:, dc, :, sl],
                            start=False, stop=(dc == 3), perf_mode=DR,
                        )
                    nc.scalar.activation(
                        h_t[:, mf, sl], psa[:],
                        AF.Gelu_apprx_tanh, bias=b1t[:, mf:mf + 1],
                        scale=1.0 / 16384.0,
                    )

            # gate row broadcast to all partitions (needed from W2 phase on)
            gb_sb = big.tile([P, C], F32)
            for nch in range(NCH):
                gps = ps_gb.tile([P, CN], F32, tag="gb")
                nc.tensor.matmul(gps[:], ones_row[:], g_sb[:, nch * CN:(nch + 1) * CN],
                                 start=True, stop=True)
                nc.vector.tensor_copy(gb_sb[:, nch * CN:(nch + 1) * CN], gps[:])

            # ---- W2 pass (fp8 DoubleRow) + bias + gate ----
            hv = h_t[:].rearrange("p (dc i) t -> p dc i t", i=2)
            for oh in range(KO):
                w2_oh = w2_tiles[oh]
                wv2 = w2_oh[:].rearrange("p a (dc i) m -> p a dc i m", i=2)
                y_sb = work.tile([P, C], BF16, tag="y")
                for nch in range(NCH):
                    sl = slice(nch * CN, (nch + 1) * CN)
                    yps = ps_mm.tile([P, CN], F32, tag="mm")
                    for a in range(NW2):
                        for dc in range(MF // 2):
                            nc.tensor.matmul(
                                yps[:], wv2[:, a, dc], hv[:, dc, :, sl],
                                start=(a == 0 and dc == 0),
                                stop=(a == NW2 - 1 and dc == MF // 2 - 1),
                                perf_mode=DR,
                            )
                    ytmp = work.tile([P, CN], F32, tag="ytmp")
                    nc.scalar.activation(ytmp[:], yps[:], AF.Identity,
                                         bias=b2t[:, oh:oh + 1], scale=1.0 / 64.0)
                    nc.vector.tensor_mul(
                        y_sb[:, sl], ytmp[:], gb_sb[:, sl],
                    )
                    nc.sync.dma_start(
                        y_h[:].rearrange("(ko p) t -> p ko t", p=P)[:, oh, sl],
                        y_sb[:, sl],
                    )

    nc.finalize()
    return nc


def _build_expert(C, CN):
    """Launch B program: one expert FFN over C routed tokens, feature-major
    in/out.  The W1 pass runs as 3 fp8-e4m3 DoubleRow passes over host-split
    hi/lo components of x and W1 (x: x16 / x256, W1: x1024 / x16384; the two
    cross products share psum scale 2^18, hi*hi is 2^14) — more accurate than
    bf16 and 25% fewer PE cycles.  h and the W2 pass stay bf16."""
    import concourse.bacc as bacc
    import concourse.mybir as mybir
    import concourse.tile as tile

    F32, F32R, BF16 = mybir.dt.float32, mybir.dt.float32r, mybir.dt.bfloat16
    FP8 = mybir.dt.float8e4
    AF = mybir.ActivationFunctionType
    DR = mybir.MatmulPerfMode.DoubleRow
    NCH = C // CN

    nc = bacc.Bacc()
    xh_h = nc.dram_tensor("xh", [P, KO, C], FP8, kind="ExternalInput")   # LN2(x)^T hi
    xl_h = nc.dram_tensor("xl", [P, KO, C], FP8, kind="ExternalInput")   # LN2(x)^T lo
    w1_h = nc.dram_tensor("w1", [MF, P, 2, KO, P], FP8, kind="ExternalInput")
    w2_h = nc.dram_tensor("w2", [KO, P, MF, P], BF16, kind="ExternalInput")
    b1_h = nc.dram_tensor("b1c", [P, MF], F32, kind="ExternalInput")
    b2_h = nc.dram_tensor("b2c", [P, KO], F32, kind="ExternalInput")
    g_h = nc.dram_tensor("gates", [1, C], F32, kind="ExternalInput")
    y_h = nc.dram_tensor("y", [H, C], F32, kind="ExternalOutput")        # gated expert out^T

    with tile.TileContext(nc) as tc:
        with tc.tile_pool(name="consts", bufs=1) as consts, \
             tc.tile_pool(name="big", bufs=1) as big, \
             tc.tile_pool(name="w1s", bufs=4) as w1s, \
             tc.tile_pool(name="w2s", bufs=8) as w2s, \
             tc.tile_pool(name="work", bufs=2) as work, \
             tc.tile_pool(name="ps_mm", bufs=3, space="PSUM") as ps_mm, \
             tc.tile_pool(name="ps_gb", bufs=1, space="PSUM") as ps_gb:

            ones_f = consts.tile([1, P], F32)
            nc.vector.memset(ones_f[:], 1.0)
            ones_row = consts.tile([1, P], F32R)
            nc.vector.tensor_copy(ones_row[:], ones_f[:])

            # x first (hi then the first weight chunk then lo), then the
            # tiny bias/gate tensors (needed only once compute is rolling)
            x_hi = big.tile([P, KO, C], FP8)
            nc.sync.dma_start(x_hi[:], xh_h[:])
            w1_first = w1s.tile([P, 2, KO, P], FP8, tag="w1")
            nc.sync.dma_start(w1_first[:], w1_h[0])
            x_lo = big.tile([P, KO, C], FP8)
            nc.sync.dma_start(x_lo[:], xl_h[:])
            xhv = x_hi[:].rearrange("p (dc i) t -> p dc i t", i=2)
            xlv = x_lo[:].rearrange("p (dc i) t -> p dc i t", i=2)
            b1t = consts.tile([P, MF], F32)
            nc.sync.dma_start(b1t[:], b1_h[:])
            b2t = consts.tile([P, KO], F32)
            nc.sync.dma_start(b2t[:], b2_h[:])
            g_sb = consts.tile([1, C], F32R)
            nc.sync.dma_start(g_sb[:], g_h[:].bitcast(F32R))

            # ---- W1 pass (fp8 hi/lo, 3 DoubleRow passes) + gelu ----
            h_t = big.tile([P, MF, C], BF16)
            for mf in range(MF):
                if mf == 0:
                    w1_mf = w1_first
                else:
                    w1_mf = w1s.tile([P, 2, KO, P], FP8, tag="w1")
                    nc.sync.dma_start(w1_mf[:], w1_h[mf])
                w1v = w1_mf[:].rearrange("p a (dc i) m -> p a dc i m", i=2)
                for nch in range(NCH):
                    sl = slice(nch * CN, (nch + 1) * CN)
                    psa = ps_mm.tile([P, CN], F32, tag="mmA", bufs=2)
                    for dc in range(4):
                        nc.tensor.matmul(
                            psa[:], w1v[:, 0, dc], xhv[:, dc, :, sl],
                            start=(dc == 0), stop=(dc == 3), perf_mode=DR,
                        )
                    psb = ps_mm.tile([P, CN], F32, tag="mmB", bufs=2)
                    for dc in range(4):
                        nc.tensor.matmul(
                            psb[:], w1v[:, 1, dc], xhv[:, dc, :, sl],
                            start=(dc == 0), stop=False, perf_mode=DR,
                        )
                    for dc in range(4):
                        nc.tensor.matmul(
                            psb[:], w1v[:, 0, dc], xlv[:, dc, :, sl],
                            start=False, stop=(dc == 3), perf_mode=DR,
                        )
                    psa_sb = work.tile([P, CN], F32, tag="psa_sb")
                    nc.vector.tensor_copy(psa_sb[:], psa[:])
                    cmb = work.tile([P, CN], F32, tag="cmb")
                    nc.vector.scalar_tensor_tensor(
                        cmb[:], psb[:], 1.0 / 16.0, psa_sb[:],
                        mybir.AluOpType.mult, mybir.AluOpType.add,
                    )
                    nc.scalar.activation(
                        h_t[:, mf, sl], cmb[:],
                        AF.Gelu_apprx_tanh, bias=b1t[:, mf:mf + 1],
                        scale=1.0 / 16384.0,
                    )

            # gate row broadcast to all partitions (needed from W2 phase on)
            gb_sb = big.tile([P, C], F32)
            for nch in range(NCH):
                gps = ps_gb.tile([P, CN], F32, tag="gb")
                nc.tensor.matmul(gps[:], ones_row[:], g_sb[:, nch * CN:(nch + 1) * CN],
                                 start=True, stop=True)
                nc.vector.tensor_copy(gb_sb[:, nch * CN:(nch + 1) * CN], gps[:])

            # ---- W2 pass + bias + gate ----
            for oh in range(KO):
                w2_oh = w2s.tile([P, MF, P], BF16, tag="w2")
                nc.sync.dma_start(w2_oh[:], w2_h[oh])
                y_sb = work.tile([P, C], F32, tag="y")
                for nch in range(NCH):
                    yps = ps_mm.tile([P, CN], F32, tag="mm")
                    for kc2 in range(MF):
                        nc.tensor.matmul(
                            yps[:], w2_oh[:, kc2, :], h_t[:, kc2, nch * CN:(nch + 1) * CN],
                            start=(kc2 == 0), stop=(kc2 == MF - 1),
                        )
                    ytmp = work.tile([P, CN], F32, tag="ytmp")
                    nc.scalar.activation(ytmp[:], yps[:], AF.Identity, bias=b2t[:, oh:oh + 1])
                    nc.vector.tensor_mul(
                        y_sb[:, nch * CN:(nch + 1) * CN], ytmp[:],
                        gb_sb[:, nch * CN:(nch + 1) * CN],
                    )
                    nc.sync.dma_start(
                        y_h[:].rearrange("(ko p) t -> p ko t", p=P)[
                            :, oh, nch * CN:(nch + 1) * CN],
                        y_sb[:, nch * CN:(nch + 1) * CN],
                    )

    nc.finalize()
    return nc


def _get_attn(wb=True):
    key = ("attn", wb)
    if key not in _CACHE:
        _CACHE[key] = _build_attn(wb)
    return _CACHE[key]


def _get_attn_v3(inv_scale):
    key = ("attn_v3", inv_scale)
    if key not in _CACHE:
        _CACHE[key] = _build_attn_v3(inv_scale)
    return _CACHE[key]


def _get_expert(C, CN, mode):
    key = ("exp", C, CN, mode)
    if key not in _CACHE:
        if mode == "fp8":
            _CACHE[key] = _build_expert_fp8(C, CN)
        elif mode == "hilo":
            _CACHE[key] = _build_expert(C, CN)
        else:
            _CACHE[key] = _build_expert_v2(C, CN, w2_hilo=(mode == "v2hilo"))
    return _CACHE[key]


def _ln(x64):
    m = x64.mean(-1, keepdims=True)
    v = x64.var(-1, keepdims=True)
    return (x64 - m) / np.sqrt(v + EPS)


def _bf16(a):
    import ml_dtypes
    return np.ascontiguousarray(np.asarray(a).astype(ml_dtypes.bfloat16))


def _fp8(a):
    import ml_dtypes
    return np.ascontiguousarray(np.asarray(a).astype(ml_dtypes.float8_e4m3))


def _pko(a2d, x):
    """[H-like, X] row-major -> [P, n, X] SBUF tile layout (casts to f32)."""
    n = a2d.shape[0] // P
    return np.ascontiguousarray(
        np.asarray(a2d, dtype=np.float32).reshape(n, P, x).transpose(1, 0, 2))


def _pkod(a2d, x):
    """Same as _pko but dtype-preserving."""
    a = np.asarray(a2d)
    n = a.shape[0] // P
    return np.ascontiguousarray(a.reshape(n, P, x).transpose(1, 0, 2))


def kernel(**inputs):
    import os as _os
    import time as _time
    from concourse.bass_utils import run_bass_kernel_spmd

    f = lambda k: np.asarray(inputs[k], dtype=np.float32)
    x = f("hidden_states")                       # [B, S, H]
    mask = np.asarray(inputs["attention_mask"])  # [B, S] int32
    ln1_g, ln1_b = f("ln1_g").astype(np.float64), f("ln1_b").astype(np.float64)
    ln2_g, ln2_b = f("ln2_g").astype(np.float64), f("ln2_b").astype(np.float64)
    Wq, Wk, Wv, Wo = (f(k).astype(np.float64) for k in ("Wq", "Wk", "Wv", "Wo"))
    bq, bk, bv, bo = (f(k).astype(np.float64) for k in ("bq", "bk", "bv", "bo"))
    level_logits = f("level_logits").astype(np.float64)
    Wr, br = f("Wr").astype(np.float64), f("br").astype(np.float64)
    W1, b1 = f("W1").astype(np.float64), f("b1").astype(np.float64)
    W2, b2 = f("W2").astype(np.float64), f("b2").astype(np.float64)

    # ---- host folding ----
    scale = 1.0 / np.sqrt(DH)
    wq_eff = (ln1_g[None, :, None] * Wq) * scale              # [L,H,H]
    bq_eff = (bq + ln1_b @ Wq) * scale                        # [L,H]
    wk_eff = ln1_g[None, :, None] * Wk
    bk_eff = bk + ln1_b @ Wk
    wv_eff = ln1_g[None, :, None] * Wv
    bv_eff = bv + ln1_b @ Wv                                  # folded into boc below
    lw = np.exp(level_logits - level_logits.max())
    lw = lw / lw.sum()                                        # softmax(level_logits)
    wo_eff = lw[:, None, None] * Wo
    boc_eff = np.einsum("l,lh->h", lw, bo) + np.einsum("lf,lfh->h", bv_eff, wo_eff)

    xn1 = _ln(x.astype(np.float64)).astype(np.float32)        # LN1 (gamma/beta folded)

    def colt(vec):  # [H or F] -> [P, n] per-partition column layout
        v32 = np.ascontiguousarray(np.asarray(vec, dtype=np.float32))
        return np.ascontiguousarray(v32.reshape(-1, P).T)

    mbias = ((1.0 - mask.astype(np.float32)) * np.float32(-1e9))  # [B,S]
    xn1_T = np.swapaxes(xn1, 1, 2)                            # [B,H,S]

    wb = max(float(np.abs(np.concatenate([bq_eff, bk_eff], -1)).max()),
             0.0) > 0.0
    mask_ones = bool((mask == 1).all())
    attn_mode = _os.environ.get(
        "KERNEL_ATTN_MODE", "v3" if (mask_ones and not wb) else "v1")
    _PERF["attn_mode"] = attn_mode

    if attn_mode == "v3":
        xsc = np.float32(_os.environ.get("KERNEL_ATTN_XS", "16"))
        wsc = np.float32(_os.environ.get("KERNEL_ATTN_WS", "256"))
        _PERF["attn_scales"] = (float(xsc), float(wsc))

        def hilo(a32, scale):
            hi = _fp8(a32 * scale)
            lo = _fp8((a32 - hi.astype(np.float32) / scale) * scale)
            return hi, lo

        def xtile(x8):            # [H, S] fp8 -> [P, 4dc, KT, 2i, P]
            return np.ascontiguousarray(
                x8.reshape(4, 2, P, KT, P).transpose(2, 0, 3, 1, 4))

        def wtile(w8):            # [H, 512] fp8 -> [FB-chunks of [P, KO, P]]
            return [_pkod(np.ascontiguousarray(w8[:, fb * P:(fb + 1) * P]), P)
                    for fb in range(FB)]

        in_maps = []
        for c in range(NCORES):
            b, l, hh = c >> 2, (c >> 1) & 1, c & 1
            sl = slice(hh * 512, (hh + 1) * 512)
            xh8, xl8 = hilo(xn1_T[b].astype(np.float32), xsc)
            wqh, wql = hilo(wq_eff[l][:, sl].astype(np.float32), wsc)
            wkh, wkl = hilo(wk_eff[l][:, sl].astype(np.float32), wsc)
            wvh, wvl = hilo(wv_eff[l][:, sl].astype(np.float32), wsc)
            in_maps.append({
                "xh8v": xtile(xh8),
                "xl8v": xtile(xl8),
                "wq8": np.ascontiguousarray(
                    np.stack([np.stack(wtile(w)) for w in (wqh, wql)], axis=1)),
                "wk8": np.ascontiguousarray(
                    np.stack([np.stack(wtile(w)) for w in (wkh, wkl)], axis=1)),
                "wv8": np.ascontiguousarray(
                    np.stack([_pkod(w, 512) for w in (wvh, wvl)])),
                "wo": _pko(wo_eff[l][sl, :].astype(np.float32), H),
            })
        nc_a = _get_attn_v3(1.0 / float(xsc * wsc))
    else:
        in_maps = []
        for c in range(NCORES):
            b, l, hh = c >> 2, (c >> 1) & 1, c & 1
            sl = slice(hh * 512, (hh + 1) * 512)
            wq32 = wq_eff[l][:, sl].astype(np.float32)        # [H,512]
            wk32 = wk_eff[l][:, sl].astype(np.float32)
            in_maps.append({
                "xn": _pko(xn1_T[b], S),
                "wq": np.ascontiguousarray(
                    _pko(wq32, 512).reshape(P, KO, FB, P).transpose(2, 0, 1, 3)),
                "wk": np.ascontiguousarray(
                    _pko(wk32, 512).reshape(P, KO, FB, P).transpose(2, 0, 1, 3)),
                "wv": _pko(wv_eff[l][:, sl].astype(np.float32), 512),
                "wo": _pko(wo_eff[l][sl, :].astype(np.float32), H),
                "bqk": np.concatenate([bq_eff[l][sl], bk_eff[l][sl]]).astype(np.float32)[None, :],
                "mb": colt(mbias[b]),
            })
        nc_a = _get_attn(wb)
    t0 = _time.time()
    res_a = run_bass_kernel_spmd(nc_a, in_maps, core_ids=list(range(NCORES)))
    _PERF["a_wall_s"] = _time.time() - t0
    _PERF["attn_wb"] = wb
    _PERF["a_exec_ns"] = res_a.exec_time_ns

    # ---- host: combine partials, residual, LN2, router, top-2 routing ----
    xres = x.astype(np.float64)                                # [B,S,H]
    for c in range(NCORES):
        b = c >> 2
        xres[b] += res_a.results[c]["attnp"].astype(np.float64).T
    xres += boc_eff[None, None, :]
    xres = xres.reshape(B * S, H)

    xn2 = _ln(xres)                                           # [B*S, H] (gamma/beta folded)
    logits = xn2 @ (ln2_g[:, None] * Wr) + (br + ln2_b @ Wr)  # [B*S, E]
    pm = logits.max(-1, keepdims=True)
    probs = np.exp(logits - pm)
    probs /= probs.sum(-1, keepdims=True)
    order = np.argsort(-probs, axis=-1, kind="stable")
    topi = order[:, :2]                                       # [T,2]
    topv = np.take_along_axis(probs, topi, axis=-1)
    gates = topv / topv.sum(-1, keepdims=True)                # [T,2]

    ps = np.sort(probs, axis=-1)
    _PERF["router_gap23"] = float((ps[:, -2] - ps[:, -3]).min())
    _PERF["topi"] = topi.copy()

    tok_idx, gate_val = [], []
    for e in range(E):
        sel = np.nonzero(topi == e)
        tok_idx.append(sel[0])
        gate_val.append(gates[sel[0], sel[1]])
    counts = [len(t) for t in tok_idx]
    C = max(512, ((max(counts) + 3) // 4) * 4)
    while True:  # need NCH with C % NCH == 0 and 256 <= C/NCH <= 512
        nch = (C + 511) // 512
        if C % nch == 0 and C // nch >= 256:
            break
        C += 4
    CN = C // ((C + 511) // 512)

    w1f = ln2_g[None, :, None] * W1                           # [E,H,F]
    b1f = b1 + ln2_b @ W1                                     # [E,F]
    xn2_T32 = np.ascontiguousarray(xn2.T.astype(np.float32))  # [H, B*S]

    mode = _os.environ.get("KERNEL_MOE_MODE", "v2")
    if mode != "fp8":
        # hi/lo fp8 split of the LN2 output for the W1 pass (done once).
        # v2 modes encode lo at the SAME scale as hi (x16) so the hi and lo
        # passes accumulate in one psum; the old "hilo" kernel wants x256.
        xh_full = _fp8(xn2_T32 * np.float32(16.0))
        xlo_scale = np.float32(256.0) if mode == "hilo" else np.float32(16.0)
        xl_full = _fp8(
            (xn2_T32 - xh_full.astype(np.float32) / np.float32(16.0)) * xlo_scale)
    in_maps_b = []
    for e in range(E):
        g = np.zeros((1, C), np.float32)
        g[0, :counts[e]] = gate_val[e].astype(np.float32)
        if mode == "fp8":
            xt = np.zeros((H, C), np.float32)
            xt[:, :counts[e]] = xn2_T32[:, tok_idx[e]]
            w1_32 = (w1f[e] * 64.0).astype(np.float32)        # [H,FF]
            w2_32 = (W2[e] * 64.0).astype(np.float32)         # [FF,H]
            in_maps_b.append({
                "xt": _fp8(_pko(xt, C)),
                "w1": _fp8(_pko(w1_32, FF).reshape(P, KO, MF, P).transpose(2, 0, 1, 3)),
                "w2": _fp8(_pko(w2_32, H).reshape(P, MF, KO, P).transpose(2, 0, 1, 3)),
                "b1c": colt(b1f[e]), "b2c": colt(b2[e]), "gates": g,
            })
            continue
        xh = np.zeros((H, C), xh_full.dtype)
        xh[:, :counts[e]] = xh_full[:, tok_idx[e]]
        xl = np.zeros((H, C), xl_full.dtype)
        xl[:, :counts[e]] = xl_full[:, tok_idx[e]]
        w1_32 = w1f[e].astype(np.float32)                     # [H,FF]
        w1h = _fp8(w1_32 * np.float32(1024.0))
        w1lo_scale = np.float32(16384.0) if mode == "hilo" else np.float32(1024.0)
        w1l = _fp8((w1_32 - w1h.astype(np.float32) / np.float32(1024.0))
                   * w1lo_scale)
        w1h_t = _pkod(w1h, FF).reshape(P, KO, MF, P).transpose(2, 0, 1, 3)
        w1l_t = _pkod(w1l, FF).reshape(P, KO, MF, P).transpose(2, 0, 1, 3)
        imap = {
            "xh": _pkod(xh, C),
            "xl": _pkod(xl, C),
            "w1": np.ascontiguousarray(np.stack([w1h_t, w1l_t], axis=2)),
            "b1c": colt(b1f[e]),
            "b2c": colt(b2[e]),
            "gates": g,
        }
        if mode == "hilo":
            imap["w2"] = _bf16(_pko(W2[e].astype(np.float32), H)
                               .reshape(P, MF, KO, P).transpose(2, 0, 1, 3))
        else:
            # fp8 W2, prescaled x64 (undone by the output activation scale)
            w2_32 = W2[e].astype(np.float32)                  # [FF,H]
            w2h = _fp8(w2_32 * np.float32(64.0))
            parts = [_pkod(w2h, H).reshape(P, MF, KO, P).transpose(2, 0, 1, 3)]
            if mode == "v2hilo":
                # lo at the same x64 scale as hi: single-psum accumulation
                w2l = _fp8((w2_32 - w2h.astype(np.float32) / np.float32(64.0))
                           * np.float32(64.0))
                parts.append(_pkod(w2l, H).reshape(P, MF, KO, P).transpose(2, 0, 1, 3))
            imap["w2"] = np.ascontiguousarray(np.stack(parts, axis=2))
        in_maps_b.append(imap)

    nc_b = _get_expert(C, CN, mode)
    t0 = _time.time()
    res_b = run_bass_kernel_spmd(nc_b, in_maps_b, core_ids=list(range(NCORES)))
    _PERF["b_wall_s"] = _time.time() - t0
    _PERF["b_exec_ns"] = res_b.exec_time_ns
    _PERF["capacity"] = C
    _PERF["counts"] = counts
    _PERF["moe_mode"] = mode

    if _os.environ.get("KERNEL_STASH"):
        _PERF["a_prog"] = (nc_a, in_maps)
        _PERF["b_prog"] = (nc_b, in_maps_b)

    out = xres.copy()
    for e in range(E):
        if counts[e]:
            out[tok_idx[e]] += res_b.results[e]["y"][:, :counts[e]].astype(np.float64).T
    return out.reshape(B, S, H).astype(np.float32)



# revision 33
# speedup vs baseline: 1.2854x; 1.0363x over previous
"""Trainium2 Bass kernel for nn_MoEMLABlock (MoE + multi-level attention block).

Strategy (8 NeuronCores, full inputs in / full output out):
  Launch A (attention, sharded over batch x level x head-half): core
    c = (b, l, hh) computes, for batch b, level l, heads hh*8..hh*8+7:
    Q/K/V projections over all 1024 tokens, softmax attention, and the
    partial O-projection [H, S] (feature-major).  No K/V recompute across
    cores.  LayerNorm 1 runs on the host (fp64) with gamma/beta folded
    into the projection weights; 1/sqrt(DH), the softmax level weights,
    and all biases are folded on the host.  Q/K biases enter the
    projection matmul as an extra ones-row contraction term; V bias and
    the O bias fold into a single per-batch constant added on the host.
    The softmax denominator is produced by the context matmul itself via
    a ones-column appended to V (psum row 64 = sumexp).  All device
    tensors arrive pre-laid-out in SBUF tile order so every DMA is one
    descriptor per partition.
  Host: sum the 4 partials per batch (+ residual + folded bias), LN2,
    router logits/softmax/top-2 (fp64), per-expert token gather.
  Launch B (expert-parallel): core e runs expert e's FFN
    gelu(x@W1+b1)@W2+b2 in bf16 (fp32 psum), gate-scaled on device, over
    its routed tokens, feature-major in and out (no device transposes).
  Host: scatter-add combine + residual.
"""

import numpy as np

H = 1024
NH = 16
DH = 64
L = 2
E = 8
FF = 4096
B = 2
S = 1024
EPS = 1e-5
P = 128
NCORES = 8
KO = H // P              # 8 contraction chunks over H
FB = 4                   # feature blocks of 128 (= head pairs) per core
QC = 2                   # query chunks of 512
KT = 8                   # key tiles of 128
MF = FF // P             # 32

_CACHE = {}
_PERF = {}


def _build_attn(wb=True):
    """Launch A program: one (batch, level, head-half) attention slice.
    wb=False elides the Q/K bias ones-row matmuls (all cores' folded
    biases are exactly zero for this input, decided by the host)."""
    import concourse.bacc as bacc
    import concourse.mybir as mybir
    import concourse.tile as tile

    F32, F32R = mybir.dt.float32, mybir.dt.float32r
    AF = mybir.ActivationFunctionType

    nc = bacc.Bacc()
    xn_h = nc.dram_tensor("xn", [P, KO, S], F32, kind="ExternalInput")   # LN1(x_b)^T tiled
    wq_h = nc.dram_tensor("wq", [FB, P, KO, P], F32, kind="ExternalInput")
    wk_h = nc.dram_tensor("wk", [FB, P, KO, P], F32, kind="ExternalInput")
    wv_h = nc.dram_tensor("wv", [P, KO, 512], F32, kind="ExternalInput")
    wo_h = nc.dram_tensor("wo", [P, FB, H], F32, kind="ExternalInput")
    bqk_h = nc.dram_tensor("bqk", [1, 1024], F32, kind="ExternalInput")  # bq | bk rows
    mb_h = nc.dram_tensor("mb", [P, KT], F32, kind="ExternalInput")      # key mask bias cols
    out_h = nc.dram_tensor("attnp", [H, S], F32, kind="ExternalOutput")

    with tile.TileContext(nc) as tc:
        with tc.tile_pool(name="consts", bufs=1) as consts, \
             tc.tile_pool(name="big", bufs=1) as big, \
             tc.tile_pool(name="wqk_s", bufs=2) as wqk_s, \
             tc.tile_pool(name="work", bufs=3) as work, \
             tc.tile_pool(name="outp", bufs=4) as outp, \
             tc.tile_pool(name="ps_mm", bufs=2, space="PSUM") as ps_mm, \
             tc.tile_pool(name="ps_sc", bufs=2, space="PSUM") as ps_sc, \
             tc.tile_pool(name="ps_cx", bufs=4, space="PSUM") as ps_cx:

            ones_f = consts.tile([1, 512], F32)
            nc.vector.memset(ones_f[:], 1.0)
            ones_row = consts.tile([1, 512], F32R)
            nc.vector.tensor_copy(ones_row[:], ones_f[:])

            bqk_sb = consts.tile([1, 1024], F32R)
            if wb:
                nc.sync.dma_start(bqk_sb[:], bqk_h[:].bitcast(F32R))
            mb_sb = consts.tile([P, KT], F32)
            nc.sync.dma_start(mb_sb[:], mb_h[:])

            # inputs, pre-tiled on the host: 1 descriptor per partition.
            # DMA issue order = first-use order (transfers share HBM bw):
            # first query-token half of xn, then wq0/wk0 so the head-pair-0
            # projections start ~10us in, with wv/xnB streaming behind.
            xn_t = big.tile([P, KO, S], F32R)
            nc.sync.dma_start(xn_t[:, :, 0:512], xn_h[:, :, 0:512].bitcast(F32R))
            wv_sb = big.tile([P, KO, 512], F32R)
            v_t = big.tile([P, KT, 8 * 65], F32R)       # per head: 64 cols V + 1 col ones

            # ---- interleaved per-head-pair: Q/K projection then attention ----
            # PE stays busy on the next pair's projections while the Act
            # engine works through this pair's exps; the normalize of block i
            # is emitted during block i+1 so its reciprocal never stalls PE.
            q_t = big.tile([P, FB, S], F32R)
            k_t = big.tile([P, FB, S], F32R)
            ctx_t = big.tile([P, FB, S], F32R)

            def proj_dma(w_h, fb, tag):
                w_fb = wqk_s.tile([P, KO, P], F32R, tag=tag, name=f"w_{tag}{fb}")
                nc.sync.dma_start(w_fb[:], w_h[fb].bitcast(F32R))
                return w_fb

            def proj_steps(dst, w_fb, bias_off, fb, qc):
                """One projection psum group as single-instruction steps, so
                it can be sprinkled into Act-bound attention sections."""
                box = {}

                def step(kc):
                    if kc == 0:
                        box["t"] = ps_mm.tile([P, 512], F32, tag="mm",
                                              name=f"qps{fb}_{qc}")
                    if kc < KO:
                        nc.tensor.matmul(
                            box["t"][:], w_fb[:, kc, :],
                            xn_t[:, kc, qc * 512:(qc + 1) * 512],
                            start=(kc == 0), stop=(kc == KO - 1 and not wb),
                        )
                    elif kc == KO and wb:
                        nc.tensor.matmul(
                            box["t"][:],
                            bqk_sb[:, bias_off + fb * P:bias_off + (fb + 1) * P],
                            ones_row[:], start=False, stop=True,
                        )
                    else:
                        nc.vector.tensor_copy(
                            dst[:, fb, qc * 512:(qc + 1) * 512], box["t"][:])

                ks = list(range(KO)) + ([KO] if wb else []) + [KO + 1]
                return [lambda k=k: step(k) for k in ks]

            def proj_fb(dst, w_h, bias_off, fb, tag):
                w_fb = proj_dma(w_h, fb, tag)
                for qc in range(QC):
                    for st in proj_steps(dst, w_fb, bias_off, fb, qc):
                        st()

            def normalize(fb, qc, cx):
                # 1/sumexp (psum row 64) broadcast to 64 partitions on the
                # otherwise-idle Pool engine, then scale ctx on DVE.
                for hh in range(2):
                    rcp = work.tile([1, 512], F32, tag="rcp")
                    nc.vector.reciprocal(rcp[:], cx[hh][64:65, :])
                    rb_sb = work.tile([64, 512], F32, tag="rb_sb")
                    nc.gpsimd.partition_broadcast(rb_sb[:], rcp[:])
                    nc.vector.tensor_mul(
                        ctx_t[hh * DH:(hh + 1) * DH, fb, qc * 512:(qc + 1) * 512],
                        cx[hh][0:64, :], rb_sb[:],
                    )

            wo_sb = big.tile([P, FB, H], F32R)

            def o_steps(ob, qc):
                # one O-projection psum group as steps (4 matmuls, copy, DMA)
                box = {}

                def step(i):
                    if i == 0:
                        box["t"] = ps_mm.tile([P, 512], F32, tag="mm",
                                              name=f"ops{ob}_{qc}")
                    if i < FB:
                        nc.tensor.matmul(
                            box["t"][:], wo_sb[:, i, ob * P:(ob + 1) * P],
                            ctx_t[:, i, qc * 512:(qc + 1) * 512],
                            start=(i == 0), stop=(i == FB - 1),
                        )
                    elif i == FB:
                        box["o"] = outp.tile([P, 512], F32, tag="o",
                                             name=f"oh{ob}_{qc}")
                        nc.vector.tensor_copy(box["o"][:], box["t"][:])
                    else:
                        nc.sync.dma_start(
                            out_h[:].rearrange("(ko p) t -> p ko t", p=P)[
                                :, ob, qc * 512:(qc + 1) * 512],
                            box["o"][:],
                        )

                return [lambda i=i: step(i) for i in range(FB + 2)]

            # Filler queue: PE work interleaved into the Act-bound attention
            # sections. Block (fb,qc) hides the next pair's Q/K projections;
            # the last pair's blocks hide the O projection of already-
            # normalized query chunks.
            def v_group(tt):
                # V projection for one key tile (token-major), ones col via memset
                vps = ps_mm.tile([P, 512], F32, tag="mm", name=f"vps{tt}")
                for kc in range(KO):
                    nc.tensor.matmul(
                        vps[:], xn_t[:, kc, tt * P:(tt + 1) * P], wv_sb[:, kc, :],
                        start=(kc == 0), stop=(kc == KO - 1),
                    )
                nc.vector.tensor_copy(
                    v4[:, tt, :, 0:64],
                    vps[:].rearrange("p (h c) -> p h c", c=64),
                )

            # Head-pair 0 queries (token half A) start as soon as xnA+wq0
            # land; wv/xnB stream behind them.  V key-tiles, the half-B
            # projections of pair 0, and everything else weave into the
            # first attention block just before each first use.
            fillers = []
            pending = None
            wq0 = proj_dma(wq_h, 0, "wq")
            wk0 = proj_dma(wk_h, 0, "wk")
            nc.sync.dma_start(wv_sb[:], wv_h[:].bitcast(F32R))
            nc.sync.dma_start(xn_t[:, :, 512:1024], xn_h[:, :, 512:1024].bitcast(F32R))
            for st in proj_steps(q_t, wq0, 0, 0, 0):
                st()
            for st in proj_steps(k_t, wk0, 512, 0, 0):
                st()
            v4 = v_t[:].rearrange("p a (h c) -> p a h c", c=65)
            nc.vector.memset(v4[:, :, :, 64:65].bitcast(F32), 1.0)

            last_w = {}
            for fb in range(FB):
                pops = 2
                if fb + 1 < FB:
                    n = fb + 1
                    wqf = proj_dma(wq_h, n, "wq")
                    wkf = proj_dma(wk_h, n, "wk")
                    if n < FB - 1:
                        fillers = [
                            st for qcx in range(QC)
                            for st in proj_steps(q_t, wqf, 0, n, qcx)
                        ] + [
                            st for qcx in range(QC)
                            for st in proj_steps(k_t, wkf, 512, n, qcx)
                        ]
                    else:
                        # only the half-A projections of the last pair here;
                        # its half-B work fills the pair's own first block
                        fillers = (
                            proj_steps(q_t, wqf, 0, n, 0)
                            + proj_steps(k_t, wkf, 512, n, 0)
                        )
                        last_w["q"], last_w["k"] = wqf, wkf
                else:
                    # scores kt>=4 of this pair need its half-B keys: pop 3
                    # per key-tile so that projection closes by kt 3
                    fillers = (
                        proj_steps(k_t, last_w["k"], 512, fb, 1)
                        + proj_steps(q_t, last_w["q"], 0, fb, 1)
                    )
                    pops = 3
                for qc in range(QC):
                    first = fb == 0 and qc == 0
                    cx0 = ps_cx.tile([65, 512], F32, tag="cx")
                    cx1 = ps_cx.tile([65, 512], F32, tag="cx")
                    cx = (cx0, cx1)
                    for kt in range(KT):
                        if first:
                            if kt == 4:
                                for st in proj_steps(k_t, wk0, 512, 0, 1):
                                    st()
                            v_group(kt)
                            if kt == 6:
                                for st in proj_steps(q_t, wq0, 0, 0, 1):
                                    st()
                        for hh in range(2):
                            sps = ps_sc.tile([P, 512], F32, tag="sc")
                            nc.tensor.matmul(
                                sps[:],
                                k_t[hh * DH:(hh + 1) * DH, fb, kt * P:(kt + 1) * P],
                                q_t[hh * DH:(hh + 1) * DH, fb, qc * 512:(qc + 1) * 512],
                                start=True, stop=True,
                            )
                            p_sb = work.tile([P, 512], F32R, tag="p")
                            nc.scalar.activation(
                                p_sb[:], sps[:], AF.Exp, bias=mb_sb[:, kt:kt + 1],
                            )
                            h = 2 * fb + hh
                            nc.tensor.matmul(
                                cx[hh][:],
                                v_t[:, kt, h * 65:(h + 1) * 65],
                                p_sb[:],
                                start=(kt == 0), stop=(kt == KT - 1),
                            )
                        if not first:
                            for _ in range(pops):
                                if fillers:
                                    fillers.pop(0)()
                    if pending is not None:
                        normalize(*pending)
                    pending = (fb, qc, cx)
                    if fb == FB - 1 and qc == 0:
                        # last pair: qc0 normalizes now so its O groups can
                        # fill qc1's attention section
                        normalize(*pending)
                        pending = None
                        fillers = [
                            st for ob in range(KO) for st in o_steps(ob, 0)
                        ]
                while fillers:
                    fillers.pop(0)()
                if fb == 0:
                    nc.sync.dma_start(wo_sb[:], wo_h[:].bitcast(F32R))
            normalize(*pending)

            # ---- remaining O projection (all of qc1) ----
            for ob in range(KO):
                for st in o_steps(ob, 1):
                    st()

    nc.finalize()
    return nc


def _build_expert_fp8(C, CN):
    """Launch B program, fp8 e4m3 DoubleRow variant: one expert FFN over C
    routed tokens, feature-major in/out.  Weights arrive pre-scaled by 64;
    the activation's scale=1/64 undoes it exactly.  Contraction runs 256
    deep per matmul (2 rows per partition, MatmulPerfMode.DoubleRow)."""
    import concourse.bacc as bacc
    import concourse.mybir as mybir
    import concourse.tile as tile

    F32, F32R, FP8 = mybir.dt.float32, mybir.dt.float32r, mybir.dt.float8e4
    AF = mybir.ActivationFunctionType
    DR = mybir.MatmulPerfMode.DoubleRow
    NCH = C // CN
    INV = 1.0 / 64.0

    nc = bacc.Bacc()
    xt_h = nc.dram_tensor("xt", [P, KO, C], FP8, kind="ExternalInput")   # LN2(x)^T tiled
    w1_h = nc.dram_tensor("w1", [MF, P, KO, P], FP8, kind="ExternalInput")
    w2_h = nc.dram_tensor("w2", [KO, P, MF, P], FP8, kind="ExternalInput")
    b1_h = nc.dram_tensor("b1c", [P, MF], F32, kind="ExternalInput")
    b2_h = nc.dram_tensor("b2c", [P, KO], F32, kind="ExternalInput")
    g_h = nc.dram_tensor("gates", [1, C], F32, kind="ExternalInput")
    y_h = nc.dram_tensor("y", [H, C], F32, kind="ExternalOutput")        # gated expert out^T

    with tile.TileContext(nc) as tc:
        with tc.tile_pool(name="consts", bufs=1) as consts, \
             tc.tile_pool(name="big", bufs=1) as big, \
             tc.tile_pool(name="w1s", bufs=4) as w1s, \
             tc.tile_pool(name="w2s", bufs=8) as w2s, \
             tc.tile_pool(name="work", bufs=2) as work, \
             tc.tile_pool(name="ps_mm", bufs=3, space="PSUM") as ps_mm, \
             tc.tile_pool(name="ps_gb", bufs=1, space="PSUM") as ps_gb:

            ones_f = consts.tile([1, P], F32)
            nc.vector.memset(ones_f[:], 1.0)
            ones_row = consts.tile([1, P], F32R)
            nc.vector.tensor_copy(ones_row[:], ones_f[:])

            x_t = big.tile([P, KO, C], FP8)
            nc.sync.dma_start(x_t[:], xt_h[:])
            xv = x_t[:].rearrange("p (dc i) t -> p dc i t", i=2)
            b1t = consts.tile([P, MF], F32)
            nc.sync.dma_start(b1t[:], b1_h[:])
            b2t = consts.tile([P, KO], F32)
            nc.sync.dma_start(b2t[:], b2_h[:])
            g_sb = consts.tile([1, C], F32R)
            nc.sync.dma_start(g_sb[:], g_h[:].bitcast(F32R))

            # ---- W1 pass + gelu (scale undoes the x64 weight prescale) ----
            h_t = big.tile([P, MF, C], FP8)
            for mf in range(MF):
                w1_mf = w1s.tile([P, KO, P], FP8, tag="w1")
                nc.sync.dma_start(w1_mf[:], w1_h[mf])
                wv1 = w1_mf[:].rearrange("p (dc i) m -> p dc i m", i=2)
                for nch in range(NCH):
                    hps = ps_mm.tile([P, CN], F32, tag="mm")
                    for dc in range(4):
                        nc.tensor.matmul(
                            hps[:], wv1[:, dc], xv[:, dc, :, nch * CN:(nch + 1) * CN],
                            start=(dc == 0), stop=(dc == 3), perf_mode=DR,
                        )
                    nc.scalar.activation(
                        h_t[:, mf, nch * CN:(nch + 1) * CN], hps[:],
                        AF.Gelu_apprx_tanh, bias=b1t[:, mf:mf + 1], scale=INV,
                    )

            # gate row broadcast to all partitions (needed from W2 phase on)
            gb_sb = big.tile([P, C], F32)
            for nch in range(NCH):
                gps = ps_gb.tile([P, CN], F32, tag="gb")
                nc.tensor.matmul(gps[:], ones_row[:], g_sb[:, nch * CN:(nch + 1) * CN],
                                 start=True, stop=True)
                nc.vector.tensor_copy(gb_sb[:, nch * CN:(nch + 1) * CN], gps[:])

            # ---- W2 pass + bias + gate ----
            hv = h_t[:].rearrange("p (dc i) t -> p dc i t", i=2)
            for oh in range(KO):
                w2_oh = w2s.tile([P, MF, P], FP8, tag="w2")
                nc.sync.dma_start(w2_oh[:], w2_h[oh])
                wv2 = w2_oh[:].rearrange("p (dc i) m -> p dc i m", i=2)
                y_sb = work.tile([P, C], F32, tag="y")
                for nch in range(NCH):
                    yps = ps_mm.tile([P, CN], F32, tag="mm")
                    for dc in range(MF // 2):
                        nc.tensor.matmul(
                            yps[:], wv2[:, dc], hv[:, dc, :, nch * CN:(nch + 1) * CN],
                            start=(dc == 0), stop=(dc == MF // 2 - 1), perf_mode=DR,
                        )
                    ytmp = work.tile([P, CN], F32, tag="ytmp")
                    nc.scalar.activation(ytmp[:], yps[:], AF.Identity,
                                         bias=b2t[:, oh:oh + 1], scale=INV)
                    nc.vector.tensor_mul(
                        y_sb[:, nch * CN:(nch + 1) * CN], ytmp[:],
                        gb_sb[:, nch * CN:(nch + 1) * CN],
                    )
                nc.sync.dma_start(
                    y_h[:].rearrange("(ko p) t -> p ko t", p=P)[:, oh, :], y_sb[:],
                )

    nc.finalize()
    return nc


def _build_attn_v2(qs=8.0):
    """Launch A v2: fp8-DoubleRow attention for the all-ones-mask / zero-bias
    fast path.  One (batch, level, head-half) slice = 8 heads per core.

    Layouts (dual-fp8 Ldweights needs its two DR rows exactly 128 cols apart):
      xn8v [P, 4dc, 8tt, 2i, 128]  fp8(16*LN1x)^T: V stationary + Q/K moving
      wq8/wk8 [4pb, P, KO, P]      pb=(hg,j): cols = head 4hg+c/32, feat 32j+c%32
      q8 [P, 2hg, 2j, S]           = qs*q   (DVE scalar-mul of proj psum)
      k8 [P, 2hg, 8kt, 2j, 128]    = k
      v8 [P, 8h, 4tp, 2j, 128]     = v cols 0:64, col 64 = 1.0 (sumexp), rest pad
      p8 [P, 2j, 512]              = exp(scores) per (h, qc, ktpair)
    Scores psum pairs (kt even/odd) land in one 2-bank [128, 1024] psum tile so
    a single Act instruction computes both exps (scale 1/qs folds the q scale).
    ctx DR([128,2j,65] x [128,2j,512]) accumulates [65,512]; row 64 = sumexp.
    Normalize (recip + Pool broadcast + DVE mul) -> ctx_t f32r; O-proj f32r."""
    import concourse.bacc as bacc
    import concourse.mybir as mybir
    import concourse.tile as tile

    F32, F32R, FP8 = mybir.dt.float32, mybir.dt.float32r, mybir.dt.float8e4
    AF = mybir.ActivationFunctionType
    DR = mybir.MatmulPerfMode.DoubleRow
    TP = KT // 2

    nc = bacc.Bacc()
    xn8_h = nc.dram_tensor("xn8v", [P, 4, KT, 2, P], FP8, kind="ExternalInput")
    wq8_h = nc.dram_tensor("wq8", [4, P, KO, P], FP8, kind="ExternalInput")
    wk8_h = nc.dram_tensor("wk8", [4, P, KO, P], FP8, kind="ExternalInput")
    wv8_h = nc.dram_tensor("wv8", [P, KO, 512], FP8, kind="ExternalInput")
    wo_h = nc.dram_tensor("wo", [P, FB, H], F32, kind="ExternalInput")
    out_h = nc.dram_tensor("attnp", [H, S], F32, kind="ExternalOutput")

    with tile.TileContext(nc) as tc:
        with tc.tile_pool(name="big", bufs=1) as big, \
             tc.tile_pool(name="p8p", bufs=4) as p8p, \
             tc.tile_pool(name="work", bufs=3) as work, \
             tc.tile_pool(name="outp", bufs=4) as outp, \
             tc.tile_pool(name="ps_mm", bufs=2, space="PSUM") as ps_mm, \
             tc.tile_pool(name="ps_sc", bufs=2, space="PSUM") as ps_sc, \
             tc.tile_pool(name="ps_cx", bufs=2, space="PSUM") as ps_cx:

            xn8 = big.tile([P, 4, KT, 2, P], FP8)
            q8 = big.tile([P, 2, 2, S], FP8)
            k8 = big.tile([P, 2, KT, 2, P], FP8)
            v8 = big.tile([P, 8, TP, 2, P], FP8)
            ctx_t = big.tile([P, FB, S], F32R)
            wo_sb = big.tile([P, FB, H], F32R)
            wv8_sb = big.tile([P, KO, 512], FP8)
            wq8_sb = big.tile([P, 4, KO, P], FP8)
            wk8_sb = big.tile([P, 4, KO, P], FP8)

            # DMA order = first-use order (transfers serialize on the bus)
            nc.sync.dma_start(xn8[:, :, 0:4], xn8_h[:, :, 0:4])
            nc.sync.dma_start(wq8_sb[:], wq8_h[:].rearrange("b p ko c -> p b ko c"))
            nc.sync.dma_start(wk8_sb[:], wk8_h[:].rearrange("b p ko c -> p b ko c"))
            nc.sync.dma_start(wv8_sb[:], wv8_h[:])
            nc.sync.dma_start(xn8[:, :, 4:8], xn8_h[:, :, 4:8])

            nc.vector.memset(v8[:, :, :, :, 64:65], 1.0)

            def qk_group(dst8, w_sb, hg, j, qc, scale):
                """One Q/K projection psum group -> fp8 dst, as steps.
                Each step covers one 128-token tile (4 DR matmuls over dc)."""
                pb = 2 * hg + j
                box = {}

                def step(st):
                    if st == 0:
                        box["t"] = ps_mm.tile([P, 512], F32, tag="mm",
                                              name=f"qkps{pb}_{qc}_{id(dst8) % 97}")
                    if st < 4:
                        tt = 4 * qc + st
                        wv = w_sb[:, pb].rearrange("p (dc i) m -> p dc i m", i=2)
                        for dc in range(4):
                            nc.tensor.matmul(
                                box["t"][:, 128 * st:128 * st + 128],
                                wv[:, dc], xn8[:, dc, tt],
                                start=(dc == 0), stop=(dc == 3), perf_mode=DR,
                            )
                    else:
                        if dst8 is q8:
                            dst = q8[:, hg, j, 512 * qc:512 * qc + 512]
                        else:
                            dst = k8[:, hg, 4 * qc:4 * qc + 4, j, :]
                        nc.vector.tensor_scalar_mul(dst, box["t"][:], scale)

                return [lambda d=d: step(d) for d in range(5)]

            def v_group(tt):
                """V projection for one key tile -> fp8 v8, as steps."""
                box = {}

                def step(dc):
                    if dc == 0:
                        box["t"] = ps_mm.tile([P, 512], F32, tag="mm",
                                              name=f"vps{tt}")
                    if dc < 4:
                        wvv = wv8_sb[:].rearrange("p (dc i) m -> p dc i m", i=2)
                        nc.tensor.matmul(
                            box["t"][:], xn8[:, dc, tt], wvv[:, dc],
                            start=(dc == 0), stop=(dc == 3), perf_mode=DR,
                        )
                    else:
                        nc.vector.tensor_scalar_mul(
                            v8[:, :, tt // 2, tt % 2, 0:64],
                            box["t"][:].rearrange("p (h c) -> p h c", c=64),
                            1.0 / 4096.0,
                        )

                return [lambda d=d: step(d) for d in range(5)]

            def o_steps(ob, qc):
                box = {}

                def step(i):
                    if i == 0:
                        box["t"] = ps_mm.tile([P, 512], F32, tag="mm",
                                              name=f"ops{ob}_{qc}")
                    if i < FB:
                        nc.tensor.matmul(
                            box["t"][:], wo_sb[:, i, ob * P:(ob + 1) * P],
                            ctx_t[:, i, qc * 512:(qc + 1) * 512],
                            start=(i == 0), stop=(i == FB - 1),
                        )
                    elif i == FB:
                        box["o"] = outp.tile([P, 512], F32, tag="o",
                                             name=f"oh{ob}_{qc}")
                        nc.vector.tensor_copy(box["o"][:], box["t"][:])
                    else:
                        nc.sync.dma_start(
                            out_h[:].rearrange("(ko p) t -> p ko t", p=P)[
                                :, ob, qc * 512:(qc + 1) * 512],
                            box["o"][:],
                        )

                return [lambda i=i: step(i) for i in range(FB + 2)]

            def normalize(h, qc, cx):
                rcp = work.tile([1, 512], F32, tag="rcp")
                nc.vector.reciprocal(rcp[:], cx[64:65, :])
                rb_sb = work.tile([64, 512], F32, tag="rb_sb")
                nc.gpsimd.partition_broadcast(rb_sb[:], rcp[:])
                nc.vector.tensor_mul(
                    ctx_t[(h % 2) * DH:(h % 2 + 1) * DH, h // 2,
                          qc * 512:(qc + 1) * 512],
                    cx[0:64, :], rb_sb[:],
                )

            # Bootstrap: q/k for heads 0-3 over the first token/key halves
            for st in qk_group(q8, wq8_sb, 0, 0, 0, 1.0 / 512.0):
                st()
            for st in qk_group(q8, wq8_sb, 0, 1, 0, 1.0 / 512.0):
                st()
            for st in qk_group(k8, wk8_sb, 0, 0, 0, 1.0 / 4096.0):
                st()
            for st in qk_group(k8, wk8_sb, 0, 1, 0, 1.0 / 4096.0):
                st()
            nc.sync.dma_start(wo_sb[:], wo_h[:].bitcast(F32R))

            # Named filler groups: popped for PE pacing during the Act-bound
            # attention stream, but force-drained via need() before any
            # consumer is emitted (emission order defines the dataflow).
            fillers = []                             # [name, step, step, ...]
            done = set()

            def add_group(name, steps):
                fillers.extend(steps)
                fillers.append(name)     # marker AFTER steps: done == emitted

            def pop_one():
                while fillers and isinstance(fillers[0], str):
                    done.add(fillers.pop(0))
                if fillers:
                    fillers.pop(0)()
                while fillers and isinstance(fillers[0], str):
                    done.add(fillers.pop(0))

            def need(*names):
                while any(n not in done for n in names):
                    assert fillers, f"missing groups: {names}"
                    pop_one()

            for j in range(2):                       # V first key half
                add_group(f"v{2*j}", v_group(2 * j))
                add_group(f"v{2*j+1}", v_group(2 * j + 1))
            for j in range(2):                       # keys half 2, heads 0-3
                add_group(f"k0{j}1", qk_group(k8, wk8_sb, 0, j, 1, 1.0 / 4096.0))
            for tt in range(4, 8):                   # V second key half
                add_group(f"v{tt}", v_group(tt))
            for j in range(2):                       # k heads 4-7
                for kc in range(2):
                    add_group(f"k1{j}{kc}",
                              qk_group(k8, wk8_sb, 1, j, kc, 1.0 / 4096.0))
            for j in range(2):                       # q heads 4-7 qc0
                add_group(f"q1{j}0", qk_group(q8, wq8_sb, 1, j, 0, 1.0 / 512.0))
            for hg in range(2):                      # q qc1 (all heads)
                for j in range(2):
                    add_group(f"q{hg}{j}1",
                              qk_group(q8, wq8_sb, hg, j, 1, 1.0 / 512.0))
            done.update(["q000", "q010", "k000", "k010"])   # bootstrap groups

            pending = None
            for qc in range(2):
                for h in range(8):
                    hg, hl = h // 4, h % 4
                    psl = slice(32 * hl, 32 * hl + 32)
                    need(f"q{hg}0{qc}", f"q{hg}1{qc}")
                    cx = ps_cx.tile([65, 512], F32, tag="cx")
                    for tp in range(TP):
                        need(f"k{hg}0{tp // 2}", f"k{hg}1{tp // 2}")
                        sps = ps_sc.tile([P, 1024], F32, tag="sc")
                        for j2 in range(2):
                            nc.tensor.matmul(
                                sps[:, 512 * j2:512 * j2 + 512],
                                k8[psl, hg, 2 * tp + j2, :, :],
                                q8[psl, hg, :, 512 * qc:512 * qc + 512],
                                start=True, stop=True, perf_mode=DR,
                                tile_position=(32 * hl, 0),
                            )
                        p8t = p8p.tile([P, 2, 512], FP8, tag="p8")
                        nc.scalar.activation(p8t[:], sps[:], AF.Exp,
                                             scale=1.0 / qs)
                        need(f"v{2*tp}", f"v{2*tp+1}")
                        nc.tensor.matmul(
                            cx[:], v8[:, h, tp, :, 0:65], p8t[:],
                            start=(tp == 0), stop=(tp == TP - 1), perf_mode=DR,
                        )
                        for _ in range(5):
                            pop_one()
                    if pending is not None:
                        normalize(*pending)
                    pending = (h, qc, cx)
                    if qc == 0 and h == 7:
                        # O-projection of qc0 fills qc1's attention stream
                        normalize(*pending)
                        pending = None
                        for ob in range(KO):
                            add_group(f"o{ob}0", o_steps(ob, 0))
            normalize(*pending)
            while fillers:
                pop_one()
            for ob in range(KO):                     # O-projection tail (qc1)
                for st in o_steps(ob, 1):
                    st()

    nc.finalize()
    return nc


def _build_attn_v3(inv_scale=1.0 / 4096.0):
    """Launch A v3: baseline f32r attention core + hi/lo fp8 DoubleRow
    projections (all-ones-mask / zero-bias fast path).

    Q/K/V projections run as 3 scale-matched fp8 passes (wh*xh, wl*xh,
    wh*xl; x encoded x16 hi and lo, w x256 hi and lo) accumulating in one
    psum: 25% fewer PE cycles than f32r and ~0.13% component error --
    small enough that router top-2 selections stay glued to the reference
    (logit noise ~1e-5 vs min top-2/3 gap ~1.5e-5 after the softmax
    contraction).  Scores, exp, ctx, normalize, and the O-projection are
    bit-identical to the f32r baseline.  x ships only in the DR-stationary
    layout [P, 4dc, 8tt, 2i, 128] (1MB/component vs 4MB f32): Q/K consume
    it as 128-token moving slices, V as stride-128 stationary."""
    import concourse.bacc as bacc
    import concourse.mybir as mybir
    import concourse.tile as tile

    F32, F32R, FP8 = mybir.dt.float32, mybir.dt.float32r, mybir.dt.float8e4
    AF = mybir.ActivationFunctionType
    DR = mybir.MatmulPerfMode.DoubleRow

    nc = bacc.Bacc()
    xh_h = nc.dram_tensor("xh8v", [P, 4, KT, 2, P], FP8, kind="ExternalInput")
    xl_h = nc.dram_tensor("xl8v", [P, 4, KT, 2, P], FP8, kind="ExternalInput")
    wq_h = nc.dram_tensor("wq8", [FB, 2, P, KO, P], FP8, kind="ExternalInput")
    wk_h = nc.dram_tensor("wk8", [FB, 2, P, KO, P], FP8, kind="ExternalInput")
    wv_h = nc.dram_tensor("wv8", [2, P, KO, 512], FP8, kind="ExternalInput")
    wo_h = nc.dram_tensor("wo", [P, FB, H], F32, kind="ExternalInput")
    out_h = nc.dram_tensor("attnp", [H, S], F32, kind="ExternalOutput")

    with tile.TileContext(nc) as tc:
        with tc.tile_pool(name="big", bufs=1) as big, \
             tc.tile_pool(name="wqk_s", bufs=2) as wqk_s, \
             tc.tile_pool(name="work", bufs=3) as work, \
             tc.tile_pool(name="outp", bufs=4) as outp, \
             tc.tile_pool(name="ps_mm", bufs=2, space="PSUM") as ps_mm, \
             tc.tile_pool(name="ps_sc", bufs=3, space="PSUM") as ps_sc, \
             tc.tile_pool(name="ps_cx", bufs=3, space="PSUM") as ps_cx:

            xh8 = big.tile([P, 4, KT, 2, P], FP8)
            xl8 = big.tile([P, 4, KT, 2, P], FP8)
            wv8_sb = big.tile([P, 2, KO, 512], FP8)
            q_t = big.tile([P, FB, S], F32R)
            k_t = big.tile([P, FB, S], F32R)
            ctx_t = big.tile([P, FB, S], F32R)
            v_t = big.tile([P, KT, 8 * 65], F32R)    # 64 cols V + 1 col ones
            wo_sb = big.tile([P, FB, H], F32R)

            # DMA order = first-use order
            def wqk_dma(w_h, fb, tag):
                w_fb = wqk_s.tile([P, 2, KO, P], FP8, tag=tag, name=f"w_{tag}{fb}")
                nc.sync.dma_start(w_fb[:], w_h[fb].rearrange("a p ko c -> p a ko c"))
                return w_fb

            wq0 = wqk_dma(wq_h, 0, "wq")
            wk0 = wqk_dma(wk_h, 0, "wk")
            nc.sync.dma_start(xh8[:, :, 0:4], xh_h[:, :, 0:4])
            nc.sync.dma_start(xl8[:, :, 0:4], xl_h[:, :, 0:4])
            nc.sync.dma_start(wv8_sb[:, 0], wv_h[0])
            nc.sync.dma_start(wv8_sb[:, 1], wv_h[1])
            nc.sync.dma_start(xh8[:, :, 4:8], xh_h[:, :, 4:8])
            nc.sync.dma_start(xl8[:, :, 4:8], xl_h[:, :, 4:8])

            v4 = v_t[:].rearrange("p a (h c) -> p a h c", c=65)
            nc.vector.memset(v4[:, :, :, 64:65].bitcast(F32), 1.0)

            # warm the Act exp table during the startup DMAs (first real
            # exp would otherwise pay the 1.3us table load mid-stream)
            warm = work.tile([1, 1], F32, tag="warm")
            nc.vector.memset(warm[:], 0.0)
            warm_o = work.tile([1, 1], F32, tag="warm_o")
            nc.scalar.activation(warm_o[:], warm[:], AF.Exp)

            def qk_steps(dst, w_fb, fb, qc):
                """One Q/K hi/lo projection psum group as steps: per token
                tile, 12 DR matmuls (3 passes x 4 dc) share one psum."""
                box = {}
                wv_ = w_fb[:].rearrange("p a (dc i) m -> p a dc i m", i=2)

                def step(st):
                    if st == 0:
                        box["t"] = ps_mm.tile([P, 512], F32, tag="mm",
                                              name=f"qkps{fb}_{qc}")
                    if st < 4:
                        tt = 4 * qc + st
                        for pa, (wa, xa) in enumerate(
                                ((0, xh8), (1, xh8), (0, xl8))):
                            for dc in range(4):
                                nc.tensor.matmul(
                                    box["t"][:, 128 * st:128 * st + 128],
                                    wv_[:, wa, dc], xa[:, dc, tt],
                                    start=(pa == 0 and dc == 0),
                                    stop=(pa == 2 and dc == 3), perf_mode=DR,
                                )
                    else:
                        nc.vector.tensor_scalar_mul(
                            dst[:, fb, qc * 512:(qc + 1) * 512], box["t"][:],
                            inv_scale)

                return [lambda d=d: step(d) for d in range(5)]

            def v_steps(tt):
                """V hi/lo projection for one key tile (token-major psum)."""
                box = {}
                wvv = wv8_sb[:].rearrange("p a (dc i) m -> p a dc i m", i=2)

                def step(st):
                    if st == 0:
                        box["t"] = ps_mm.tile([P, 512], F32, tag="mm",
                                              name=f"vps{tt}")
                    if st < 3:
                        xa = (xh8, xl8, xh8)[st]
                        wa = (0, 0, 1)[st]
                        for dc in range(4):
                            nc.tensor.matmul(
                                box["t"][:], xa[:, dc, tt], wvv[:, wa, dc],
                                start=(st == 0 and dc == 0),
                                stop=(st == 2 and dc == 3), perf_mode=DR,
                            )
                    else:
                        nc.vector.tensor_scalar_mul(
                            v4[:, tt, :, 0:64],
                            box["t"][:].rearrange("p (h c) -> p h c", c=64),
                            inv_scale)

                return [lambda d=d: step(d) for d in range(4)]

            def o_steps(ob, qc):
                box = {}

                def step(i):
                    if i == 0:
                        box["t"] = ps_mm.tile([P, 512], F32, tag="mm",
                                              name=f"ops{ob}_{qc}")
                    if i < FB:
                        nc.tensor.matmul(
                            box["t"][:], wo_sb[:, i, ob * P:(ob + 1) * P],
                            ctx_t[:, i, qc * 512:(qc + 1) * 512],
                            start=(i == 0), stop=(i == FB - 1),
                        )
                    elif i == FB:
                        box["o"] = outp.tile([P, 512], F32, tag="o",
                                             name=f"oh{ob}_{qc}")
                        if qc == 1 and ob % 2 == 0:
                            # tail: Act is idle and can read psum
                            nc.scalar.copy(box["o"][:], box["t"][:])
                        else:
                            nc.vector.tensor_copy(box["o"][:], box["t"][:])
                    else:
                        nc.sync.dma_start(
                            out_h[:].rearrange("(ko p) t -> p ko t", p=P)[
                                :, ob, qc * 512:(qc + 1) * 512],
                            box["o"][:],
                        )

                return [lambda i=i: step(i) for i in range(FB + 2)]

            def normalize(fb, qc, cx):
                for hh in range(2):
                    rcp = work.tile([1, 512], F32, tag="rcp")
                    nc.vector.reciprocal(rcp[:], cx[hh][64:65, :])
                    rb_sb = work.tile([64, 512], F32, tag="rb_sb")
                    nc.gpsimd.partition_broadcast(rb_sb[:], rcp[:])
                    nc.vector.tensor_mul(
                        ctx_t[hh * DH:(hh + 1) * DH, fb, qc * 512:(qc + 1) * 512],
                        cx[hh][0:64, :], rb_sb[:],
                    )

            # Named filler groups with forced prerequisite draining
            fillers = []
            done = set()

            def add_group(name, steps):
                fillers.extend(steps)
                fillers.append(name)

            def pop_one():
                while fillers and isinstance(fillers[0], str):
                    done.add(fillers.pop(0))
                if fillers:
                    fillers.pop(0)()
                while fillers and isinstance(fillers[0], str):
                    done.add(fillers.pop(0))

            def need(*names):
                while any(n not in done for n in names):
                    assert fillers, f"missing groups: {names}"
                    pop_one()

            # Bootstrap: pair-0 queries (first half) + keys (first half)
            for st in qk_steps(q_t, wq0, 0, 0):
                st()
            for st in qk_steps(k_t, wk0, 0, 0):
                st()
            done.update(["q00", "k00"])
            nc.sync.dma_start(wo_sb[:], wo_h[:].bitcast(F32R))

            add_group("k01", qk_steps(k_t, wk0, 0, 1))
            for tt in range(4):
                add_group(f"v{tt}", v_steps(tt))
            add_group("q01", qk_steps(q_t, wq0, 0, 1))
            for tt in range(4, 8):
                add_group(f"v{tt}", v_steps(tt))
            for fb in range(1, FB):
                wqf = wqk_dma(wq_h, fb, "wq")
                wkf = wqk_dma(wk_h, fb, "wk")
                add_group(f"k{fb}0", qk_steps(k_t, wkf, fb, 0))
                add_group(f"k{fb}1", qk_steps(k_t, wkf, fb, 1))
                add_group(f"q{fb}0", qk_steps(q_t, wqf, fb, 0))
                add_group(f"q{fb}1", qk_steps(q_t, wqf, fb, 1))

            pending = None
            for qc in range(QC):
                for fb in range(FB):
                    need(f"q{fb}{qc}", f"k{fb}0", f"k{fb}1")
                    cx0 = ps_cx.tile([65, 512], F32, tag="cx")
                    cx1 = ps_cx.tile([65, 512], F32, tag="cx")
                    cx = (cx0, cx1)
                    # ctx lags its scores/exp by one (kt,hh) unit so the PE
                    # queue never head-blocks waiting for the Act exp
                    ctx_q = []
                    for kt in range(KT):
                        need(f"v{kt}")
                        for hh in range(2):
                            sps = ps_sc.tile([P, 512], F32, tag="sc")
                            nc.tensor.matmul(
                                sps[:],
                                k_t[hh * DH:(hh + 1) * DH, fb, kt * P:(kt + 1) * P],
                                q_t[hh * DH:(hh + 1) * DH, fb, qc * 512:(qc + 1) * 512],
                                start=True, stop=True,
                            )
                            p_sb = work.tile([P, 512], F32R, tag="p",
                                             bufs=6)
                            nc.scalar.activation(p_sb[:], sps[:], AF.Exp)

                            def ctx_mm(kt=kt, hh=hh, p_sb=p_sb):
                                nc.tensor.matmul(
                                    cx[hh][:],
                                    v_t[:, kt, (2 * fb + hh) * 65:(2 * fb + hh + 1) * 65],
                                    p_sb[:],
                                    start=(kt == 0), stop=(kt == KT - 1),
                                )

                            ctx_q.append(ctx_mm)
                            if len(ctx_q) > 1:
                                ctx_q.pop(0)()
                            if (2 * kt + hh) % 4 != 3:
                                pop_one()
                    ctx_q.pop(0)()
                    if pending is not None:
                        normalize(*pending)
                    pending = (fb, qc, cx)
                    if qc == 0 and fb == FB - 1:
                        # O-projection of qc0 fills qc1's attention stream
                        normalize(*pending)
                        pending = None
                        for ob in range(KO):
                            add_group(f"o{ob}0", o_steps(ob, 0))
            normalize(*pending)
            while fillers:
                pop_one()
            for ob in range(KO):                     # O-projection tail (qc1)
                for st in o_steps(ob, 1):
                    st()

    nc.finalize()
    return nc


def _build_expert_v2(C, CN, w2_hilo=False):
    """Launch B v2: one expert FFN over C routed tokens, feature-major in/out.
    W1 pass: 3 fp8-e4m3 DoubleRow passes over host-split hi/lo of x and W1
    (the combine reads both psums directly).  h is stored as single fp8
    (scale 1: gelu output magnitudes sit in e4m3's normal range).  W2 pass:
    fp8 DoubleRow — single pass over W2*64 (w2_hilo=False) or two passes
    over hi/lo-split W2 for ~2x tighter output error."""
    import concourse.bacc as bacc
    import concourse.mybir as mybir
    import concourse.tile as tile

    F32, F32R = mybir.dt.float32, mybir.dt.float32r
    FP8 = mybir.dt.float8e4
    AF = mybir.ActivationFunctionType
    DR = mybir.MatmulPerfMode.DoubleRow
    NCH = C // CN

    nc = bacc.Bacc()
    xh_h = nc.dram_tensor("xh", [P, KO, C], FP8, kind="ExternalInput")   # LN2(x)^T hi
    xl_h = nc.dram_tensor("xl", [P, KO, C], FP8, kind="ExternalInput")   # LN2(x)^T lo
    w1_h = nc.dram_tensor("w1", [MF, P, 2, KO, P], FP8, kind="ExternalInput")
    NW2 = 2 if w2_hilo else 1
    w2_h = nc.dram_tensor("w2", [KO, P, NW2, MF, P], FP8, kind="ExternalInput")
    b1_h = nc.dram_tensor("b1c", [P, MF], F32, kind="ExternalInput")
    b2_h = nc.dram_tensor("b2c", [P, KO], F32, kind="ExternalInput")
    g_h = nc.dram_tensor("gates", [1, C], F32, kind="ExternalInput")
    BF16 = mybir.dt.bfloat16
    y_h = nc.dram_tensor("y", [H, C], BF16, kind="ExternalOutput")       # gated expert out^T

    with tile.TileContext(nc) as tc:
        with tc.tile_pool(name="consts", bufs=1) as consts, \
             tc.tile_pool(name="big", bufs=1) as big, \
             tc.tile_pool(name="w1s", bufs=4) as w1s, \
             tc.tile_pool(name="w2s", bufs=8) as w2s, \
             tc.tile_pool(name="work", bufs=2) as work, \
             tc.tile_pool(name="ps_mm", bufs=3, space="PSUM") as ps_mm, \
             tc.tile_pool(name="ps_gb", bufs=1, space="PSUM") as ps_gb:

            ones_f = consts.tile([1, P], F32)
            nc.vector.memset(ones_f[:], 1.0)
            ones_row = consts.tile([1, P], F32R)
            nc.vector.tensor_copy(ones_row[:], ones_f[:])

            # x first (hi then the first weight chunk then lo), then the
            # tiny bias/gate tensors (needed only once compute is rolling)
            x_hi = big.tile([P, KO, C], FP8)
            nc.sync.dma_start(x_hi[:], xh_h[:])
            w1_first = w1s.tile([P, 2, KO, P], FP8, tag="w1")
            nc.sync.dma_start(w1_first[:], w1_h[0])
            x_lo = big.tile([P, KO, C], FP8)
            nc.sync.dma_start(x_lo[:], xl_h[:])
            xhv = x_hi[:].rearrange("p (dc i) t -> p dc i t", i=2)
            xlv = x_lo[:].rearrange("p (dc i) t -> p dc i t", i=2)
            b1t = consts.tile([P, MF], F32)
            nc.sync.dma_start(b1t[:], b1_h[:])
            b2t = consts.tile([P, KO], F32)
            nc.sync.dma_start(b2t[:], b2_h[:])
            g_sb = consts.tile([1, C], F32R)
            nc.sync.dma_start(g_sb[:], g_h[:].bitcast(F32R))

            # ---- W1 pass (fp8 hi/lo, 3 DoubleRow passes) + gelu -> fp8 h ----
            # All hi/lo components share one psum scale (x: x16, w1: x1024 —
            # the lo parts are encoded at the same scale as the hi parts, which
            # fp8's floating format permits), so the 12 matmuls accumulate in a
            # single psum and gelu reads it directly: no DVE combine.
            # W2 chunks prefetch through the (compute-bound) W1 phase so the
            # W2 phase never waits on the DMA bus.
            h_t = big.tile([P, MF, C], FP8)
            w2_tiles = []
            for mf in range(MF):
                if mf == 0:
                    w1_mf = w1_first
                else:
                    w1_mf = w1s.tile([P, 2, KO, P], FP8, tag="w1")
                    nc.sync.dma_start(w1_mf[:], w1_h[mf])
                if mf % 4 == 3:
                    oh = mf // 4
                    w2t = w2s.tile([P, NW2, MF, P], FP8, tag="w2",
                                   name=f"w2c{oh}")
                    nc.sync.dma_start(w2t[:], w2_h[oh])
                    w2_tiles.append(w2t)
                w1v = w1_mf[:].rearrange("p a (dc i) m -> p a dc i m", i=2)
                for nch in range(NCH):
                    sl = slice(nch * CN, (nch + 1) * CN)
                    psa = ps_mm.tile([P, CN], F32, tag="mm")
                    for dc in range(4):
                        nc.tensor.matmul(
                            psa[:], w1v[:, 0, dc], xhv[:, dc, :, sl],
                            start=(dc == 0), stop=False, perf_mode=DR,
                        )
                    for dc in range(4):
                        nc.tensor.matmul(
                            psa[:], w1v[:, 1, dc], xhv[:, dc, :, sl],
                            start=False, stop=False, perf_mode=DR,
                        )
                    for dc in range(4):
                        nc.tensor.matmul(
                            psa[:], w1v[:, 0, dc], xlv[:, dc, :, sl],
                            start=False, stop=(dc == 3), perf_mode=DR,
                        )
                    nc.scalar.activation(
                        h_t[:, mf, sl], psa[:],
                        AF.Gelu_apprx_tanh, bias=b1t[:, mf:mf + 1],
                        scale=1.0 / 16384.0,
                    )

            # gate row broadcast to all partitions (needed from W2 phase on)
            gb_sb = big.tile([P, C], F32)
            for nch in range(NCH):
                gps = ps_gb.tile([P, CN], F32, tag="gb")
                nc.tensor.matmul(gps[:], ones_row[:], g_sb[:, nch * CN:(nch + 1) * CN],
                                 start=True, stop=True)
                nc.vector.tensor_copy(gb_sb[:, nch * CN:(nch + 1) * CN], gps[:])

            # ---- W2 pass (fp8 DoubleRow) + bias + gate ----
            hv = h_t[:].rearrange("p (dc i) t -> p dc i t", i=2)
            for oh in range(KO):
                w2_oh = w2_tiles[oh]
                wv2 = w2_oh[:].rearrange("p a (dc i) m -> p a dc i m", i=2)
                y_sb = work.tile([P, C], BF16, tag="y")
                for nch in range(NCH):
                    sl = slice(nch * CN, (nch + 1) * CN)
                    yps = ps_mm.tile([P, CN], F32, tag="mm")
                    for a in range(NW2):
                        for dc in range(MF // 2):
                            nc.tensor.matmul(
                                yps[:], wv2[:, a, dc], hv[:, dc, :, sl],
                                start=(a == 0 and dc == 0),
                                stop=(a == NW2 - 1 and dc == MF // 2 - 1),
                                perf_mode=DR,
                            )
                    ytmp = work.tile([P, CN], F32, tag="ytmp")
                    nc.scalar.activation(ytmp[:], yps[:], AF.Identity,
                                         bias=b2t[:, oh:oh + 1], scale=1.0 / 64.0)
                    nc.vector.tensor_mul(
                        y_sb[:, sl], ytmp[:], gb_sb[:, sl],
                    )
                    nc.sync.dma_start(
                        y_h[:].rearrange("(ko p) t -> p ko t", p=P)[:, oh, sl],
                        y_sb[:, sl],
                    )

    nc.finalize()
    return nc


def _build_expert(C, CN):
    """Launch B program: one expert FFN over C routed tokens, feature-major
    in/out.  The W1 pass runs as 3 fp8-e4m3 DoubleRow passes over host-split
    hi/lo components of x and W1 (x: x16 / x256, W1: x1024 / x16384; the two
    cross products share psum scale 2^18, hi*hi is 2^14) — more accurate than
    bf16 and 25% fewer PE cycles.  h and the W2 pass stay bf16."""
    import concourse.bacc as bacc
    import concourse.mybir as mybir
    import concourse.tile as tile

    F32, F32R, BF16 = mybir.dt.float32, mybir.dt.float32r, mybir.dt.bfloat16
    FP8 = mybir.dt.float8e4
    AF = mybir.ActivationFunctionType
    DR = mybir.MatmulPerfMode.DoubleRow
    NCH = C // CN

    nc = bacc.Bacc()
    xh_h = nc.dram_tensor("xh", [P, KO, C], FP8, kind="ExternalInput")   # LN2(x)^T hi
    xl_h = nc.dram_tensor("xl", [P, KO, C], FP8, kind="ExternalInput")   # LN2(x)^T lo
    w1_h = nc.dram_tensor("w1", [MF, P, 2, KO, P], FP8, kind="ExternalInput")
    w2_h = nc.dram_tensor("w2", [KO, P, MF, P], BF16, kind="ExternalInput")
    b1_h = nc.dram_tensor("b1c", [P, MF], F32, kind="ExternalInput")
    b2_h = nc.dram_tensor("b2c", [P, KO], F32, kind="ExternalInput")
    g_h = nc.dram_tensor("gates", [1, C], F32, kind="ExternalInput")
    y_h = nc.dram_tensor("y", [H, C], F32, kind="ExternalOutput")        # gated expert out^T

    with tile.TileContext(nc) as tc:
        with tc.tile_pool(name="consts", bufs=1) as consts, \
             tc.tile_pool(name="big", bufs=1) as big, \
             tc.tile_pool(name="w1s", bufs=4) as w1s, \
             tc.tile_pool(name="w2s", bufs=8) as w2s, \
             tc.tile_pool(name="work", bufs=2) as work, \
             tc.tile_pool(name="ps_mm", bufs=3, space="PSUM") as ps_mm, \
             tc.tile_pool(name="ps_gb", bufs=1, space="PSUM") as ps_gb:

            ones_f = consts.tile([1, P], F32)
            nc.vector.memset(ones_f[:], 1.0)
            ones_row = consts.tile([1, P], F32R)
            nc.vector.tensor_copy(ones_row[:], ones_f[:])

            # x first (hi then the first weight chunk then lo), then the
            # tiny bias/gate tensors (needed only once compute is rolling)
            x_hi = big.tile([P, KO, C], FP8)
            nc.sync.dma_start(x_hi[:], xh_h[:])
            w1_first = w1s.tile([P, 2, KO, P], FP8, tag="w1")
            nc.sync.dma_start(w1_first[:], w1_h[0])
            x_lo = big.tile([P, KO, C], FP8)
            nc.sync.dma_start(x_lo[:], xl_h[:])
            xhv = x_hi[:].rearrange("p (dc i) t -> p dc i t", i=2)
            xlv = x_lo[:].rearrange("p (dc i) t -> p dc i t", i=2)
            b1t = consts.tile([P, MF], F32)
            nc.sync.dma_start(b1t[:], b1_h[:])
            b2t = consts.tile([P, KO], F32)
            nc.sync.dma_start(b2t[:], b2_h[:])
            g_sb = consts.tile([1, C], F32R)
            nc.sync.dma_start(g_sb[:], g_h[:].bitcast(F32R))

            # ---- W1 pass (fp8 hi/lo, 3 DoubleRow passes) + gelu ----
            h_t = big.tile([P, MF, C], BF16)
            for mf in range(MF):
                if mf == 0:
                    w1_mf = w1_first
                else:
                    w1_mf = w1s.tile([P, 2, KO, P], FP8, tag="w1")
                    nc.sync.dma_start(w1_mf[:], w1_h[mf])
                w1v = w1_mf[:].rearrange("p a (dc i) m -> p a dc i m", i=2)
                for nch in range(NCH):
                    sl = slice(nch * CN, (nch + 1) * CN)
                    psa = ps_mm.tile([P, CN], F32, tag="mmA", bufs=2)
                    for dc in range(4):
                        nc.tensor.matmul(
                            psa[:], w1v[:, 0, dc], xhv[:, dc, :, sl],
                            start=(dc == 0), stop=(dc == 3), perf_mode=DR,
                        )
                    psb = ps_mm.tile([P, CN], F32, tag="mmB", bufs=2)
                    for dc in range(4):
                        nc.tensor.matmul(
                            psb[:], w1v[:, 1, dc], xhv[:, dc, :, sl],
                            start=(dc == 0), stop=False, perf_mode=DR,
                        )
                    for dc in range(4):
                        nc.tensor.matmul(
                            psb[:], w1v[:, 0, dc], xlv[:, dc, :, sl],
                            start=False, stop=(dc == 3), perf_mode=DR,
                        )
                    psa_sb = work.tile([P, CN], F32, tag="psa_sb")
                    nc.vector.tensor_copy(psa_sb[:], psa[:])
                    cmb = work.tile([P, CN], F32, tag="cmb")
                    nc.vector.scalar_tensor_tensor(
                        cmb[:], psb[:], 1.0 / 16.0, psa_sb[:],
                        mybir.AluOpType.mult, mybir.AluOpType.add,
                    )
                    nc.scalar.activation(
                        h_t[:, mf, sl], cmb[:],
                        AF.Gelu_apprx_tanh, bias=b1t[:, mf:mf + 1],
                        scale=1.0 / 16384.0,
                    )

            # gate row broadcast to all partitions (needed from W2 phase on)
            gb_sb = big.tile([P, C], F32)
            for nch in range(NCH):
                gps = ps_gb.tile([P, CN], F32, tag="gb")
                nc.tensor.matmul(gps[:], ones_row[:], g_sb[:, nch * CN:(nch + 1) * CN],
                                 start=True, stop=True)
                nc.vector.tensor_copy(gb_sb[:, nch * CN:(nch + 1) * CN], gps[:])

            # ---- W2 pass + bias + gate ----
            for oh in range(KO):
                w2_oh = w2s.tile([P, MF, P], BF16, tag="w2")
                nc.sync.dma_start(w2_oh[:], w2_h[oh])
                y_sb = work.tile([P, C], F32, tag="y")
                for nch in range(NCH):
                    yps = ps_mm.tile([P, CN], F32, tag="mm")
                    for kc2 in range(MF):
                        nc.tensor.matmul(
                            yps[:], w2_oh[:, kc2, :], h_t[:, kc2, nch * CN:(nch + 1) * CN],
                            start=(kc2 == 0), stop=(kc2 == MF - 1),
                        )
                    ytmp = work.tile([P, CN], F32, tag="ytmp")
                    nc.scalar.activation(ytmp[:], yps[:], AF.Identity, bias=b2t[:, oh:oh + 1])
                    nc.vector.tensor_mul(
                        y_sb[:, nch * CN:(nch + 1) * CN], ytmp[:],
                        gb_sb[:, nch * CN:(nch + 1) * CN],
                    )
                    nc.sync.dma_start(
                        y_h[:].rearrange("(ko p) t -> p ko t", p=P)[
                            :, oh, nch * CN:(nch + 1) * CN],
                        y_sb[:, nch * CN:(nch + 1) * CN],
                    )

    nc.finalize()
    return nc


def _get_attn(wb=True):
    key = ("attn", wb)
    if key not in _CACHE:
        _CACHE[key] = _build_attn(wb)
    return _CACHE[key]


def _get_attn_v3(inv_scale):
    key = ("attn_v3", inv_scale)
    if key not in _CACHE:
        _CACHE[key] = _build_attn_v3(inv_scale)
    return _CACHE[key]


def _get_expert(C, CN, mode):
    key = ("exp", C, CN, mode)
    if key not in _CACHE:
        if mode == "fp8":
            _CACHE[key] = _build_expert_fp8(C, CN)
        elif mode == "hilo":
            _CACHE[key] = _build_expert(C, CN)
        else:
            _CACHE[key] = _build_expert_v2(C, CN, w2_hilo=(mode == "v2hilo"))
    return _CACHE[key]


def _ln(x64):
    m = x64.mean(-1, keepdims=True)
    v = x64.var(-1, keepdims=True)
    return (x64 - m) / np.sqrt(v + EPS)


def _bf16(a):
    import ml_dtypes
    return np.ascontiguousarray(np.asarray(a).astype(ml_dtypes.bfloat16))


def _fp8(a):
    import ml_dtypes
    return np.ascontiguousarray(np.asarray(a).astype(ml_dtypes.float8_e4m3))


def _pko(a2d, x):
    """[H-like, X] row-major -> [P, n, X] SBUF tile layout (casts to f32)."""
    n = a2d.shape[0] // P
    return np.ascontiguousarray(
        np.asarray(a2d, dtype=np.float32).reshape(n, P, x).transpose(1, 0, 2))


def _pkod(a2d, x):
    """Same as _pko but dtype-preserving."""
    a = np.asarray(a2d)
    n = a.shape[0] // P
    return np.ascontiguousarray(a.reshape(n, P, x).transpose(1, 0, 2))


def kernel(**inputs):
    import os as _os
    import time as _time
    from concourse.bass_utils import run_bass_kernel_spmd

    f = lambda k: np.asarray(inputs[k], dtype=np.float32)
    x = f("hidden_states")                       # [B, S, H]
    mask = np.asarray(inputs["attention_mask"])  # [B, S] int32
    ln1_g, ln1_b = f("ln1_g").astype(np.float64), f("ln1_b").astype(np.float64)
    ln2_g, ln2_b = f("ln2_g").astype(np.float64), f("ln2_b").astype(np.float64)
    Wq, Wk, Wv, Wo = (f(k).astype(np.float64) for k in ("Wq", "Wk", "Wv", "Wo"))
    bq, bk, bv, bo = (f(k).astype(np.float64) for k in ("bq", "bk", "bv", "bo"))
    level_logits = f("level_logits").astype(np.float64)
    Wr, br = f("Wr").astype(np.float64), f("br").astype(np.float64)
    W1, b1 = f("W1").astype(np.float64), f("b1").astype(np.float64)
    W2, b2 = f("W2").astype(np.float64), f("b2").astype(np.float64)

    # ---- host folding ----
    scale = 1.0 / np.sqrt(DH)
    wq_eff = (ln1_g[None, :, None] * Wq) * scale              # [L,H,H]
    bq_eff = (bq + ln1_b @ Wq) * scale                        # [L,H]
    wk_eff = ln1_g[None, :, None] * Wk
    bk_eff = bk + ln1_b @ Wk
    wv_eff = ln1_g[None, :, None] * Wv
    bv_eff = bv + ln1_b @ Wv                                  # folded into boc below
    lw = np.exp(level_logits - level_logits.max())
    lw = lw / lw.sum()                                        # softmax(level_logits)
    wo_eff = lw[:, None, None] * Wo
    boc_eff = np.einsum("l,lh->h", lw, bo) + np.einsum("lf,lfh->h", bv_eff, wo_eff)

    xn1 = _ln(x.astype(np.float64)).astype(np.float32)        # LN1 (gamma/beta folded)

    def colt(vec):  # [H or F] -> [P, n] per-partition column layout
        v32 = np.ascontiguousarray(np.asarray(vec, dtype=np.float32))
        return np.ascontiguousarray(v32.reshape(-1, P).T)

    mbias = ((1.0 - mask.astype(np.float32)) * np.float32(-1e9))  # [B,S]
    xn1_T = np.swapaxes(xn1, 1, 2)                            # [B,H,S]

    wb = max(float(np.abs(np.concatenate([bq_eff, bk_eff], -1)).max()),
             0.0) > 0.0
    mask_ones = bool((mask == 1).all())
    attn_mode = _os.environ.get(
        "KERNEL_ATTN_MODE", "v3" if (mask_ones and not wb) else "v1")
    _PERF["attn_mode"] = attn_mode

    if attn_mode == "v3":
        xsc = np.float32(_os.environ.get("KERNEL_ATTN_XS", "16"))
        wsc = np.float32(_os.environ.get("KERNEL_ATTN_WS", "256"))
        _PERF["attn_scales"] = (float(xsc), float(wsc))

        def hilo(a32, scale):
            hi = _fp8(a32 * scale)
            lo = _fp8((a32 - hi.astype(np.float32) / scale) * scale)
            return hi, lo

        def xtile(x8):            # [H, S] fp8 -> [P, 4dc, KT, 2i, P]
            return np.ascontiguousarray(
                x8.reshape(4, 2, P, KT, P).transpose(2, 0, 3, 1, 4))

        def wtile(w8):            # [H, 512] fp8 -> [FB-chunks of [P, KO, P]]
            return [_pkod(np.ascontiguousarray(w8[:, fb * P:(fb + 1) * P]), P)
                    for fb in range(FB)]

        in_maps = []
        for c in range(NCORES):
            b, l, hh = c >> 2, (c >> 1) & 1, c & 1
            sl = slice(hh * 512, (hh + 1) * 512)
            xh8, xl8 = hilo(xn1_T[b].astype(np.float32), xsc)
            wqh, wql = hilo(wq_eff[l][:, sl].astype(np.float32), wsc)
            wkh, wkl = hilo(wk_eff[l][:, sl].astype(np.float32), wsc)
            wvh, wvl = hilo(wv_eff[l][:, sl].astype(np.float32), wsc)
            in_maps.append({
                "xh8v": xtile(xh8),
                "xl8v": xtile(xl8),
                "wq8": np.ascontiguousarray(
                    np.stack([np.stack(wtile(w)) for w in (wqh, wql)], axis=1)),
                "wk8": np.ascontiguousarray(
                    np.stack([np.stack(wtile(w)) for w in (wkh, wkl)], axis=1)),
                "wv8": np.ascontiguousarray(
                    np.stack([_pkod(w, 512) for w in (wvh, wvl)])),
                "wo": _pko(wo_eff[l][sl, :].astype(np.float32), H),
            })
        nc_a = _get_attn_v3(1.0 / float(xsc * wsc))
    else:
        in_maps = []
        for c in range(NCORES):
            b, l, hh = c >> 2, (c >> 1) & 1, c & 1
            sl = slice(hh * 512, (hh + 1) * 512)
            wq32 = wq_eff[l][:, sl].astype(np.float32)        # [H,512]
            wk32 = wk_eff[l][:, sl].astype(np.float32)
            in_maps.append({
                "xn": _pko(xn1_T[b], S),
                "wq": np.ascontiguousarray(
                    _pko(wq32, 512).reshape(P, KO, FB, P).transpose(2, 0, 1, 3)),
                "wk": np.ascontiguousarray(
                    _pko(wk32, 512).reshape(P, KO, FB, P).transpose(2, 0, 1, 3)),
                "wv": _pko(wv_eff[l][:, sl].astype(np.float32), 512),
                "wo": _pko(wo_eff[l][sl, :].astype(np.float32), H),
                "bqk": np.concatenate([bq_eff[l][sl], bk_eff[l][sl]]).astype(np.float32)[None, :],
                "mb": colt(mbias[b]),
            })
        nc_a = _get_attn(wb)
    t0 = _time.time()
    res_a = run_bass_kernel_spmd(nc_a, in_maps, core_ids=list(range(NCORES)))
    _PERF["a_wall_s"] = _time.time() - t0
    _PERF["attn_wb"] = wb
    _PERF["a_exec_ns"] = res_a.exec_time_ns

    # ---- host: combine partials, residual, LN2, router, top-2 routing ----
    xres = x.astype(np.float64)                                # [B,S,H]
    for c in range(NCORES):
        b = c >> 2
        xres[b] += res_a.results[c]["attnp"].astype(np.float64).T
    xres += boc_eff[None, None, :]
    xres = xres.reshape(B * S, H)

    xn2 = _ln(xres)                                           # [B*S, H] (gamma/beta folded)
    logits = xn2 @ (ln2_g[:, None] * Wr) + (br + ln2_b @ Wr)  # [B*S, E]
    pm = logits.max(-1, keepdims=True)
    probs = np.exp(logits - pm)
    probs /= probs.sum(-1, keepdims=True)
    order = np.argsort(-probs, axis=-1, kind="stable")
    topi = order[:, :2]                                       # [T,2]
    topv = np.take_along_axis(probs, topi, axis=-1)
    gates = topv / topv.sum(-1, keepdims=True)                # [T,2]

    ps = np.sort(probs, axis=-1)
    _PERF["router_gap23"] = float((ps[:, -2] - ps[:, -3]).min())
    _PERF["topi"] = topi.copy()

    tok_idx, gate_val = [], []
    for e in range(E):
        sel = np.nonzero(topi == e)
        tok_idx.append(sel[0])
        gate_val.append(gates[sel[0], sel[1]])
    counts = [len(t) for t in tok_idx]
    C = max(512, ((max(counts) + 3) // 4) * 4)
    while True:  # need NCH with C % NCH == 0 and 256 <= C/NCH <= 512
        nch = (C + 511) // 512
        if C % nch == 0 and C // nch >= 256:
            break
        C += 4
    CN = C // ((C + 511) // 512)

    w1f = ln2_g[None, :, None] * W1                           # [E,H,F]
    b1f = b1 + ln2_b @ W1                                     # [E,F]
    xn2_T32 = np.ascontiguousarray(xn2.T.astype(np.float32))  # [H, B*S]

    mode = _os.environ.get("KERNEL_MOE_MODE", "v2")
    if mode != "fp8":
        # hi/lo fp8 split of the LN2 output for the W1 pass (done once).
        # v2 modes encode lo at the SAME scale as hi (x16) so the hi and lo
        # passes accumulate in one psum; the old "hilo" kernel wants x256.
        xh_full = _fp8(xn2_T32 * np.float32(16.0))
        xlo_scale = np.float32(256.0) if mode == "hilo" else np.float32(16.0)
        xl_full = _fp8(
            (xn2_T32 - xh_full.astype(np.float32) / np.float32(16.0)) * xlo_scale)
    in_maps_b = []
    for e in range(E):
        g = np.zeros((1, C), np.float32)
        g[0, :counts[e]] = gate_val[e].astype(np.float32)
        if mode == "fp8":
            xt = np.zeros((H, C), np.float32)
            xt[:, :counts[e]] = xn2_T32[:, tok_idx[e]]
            w1_32 = (w1f[e] * 64.0).astype(np.float32)        # [H,FF]
            w2_32 = (W2[e] * 64.0).astype(np.float32)         # [FF,H]
            in_maps_b.append({
                "xt": _fp8(_pko(xt, C)),
                "w1": _fp8(_pko(w1_32, FF).reshape(P, KO, MF, P).transpose(2, 0, 1, 3)),
                "w2": _fp8(_pko(w2_32, H).reshape(P, MF, KO, P).transpose(2, 0, 1, 3)),
                "b1c": colt(b1f[e]), "b2c": colt(b2[e]), "gates": g,
            })
            continue
        xh = np.zeros((H, C), xh_full.dtype)
        xh[:, :counts[e]] = xh_full[:, tok_idx[e]]
        xl = np.zeros((H, C), xl_full.dtype)
        xl[:, :counts[e]] = xl_full[:, tok_idx[e]]
        w1_32 = w1f[e].astype(np.float32)                     # [H,FF]
        w1h = _fp8(w1_32 * np.float32(1024.0))
        w1lo_scale = np.float32(16384.0) if mode == "hilo" else np.float32(1024.0)
        w1l = _fp8((w1_32 - w1h.astype(np.float32) / np.float32(1024.0))
                   * w1lo_scale)
        w1h_t = _pkod(w1h, FF).reshape(P, KO, MF, P).transpose(2, 0, 1, 3)
        w1l_t = _pkod(w1l, FF).reshape(P, KO, MF, P).transpose(2, 0, 1, 3)
        imap = {
            "xh": _pkod(xh, C),
            "xl": _pkod(xl, C),
            "w1": np.ascontiguousarray(np.stack([w1h_t, w1l_t], axis=2)),
            "b1c": colt(b1f[e]),
            "b2c": colt(b2[e]),
            "gates": g,
        }
        if mode == "hilo":
            imap["w2"] = _bf16(_pko(W2[e].astype(np.float32), H)
                               .reshape(P, MF, KO, P).transpose(2, 0, 1, 3))
        else:
            # fp8 W2, prescaled x64 (undone by the output activation scale)
            w2_32 = W2[e].astype(np.float32)                  # [FF,H]
            w2h = _fp8(w2_32 * np.float32(64.0))
            parts = [_pkod(w2h, H).reshape(P, MF, KO, P).transpose(2, 0, 1, 3)]
            if mode == "v2hilo":
                # lo at the same x64 scale as hi: single-psum accumulation
                w2l = _fp8((w2_32 - w2h.astype(np.float32) / np.float32(64.0))
                           * np.float32(64.0))
                parts.append(_pkod(w2l, H).reshape(P, MF, KO, P).transpose(2, 0, 1, 3))
            imap["w2"] = np.ascontiguousarray(np.stack(parts, axis=2))
        in_maps_b.append(imap)

    nc_b = _get_expert(C, CN, mode)
    t0 = _time.time()
    res_b = run_bass_kernel_spmd(nc_b, in_maps_b, core_ids=list(range(NCORES)))
    _PERF["b_wall_s"] = _time.time() - t0
    _PERF["b_exec_ns"] = res_b.exec_time_ns
    _PERF["capacity"] = C
    _PERF["counts"] = counts
    _PERF["moe_mode"] = mode

    if _os.environ.get("KERNEL_STASH"):
        _PERF["a_prog"] = (nc_a, in_maps)
        _PERF["b_prog"] = (nc_b, in_maps_b)

    out = xres.copy()
    for e in range(E):
        if counts[e]:
            out[tok_idx[e]] += res_b.results[e]["y"][:, :counts[e]].astype(np.float64).T
    return out.reshape(B, S, H).astype(np.float32)

